# revision 1
# baseline (speedup 1.0000x reference)
"""MedianPool2d (3x3, stride 1, zero-pad 1) Trainium2 Bass kernel.

Full input x: (8, 64, 256, 256) fp32.  Sharding: pure data parallel over
batch -> core i processes x[i] (64, 256, 256).

Per-core layout: 128 SBUF partitions = (h, c) with p = h*64 + c, where
h in {0,1} picks the top/bottom 128-row half of the image and c the
channel.  Each partition processes a strip of HH=128 rows x 256 cols,
with a 1-row halo on each side (zero at the image border, neighbor rows
at the half boundary - both come in via DMA / memset).  Rows are stored
padded to WP=258 with zero columns at 0 and 257, so every tap of the
3x3 window is a pure free-dim offset.

Median of 9 = med3(max3(column mins), med3(column medians),
                   min3(column maxes))  -- exact, 15 min/max passes/pixel
with vertical row-pair sharing and horizontal even/odd pair sharing.
All elementwise work runs on the DVE (this toolchain rejects
TensorTensor on GPSIMD and CCE min/max accum on DMA); DMA is fully
overlapped by the Tile scheduler, and merge/final temporaries alias the
slots of dead earlier-stage buffers so R=16 chunks fit in SBUF.
"""

import numpy as np

B, C, H, W = 8, 64, 256, 256
NCORES = 8
HH = H // 2          # rows per half-strip
WP = W + 2           # padded row width

_CACHE = {}


def _build(R=8, gp_rows=0):
    """Build the Bass module for one core: x (64,256,256) f32 -> out same.

    gp_rows: number of output rows (of each chunk's R) computed on GPSIMD
    instead of the DVE.
    """
    import concourse.bacc as bacc
    import concourse.mybir as mybir
    from concourse.tile import TileContext

    MIN = mybir.AluOpType.min
    MAX = mybir.AluOpType.max
    f32 = mybir.dt.float32

    assert HH % R == 0
    assert 0 <= gp_rows < R
    K = HH // R                     # chunks per strip

    nc = bacc.Bacc("TRN2", name="median_pool2d")
    x = nc.dram_tensor("x", [C, H, W], f32, kind="ExternalInput")
    out = nc.dram_tensor("out", [C, H, W], f32, kind="ExternalOutput")

    xg = x.ap()                     # global view [c, 256, 256]
    og = out.ap()

    def tt(out_ap, in0, in1, op):
        """Elementwise tensor_tensor, row-split DVE/GPSIMD.

        All APs are [128, rows, width]; the row dim is axis 1.
        """
        rows = out_ap.shape[1]
        split = rows - gp_rows if rows > gp_rows else rows
        nc.vector.tensor_tensor(
            out=out_ap[:, 0:split], in0=in0[:, 0:split], in1=in1[:, 0:split],
            op=op,
        )
        if split < rows:
            nc.gpsimd.tensor_tensor(
                out=out_ap[:, split:rows], in0=in0[:, split:rows],
                in1=in1[:, split:rows], op=op,
            )

    with TileContext(nc) as tc:
        with (
            tc.tile_pool(name="io_in", bufs=3) as in_pool,
            tc.tile_pool(name="io_out", bufs=3) as out_pool,
            tc.tile_pool(name="vert", bufs=1) as v_pool,
            tc.tile_pool(name="merge", bufs=1) as m_pool,
        ):
            for k in range(K):
                r0 = k * R                      # first output row (half-local)
                # ---- load input chunk: rows r0-1 .. r0+R (R+2 rows) ----
                it = in_pool.tile([128, (R + 2) * WP], f32, name="it", tag="it")
                it3 = it.rearrange("p (r w) -> p r w", w=WP)
                # zero pad columns 0 and 257 for all rows
                nc.vector.memset(it3[:, :, 0:WP:WP - 1], 0.0)
                # top half: global rows r0-1 .. r0+R+1 (clip at k==0)
                if k == 0:
                    nc.vector.memset(it3[0:64, 0:1, 1:W + 1], 0.0)
                    nc.sync.dma_start(
                        out=it3[0:64, 1:R + 2, 1:W + 1],
                        in_=xg[:, 0:R + 1, :],
                    )
                else:
                    nc.sync.dma_start(
                        out=it3[0:64, :, 1:W + 1],
                        in_=xg[:, r0 - 1:r0 + R + 1, :],
                    )
                # bottom half: global rows HH+r0-1 .. HH+r0+R+1 (clip at last)
                if k == K - 1:
                    nc.vector.memset(it3[64:128, R + 1:R + 2, 1:W + 1], 0.0)
                    nc.sync.dma_start(
                        out=it3[64:128, 0:R + 1, 1:W + 1],
                        in_=xg[:, HH + r0 - 1:H, :],
                    )
                else:
                    nc.sync.dma_start(
                        out=it3[64:128, :, 1:W + 1],
                        in_=xg[:, HH + r0 - 1:HH + r0 + R + 1, :],
                    )

                # ---- vertical sort3 over rows (full padded width) ----
                X0 = it3[:, 0:R, :]
                X1 = it3[:, 1:R + 1, :]
                X2 = it3[:, 2:R + 2, :]

                def vtile(name):
                    t = v_pool.tile([128, R * WP], f32, name=name, tag=name)
                    return t.rearrange("p (r w) -> p r w", w=WP)

                P3 = vtile("bP")
                Q3 = vtile("bQ")
                Lo3 = vtile("bLo")
                W3 = vtile("bW")
                Me3 = vtile("bMe")
                Hi3 = vtile("bHi")

                tt(P3, X0, X1, MIN)
                tt(Q3, X0, X1, MAX)
                tt(Lo3, P3, X2, MIN)
                tt(W3, Q3, X2, MIN)
                tt(Me3, P3, W3, MAX)
                tt(Hi3, Q3, X2, MAX)

                # ---- horizontal merge (width 256 of 258) ----
                lo = [Lo3[:, :, d:d + W] for d in range(3)]
                me = [Me3[:, :, d:d + W] for d in range(3)]
                hi = [Hi3[:, :, d:d + W] for d in range(3)]

                def mtile(name):
                    t = m_pool.tile([128, R * W], f32, name=name, tag=name)
                    return t.rearrange("p (r w) -> p r w", w=W)

                mA = mtile("mA")
                mC = mtile("mC")
                mB = mtile("mB")
                mT = mtile("mT")
                mU = mtile("mU")
                mV = mtile("mV")

                # A = max3(lo)
                tt(mT, lo[0], lo[1], MAX)
                tt(mA, mT, lo[2], MAX)
                # C = min3(hi)
                tt(mU, hi[0], hi[1], MIN)
                tt(mC, mU, hi[2], MIN)
                # B = med3(me) = max(min(a,b), min(max(a,b), c))
                tt(mT, me[0], me[1], MIN)
                tt(mU, me[0], me[1], MAX)
                tt(mV, mU, me[2], MIN)
                tt(mB, mT, mV, MAX)

                # out = med3(A, B, C)
                ot = out_pool.tile([128, R * W], f32, name="ot", tag="ot")
                ot3 = ot.rearrange("p (r w) -> p r w", w=W)
                tt(mT, mA, mB, MIN)
                tt(mU, mA, mB, MAX)
                tt(mV, mU, mC, MIN)
                tt(ot3, mT, mV, MAX)

                # ---- store ----
                nc.sync.dma_start(out=og[:, r0:r0 + R, :], in_=ot3[0:64])
                nc.sync.dma_start(
                    out=og[:, HH + r0:HH + r0 + R, :], in_=ot3[64:128]
                )

    nc.compile()
    return nc


def _build_shared(R=8, gp_frac=0.0, dtype="float32", in_bufs=None, out_bufs=None):
    """15-op/pixel variant: vertical pair sharing + horizontal even/odd
    pair sharing in the merge.  gp_frac: fraction of rows of every
    elementwise op executed on GPSIMD instead of the DVE (unsupported by
    the current toolchain - keep 0).  dtype: compute dtype on-chip;
    float16 doubles DVE throughput on step-1 ops at ~2e-4 max rel err."""
    import concourse.bacc as bacc
    import concourse.mybir as mybir
    from concourse.tile import TileContext

    MIN = mybir.AluOpType.min
    MAX = mybir.AluOpType.max
    f32 = mybir.dt.float32
    cdt = getattr(mybir.dt, dtype)
    cast = cdt != f32

    assert HH % R == 0 and R % 2 == 0
    K = HH // R
    Rh = R // 2

    nc = bacc.Bacc("TRN2", name="median_pool2d_s")
    x = nc.dram_tensor("x", [C, H, W], f32, kind="ExternalInput")
    out = nc.dram_tensor("out", [C, H, W], f32, kind="ExternalOutput")
    xg = x.ap()
    og = out.ap()
    dma_io = nc.gpsimd if cast else nc.sync

    def tt(out_ap, in0, in1, op):
        rows = out_ap.shape[1]
        gp = int(rows * gp_frac + 0.5)
        split = rows - gp
        if split > 0:
            nc.vector.tensor_tensor(
                out=out_ap[:, 0:split], in0=in0[:, 0:split],
                in1=in1[:, 0:split], op=op,
            )
        if split < rows:
            nc.gpsimd.tensor_tensor(
                out=out_ap[:, split:rows], in0=in0[:, split:rows],
                in1=in1[:, split:rows], op=op,
            )

    if in_bufs is None:
        in_bufs = 3 if R <= 8 else 2
    if out_bufs is None:
        out_bufs = 3 if R <= 8 else 1
    with TileContext(nc) as tc:
        with (
            tc.tile_pool(name="io_in", bufs=in_bufs) as in_pool,
            tc.tile_pool(name="io_out", bufs=out_bufs) as out_pool,
            tc.tile_pool(name="work", bufs=1) as w_pool,
        ):
            def wtile(name, rows, width, tag=None):
                t = w_pool.tile([128, rows * width], cdt, name=name,
                                tag=tag or name)
                return t.rearrange("p (r w) -> p r w", w=width)

            for k in range(K):
                r0 = k * R
                it = in_pool.tile([128, (R + 2) * WP], cdt, name="it", tag="it")
                it3 = it.rearrange("p (r w) -> p r w", w=WP)
                nc.vector.memset(it3[:, :, 0:WP:WP - 1], 0.0)
                if k == 0:
                    nc.vector.memset(it3[0:64, 0:1, 1:W + 1], 0.0)
                    dma_io.dma_start(out=it3[0:64, 1:R + 2, 1:W + 1],
                                      in_=xg[:, 0:R + 1, :])
                else:
                    dma_io.dma_start(out=it3[0:64, :, 1:W + 1],
                                      in_=xg[:, r0 - 1:r0 + R + 1, :])
                if k == K - 1:
                    nc.vector.memset(it3[64:128, R + 1:R + 2, 1:W + 1], 0.0)
                    dma_io.dma_start(out=it3[64:128, 0:R + 1, 1:W + 1],
                                      in_=xg[:, HH + r0 - 1:H, :])
                else:
                    dma_io.dma_start(out=it3[64:128, :, 1:W + 1],
                                      in_=xg[:, HH + r0 - 1:HH + r0 + R + 1, :])

                # ---- vertical: shared pair sort ----
                # pairs over in-tile row pairs (2i+1, 2i+2), i = 0..R/2-1
                Pm = wtile("Pm", Rh, WP)
                PM = wtile("PM", Rh, WP)
                tt(Pm, it3[:, 1:R + 1:2, :], it3[:, 2:R + 2:2, :], MIN)
                tt(PM, it3[:, 1:R + 1:2, :], it3[:, 2:R + 2:2, :], MAX)

                Lo3 = wtile("Lo", R, WP)
                Me3 = wtile("Me", R, WP)
                Hi3 = wtile("Hi", R, WP)
                tE = wtile("tE", Rh, WP)
                tO = wtile("tO", Rh, WP)
                a_e = it3[:, 0:R:2, :]          # third element, even out rows
                a_o = it3[:, 3:R + 2:2, :]      # rows 3,5,..,R+1 (count R/2)
                # even out rows y=0,2,..  (pair index i=y/2)
                tt(Lo3[:, 0:R:2], a_e, Pm, MIN)
                tt(Hi3[:, 0:R:2], a_e, PM, MAX)
                tt(tE, a_e, PM, MIN)
                tt(Me3[:, 0:R:2], Pm, tE, MAX)
                # odd out rows y=1,3,..   (pair index i=(y-1)/2)
                tt(Lo3[:, 1:R:2], a_o, Pm, MIN)
                tt(Hi3[:, 1:R:2], a_o, PM, MAX)
                tt(tO, a_o, PM, MIN)
                tt(Me3[:, 1:R:2], Pm, tO, MAX)

                # ---- merge: horizontal shared pairs ----
                NP = W // 2 + 1                 # 129 pairs over padded width
                # Pm/PM/tE/tO are dead after the vertical completions;
                # alias their slots (Rh*WP = 2064 >= R*NP = 2064 elems).
                PA = wtile("PA", R, NP, tag="Pm")
                PC = wtile("PC", R, NP, tag="PM")
                Um = wtile("Um", R, NP, tag="tE")
                Vm = wtile("Vm", R, NP, tag="tO")
                # PA/PC (in Pm/PM slots) are dead once mA/mC are built;
                # rotate tBe/tBo through the same slots.
                tBe = wtile("tBe", R, W // 2, tag="Pm")
                tBo = wtile("tBo", R, W // 2, tag="PM")
                mA = wtile("mA", R, W)
                mB = wtile("mB", R, W)
                mC = wtile("mC", R, W)

                ev = slice(0, WP, 2)            # padded even cols (129)
                od = slice(1, WP, 2)            # padded odd cols (129)
                tt(PA, Lo3[:, :, ev], Lo3[:, :, od], MAX)
                tt(mA[:, :, 0:W:2], PA[:, :, 0:NP - 1], Lo3[:, :, 2:WP:2], MAX)
                tt(mA[:, :, 1:W:2], PA[:, :, 1:NP], Lo3[:, :, 1:WP - 2:2], MAX)

                tt(PC, Hi3[:, :, ev], Hi3[:, :, od], MIN)
                tt(mC[:, :, 0:W:2], PC[:, :, 0:NP - 1], Hi3[:, :, 2:WP:2], MIN)
                tt(mC[:, :, 1:W:2], PC[:, :, 1:NP], Hi3[:, :, 1:WP - 2:2], MIN)

                tt(Um, Me3[:, :, ev], Me3[:, :, od], MIN)
                tt(Vm, Me3[:, :, ev], Me3[:, :, od], MAX)
                tt(tBe, Me3[:, :, 2:WP:2], Vm[:, :, 0:NP - 1], MIN)
                tt(mB[:, :, 0:W:2], Um[:, :, 0:NP - 1], tBe, MAX)
                tt(tBo, Me3[:, :, 1:WP - 2:2], Vm[:, :, 1:NP], MIN)
                tt(mB[:, :, 1:W:2], Um[:, :, 1:NP], tBo, MAX)

                # ---- final med3(A, B, C) ----
                # Lo/Me/Hi are dead once the merge pairs+completions ran;
                # alias their slots (R*WP >= R*W).
                mT = wtile("mT", R, W, tag="Lo")
                mU = wtile("mU", R, W, tag="Me")
                mV = wtile("mV", R, W, tag="Hi")
                ot = out_pool.tile([128, R * W], cdt, name="ot", tag="ot")
                ot3 = ot.rearrange("p (r w) -> p r w", w=W)
                tt(mT, mA, mB, MIN)
                tt(mU, mA, mB, MAX)
                tt(mV, mU, mC, MIN)
                tt(ot3, mT, mV, MAX)

                dma_io.dma_start(out=og[:, r0:r0 + R, :], in_=ot3[0:64])
                dma_io.dma_start(out=og[:, HH + r0:HH + r0 + R, :],
                                  in_=ot3[64:128])

    nc.compile()
    return nc


def _build_copy():
    """Calibration kernel: pure DMA passthrough x -> out."""
    import concourse.bacc as bacc
    import concourse.mybir as mybir
    from concourse.tile import TileContext

    f32 = mybir.dt.float32
    nc = bacc.Bacc("TRN2", name="median_copy_cal")
    x = nc.dram_tensor("x", [C, H, W], f32, kind="ExternalInput")
    out = nc.dram_tensor("out", [C, H, W], f32, kind="ExternalOutput")
    xf = x.ap().rearrange("c h w -> (c h) w").rearrange(
        "(n p) w -> n p w", p=128)
    of = out.ap().rearrange("c h w -> (c h) w").rearrange(
        "(n p) w -> n p w", p=128)
    n = xf.shape[0]
    with TileContext(nc) as tc:
        with tc.tile_pool(name="io", bufs=4) as pool:
            for i in range(0, n, 8):
                t = pool.tile([128, 8 * W], f32, name="t", tag="t")
                t3 = t.rearrange("p (n w) -> p n w", w=W)
                nc.sync.dma_start(out=t3[:], in_=xf[i:i + 8].rearrange(
                    "n p w -> p n w"))
                nc.sync.dma_start(out=of[i:i + 8].rearrange("n p w -> p n w"),
                                  in_=t3[:])
    nc.compile()
    return nc


def _get_nc(R=8, gp_rows=0, shared=False, gp_frac=0.0, copy=False,
            dtype="float32", in_bufs=None, out_bufs=None):
    key = (R, gp_rows, shared, gp_frac, copy, dtype, in_bufs, out_bufs)
    if key not in _CACHE:
        if copy:
            _CACHE[key] = _build_copy()
        elif shared:
            _CACHE[key] = _build_shared(R=R, gp_frac=gp_frac, dtype=dtype,
                                        in_bufs=in_bufs, out_bufs=out_bufs)
        else:
            _CACHE[key] = _build(R=R, gp_rows=gp_rows)
    return _CACHE[key]


def kernel(x: np.ndarray) -> np.ndarray:
    """MedianPool2d(3x3, s=1, p=1) on 8 NeuronCores. Bit-exact vs fp32
    reference (pure min/max selection network, no arithmetic)."""
    from concourse.bass_utils import run_bass_kernel_spmd

    assert x.shape == (B, C, H, W), x.shape
    x = np.ascontiguousarray(x, dtype=np.float32)
    try:
        nc = _get_nc(shared=True, R=16)
    except Exception:
        # fall back to the simpler 18-op builder (also bit-exact)
        nc = _get_nc(R=8)
    in_maps = [{"x": x[i]} for i in range(NCORES)]
    res = run_bass_kernel_spmd(nc, in_maps, core_ids=list(range(NCORES)))
    return np.stack([r["out"] for r in res.results], axis=0)



# revision 27
# speedup vs baseline: 2.2608x; 2.2608x over previous
"""MedianPool2d (3x3, stride 1, zero-pad 1) Trainium2 Bass kernel.

Full input x: (8, 64, 256, 256) fp32.  Sharding: pure data parallel over
batch -> core i processes x[i] (64, 256, 256).

v3 design ("fp16 + custom sliding DVE ops + multi-engine"):
  - Per-core layout: 128 SBUF partitions = (h, c), h in {0,1} = top/bottom
    128-row half, c = channel.  Chunks of R output rows per partition.
  - DMA loads fp32 rows into a staging tile; the Activation engine casts
    fp32 -> fp16 into a padded row layout (WP=258: 256 cols + 2 zero pad
    cols).  fp16 is exact for the median network itself (pure min/max
    selection); only the input cast rounds (~2.4e-4 rel err, gate 2e-2).
  - Vertical sort3 (Lo/Me/Hi per output row) as fp16 tensor_tensor min/max
    on the DVE with row-pair sharing: all ops full-row step-1 4B-aligned
    -> 2x DVE mode (0.52 ns/elem).
  - Horizontal 3-tap merge via three custom DVE micro-op programs
    (HMAX3/HMIN3/HMED3): one streaming pass each computes
    out[j] = f(T[j-1], T[j], T[j+1]) over the flat row stream using two
    shifted APs of the same tile plus a -2-element temporal tap
    (NEXT_ALU_OUT_A reads the a-flop of the next pipeline stage, which
    latches src1 two elements back).  Each row carries 2 trailing zero pad
    cols, so the previous row's tail doubles as the next row's left pad;
    a 2-element zero prefix seeds row 0.  First 2 stream outputs land in
    the scrap prefix of the output tile (stale-flop garbage never touches
    real pixels).
  - Final med3(mA, mB, mC) on GPSIMD (4 tensor_tensor min/max), running
    concurrently with the DVE.
  - Output cast fp16 -> fp32 on the Activation engine, then DMA out.

Engines in steady state: DVE ~25us/chunk, GPSIMD ~24us, ACT ~8us,
DMA ~14us -> ~205us/core vs 547us baseline.
"""

import numpy as np

B, C, H, W = 8, 64, 256, 256
NCORES = 8
HH = H // 2          # rows per half-strip
WP = W + 2           # padded row width (2 trailing zero cols)

_CACHE = {}
_OPS = None


def _register_custom_ops():
    """Register the three sliding-window custom DVE ops (idempotent).

    Each op streams src0 = T[j], src1 = T[j+1] and computes a 3-tap
    window f(T[j-1], T[j], T[j+1]) per output element:
      - T[j], T[j+1] arrive spatially on the two source streams;
      - T[j-1] = src1 two elements back, read via NEXT_ALU_OUT_A from a
        pipeline stage that latches raw src1 into its a-flop each cycle.
    Outputs j=0,1 use pre-instruction flop state (garbage); callers must
    treat the first two output elements as scrap.
    """
    global _OPS
    if _OPS is not None:
        return _OPS

    import concourse.dve_ops as dve_ops
    from concourse.dve_spec import Spec, Src0, Src1, minn
    from concourse.dve_uop import (
        ENABLE,
        AluInp,
        AluOp,
        DelayInp,
        DveOpSpec,
        InpSel,
        OutPath,
        OutSel,
        Trigger,
        UopConfig,
    )

    def _shift2(in1):
        z = np.zeros_like(in1[..., :2])
        return np.concatenate([z, in1[..., :-2]], axis=-1)

    def _hmax3_ref(in0, in1, c0, c1, c2):
        return np.maximum(np.maximum(in0, in1), _shift2(in1))

    def _hmin3_ref(in0, in1, c0, c1, c2):
        return np.minimum(np.minimum(in0, in1), _shift2(in1))

    def _hmed3_ref(in0, in1, c0, c1, c2):
        c = _shift2(in1)
        return np.maximum(np.minimum(in0, in1),
                          np.minimum(np.maximum(in0, in1), c))

    def _mm3_uops(op3):
        """out[j] = op3(src0[j], src1[j], src1[j-2]) single-uop program."""
        u = UopConfig()
        u.enable_input(InpSel.SRC_0, 1)     # -> PREV_DELAY_0 at blk0
        u.enable_input(InpSel.SRC_1, 2)     # -> PREV_DELAY_1 at blk0
        u.require_inp0 = ENABLE
        u.require_inp1 = ENABLE
        u.trigger = (Trigger.SRC_TENSOR_DONE, Trigger.NONE, Trigger.NONE)
        dp = u.datapath_config
        # blk0: m = op3(src0, src1)
        dp[0].enable_alu(op3, AluInp.PREV_DELAY_0, AluInp.PREV_DELAY_1)
        dp[0].pass_through_delay(1)
        # blk1: out = op3(m, src1[j-2])  (blk2's a-flop, prev cycle)
        dp[1].enable_alu(op3, AluInp.PREV_ALU_OUT, AluInp.NEXT_ALU_OUT_A)
        dp[1].pass_through_delay(1)
        # blk2: latch raw src1 into the a-flop; stash out in delay chain 0
        dp[2].enable_alu(AluOp.BYPASS, AluInp.PREV_DELAY_1)
        dp[2].alu_out_a_enable = ENABLE
        dp[2].enable_delay_from_src(DelayInp.PREV_ALU_OUT, 0)
        for kblk in range(3, 8):
            dp[kblk].pass_through_alu().pass_through_delay(0)
        u.enable_output(OutSel.DELAY_0, OutPath.WR0_LO)
        return [u]

    def _med3_uops():
        """out[j] = med3(src0[j], src1[j], src1[j-2]) single-uop program."""
        u = UopConfig()
        u.enable_input(InpSel.SRC_0, 1)     # chain 0
        u.enable_input(InpSel.SRC_1, 2)     # chain 1
        u.require_inp0 = ENABLE
        u.require_inp1 = ENABLE
        u.trigger = (Trigger.SRC_TENSOR_DONE, Trigger.NONE, Trigger.NONE)
        dp = u.datapath_config
        # blk0: p = min(a, b)
        dp[0].enable_alu(AluOp.MIN, AluInp.PREV_DELAY_0, AluInp.PREV_DELAY_1)
        dp[0].pass_through_delay(0, 1)
        # blk1: q = max(a, b); stash p in chain 2
        dp[1].enable_alu(AluOp.MAX, AluInp.PREV_DELAY_0, AluInp.PREV_DELAY_1)
        dp[1].enable_delay_from_src(DelayInp.PREV_ALU_OUT, 2)
        dp[1].pass_through_delay(1)
        # blk2: t = min(q, c) with c = src1[j-2] via blk3's a-flop
        dp[2].enable_alu(AluOp.MIN, AluInp.PREV_ALU_OUT, AluInp.NEXT_ALU_OUT_A)
        dp[2].pass_through_delay(1, 2)
        # blk3: latch raw src1 into a-flop; stash t in chain 3
        dp[3].enable_alu(AluOp.BYPASS, AluInp.PREV_DELAY_1)
        dp[3].alu_out_a_enable = ENABLE
        dp[3].enable_delay_from_src(DelayInp.PREV_ALU_OUT, 3)
        dp[3].pass_through_delay(2)
        # blk4: out = max(p, t)
        dp[4].enable_alu(AluOp.MAX, AluInp.PREV_DELAY_2, AluInp.PREV_DELAY_3)
        for kblk in range(5, 8):
            dp[kblk].pass_through_alu()
        u.enable_output(OutSel.ALU_OUT, OutPath.WR0_LO)
        return [u]

    # ---- 2x (packed fp16) window-start variants -------------------------
    # Semantics: out[j] = f(T[j], T[j+1], T[j+2]) with src0 = T[0:N],
    # src1 = T[2:N+2], all APs fp16 step-1 4B-aligned so the RTL always
    # selects 2x_1p.  Per pair-cycle the crossbar exposes 4 consecutive
    # taps (SRC_0, SRC_0_HI, SRC_1, SRC_1_HI); both parities' windows are
    # pure spatial.  Even result -> WR0_LO, odd -> WR0_HI.

    def _shiftcat(in0, in1):
        # full stream T (len N+2) from the two views
        return np.concatenate([in0, in1[..., -2:]], axis=-1)

    def _hmax3w_ref(in0, in1, c0, c1, c2):
        T = _shiftcat(in0, in1)
        return np.maximum(np.maximum(T[..., :-2], T[..., 1:-1]), T[..., 2:])

    def _hmin3w_ref(in0, in1, c0, c1, c2):
        T = _shiftcat(in0, in1)
        return np.minimum(np.minimum(T[..., :-2], T[..., 1:-1]), T[..., 2:])

    def _hmed3w_ref(in0, in1, c0, c1, c2):
        T = _shiftcat(in0, in1)
        a, b, c = T[..., :-2], T[..., 1:-1], T[..., 2:]
        return np.maximum(np.minimum(a, b),
                          np.minimum(np.maximum(a, b), c))

    def _dummy_1x(op3):
        """Placeholder REGULAR-mode program (never selected: the APs
        always satisfy the 2x_1p conditions)."""
        u = UopConfig()
        u.enable_input(InpSel.SRC_0, 1)
        u.enable_input(InpSel.SRC_1, 2)
        u.require_inp0 = ENABLE
        u.require_inp1 = ENABLE
        u.trigger = (Trigger.SRC_TENSOR_DONE, Trigger.NONE, Trigger.NONE)
        dp = u.datapath_config
        dp[0].enable_alu(op3, AluInp.PREV_DELAY_0, AluInp.PREV_DELAY_1)
        for kblk in range(1, 8):
            dp[kblk].pass_through_alu()
        u.enable_output(OutSel.ALU_OUT, OutPath.WR0_LO)
        return [u]

    def _mm3_w2x_uops(op3):
        """2x program: E = op3(S0, S0H, S1) -> WR0_LO, O = op3(S0H, S1,
        S1H) -> WR0_HI."""
        u = UopConfig()
        u.enable_input(InpSel.SRC_0, 1)      # chain 0
        u.enable_input(InpSel.SRC_0_HI, 2)   # chain 1
        u.enable_input(InpSel.SRC_1, 3)      # chain 2
        u.enable_input(InpSel.SRC_1_HI, 4)   # chain 3
        u.require_inp0 = ENABLE
        u.require_inp1 = ENABLE
        u.trigger = (Trigger.SRC_TENSOR_DONE, Trigger.NONE, Trigger.NONE)
        dp = u.datapath_config
        dp[0].enable_alu(op3, AluInp.PREV_DELAY_0, AluInp.PREV_DELAY_1)
        dp[0].pass_through_delay(1, 2, 3)
        dp[1].enable_alu(op3, AluInp.PREV_ALU_OUT, AluInp.PREV_DELAY_2)
        dp[1].pass_through_delay(1, 2, 3)
        dp[2].enable_alu(op3, AluInp.PREV_DELAY_1, AluInp.PREV_DELAY_2)
        dp[2].enable_delay_from_src(DelayInp.PREV_ALU_OUT, 4)   # E
        dp[2].pass_through_delay(3)
        dp[3].enable_alu(op3, AluInp.PREV_ALU_OUT, AluInp.PREV_DELAY_3)
        dp[3].pass_through_delay(4)
        for kblk in range(4, 8):
            dp[kblk].pass_through_alu().pass_through_delay(4)
        u.enable_output(OutSel.DELAY_4, OutPath.WR0_LO)
        u.enable_output(OutSel.ALU_OUT, OutPath.WR0_HI)
        return [u]

    def _med3_w2x_uops():
        """2x program: E = med3(S0, S0H, S1) -> WR0_LO, O = med3(S0H, S1,
        S1H) -> WR0_HI.  Exactly 8 ALU blocks."""
        u = UopConfig()
        u.enable_input(InpSel.SRC_0, 1)      # chain 0
        u.enable_input(InpSel.SRC_0_HI, 2)   # chain 1
        u.enable_input(InpSel.SRC_1, 3)      # chain 2
        u.enable_input(InpSel.SRC_1_HI, 4)   # chain 3
        u.require_inp0 = ENABLE
        u.require_inp1 = ENABLE
        u.trigger = (Trigger.SRC_TENSOR_DONE, Trigger.NONE, Trigger.NONE)
        dp = u.datapath_config
        # E-half: med3(d0, d1, d2)
        dp[0].enable_alu(AluOp.MIN, AluInp.PREV_DELAY_0, AluInp.PREV_DELAY_1)
        dp[0].pass_through_delay(0, 1, 2, 3)
        dp[1].enable_alu(AluOp.MAX, AluInp.PREV_DELAY_0, AluInp.PREV_DELAY_1)
        dp[1].enable_delay_from_src(DelayInp.PREV_ALU_OUT, 4)   # pE
        dp[1].pass_through_delay(1, 2, 3)
        dp[2].enable_alu(AluOp.MIN, AluInp.PREV_ALU_OUT, AluInp.PREV_DELAY_2)
        dp[2].pass_through_delay(1, 2, 3, 4)
        dp[3].enable_alu(AluOp.MAX, AluInp.PREV_DELAY_4, AluInp.PREV_ALU_OUT)
        dp[3].pass_through_delay(1, 2, 3)
        # O-half: med3(d1, d2, d3)
        dp[4].enable_alu(AluOp.MIN, AluInp.PREV_DELAY_1, AluInp.PREV_DELAY_2)
        dp[4].enable_delay_from_src(DelayInp.PREV_ALU_OUT, 4)   # E
        dp[4].pass_through_delay(1, 2, 3)
        dp[5].enable_alu(AluOp.MAX, AluInp.PREV_DELAY_1, AluInp.PREV_DELAY_2)
        dp[5].enable_delay_from_src(DelayInp.PREV_ALU_OUT, 5)   # pO
        dp[5].pass_through_delay(3, 4)
        dp[6].enable_alu(AluOp.MIN, AluInp.PREV_ALU_OUT, AluInp.PREV_DELAY_3)
        dp[6].pass_through_delay(4, 5)
        dp[7].enable_alu(AluOp.MAX, AluInp.PREV_DELAY_5, AluInp.PREV_ALU_OUT)
        dp[7].pass_through_delay(4)
        u.enable_output(OutSel.DELAY_4, OutPath.WR0_LO)
        u.enable_output(OutSel.ALU_OUT, OutPath.WR0_HI)
        return [u]

    class _SlidingOp:
        """Duck-typed DveOp: hand-authored uops instead of lower(spec)."""

        def __init__(self, name, reference, uops_builder, uops_2x_builder=None):
            # body is a placeholder that reads Src0+Src1 (rd1_en) and no
            # C2/C3; only `reference` is ever evaluated (bass_interp).
            self.name = name
            self.spec = Spec(body=minn(Src0, Src1), reference=reference)
            self.subdim = False
            self.perf_en = {}
            self._builder = uops_builder
            self._builder_2x = uops_2x_builder
            self._cache = {}

        def compile(self, ver):
            if ver not in self._cache:
                s = DveOpSpec(
                    name=self.name,
                    opcode=dve_ops.get_dve_sub_opcode(self.name),
                    uops=self._builder(),
                    uops_2x=(self._builder_2x() if self._builder_2x
                             else None),
                    perf_max=1 if self._builder_2x else 0,
                    rd1_en=True,
                )
                s.validate(ver)
                self._cache[ver] = s
            return self._cache[ver]

    defs = [
        ("HMAX3_ANT", _hmax3_ref, lambda: _mm3_uops(AluOp.MAX), None),
        ("HMIN3_ANT", _hmin3_ref, lambda: _mm3_uops(AluOp.MIN), None),
        ("HMED3_ANT", _hmed3_ref, _med3_uops, None),
        ("HMAX3W_ANT", _hmax3w_ref, lambda: _dummy_1x(AluOp.MAX),
         lambda: _mm3_w2x_uops(AluOp.MAX)),
        ("HMIN3W_ANT", _hmin3w_ref, lambda: _dummy_1x(AluOp.MIN),
         lambda: _mm3_w2x_uops(AluOp.MIN)),
        ("HMED3W_ANT", _hmed3w_ref, lambda: _dummy_1x(AluOp.MIN),
         _med3_w2x_uops),
    ]
    ops = {}
    for name, ref, builder, builder2x in defs:
        if name not in dve_ops._SUB_OPCODE_FOR_NAME:
            row = max(dve_ops._SUB_OPCODE_FOR_NAME.values()) + 1
            assert row < 0x20, "custom DVE opcode rows exhausted"
            dve_ops._SUB_OPCODE_FOR_NAME[name] = row
        op = _SlidingOp(name, ref, builder, builder2x)
        # replace any previous registration (idempotent across reloads)
        dve_ops.OPS[:] = [o for o in dve_ops.OPS if o.name != name] + [op]
        dve_ops.CUSTOM_DVE_SPECS[name] = op.spec
        ops[name] = op
    _OPS = ops
    return ops


def _emit_custom_dve(v, op, out, in0, in1, perf_max=0):
    """Like bass.Vector._custom_dve but with perf_max set at construction
    (the Tile scheduler drops post-hoc attribute edits)."""
    import concourse.bass_isa as bass_isa
    import concourse.mybir as mybir
    from concourse.dve_ops import get_dve_sub_opcode

    b = v.bass
    if op.name not in b.m.ant_custom_dve_ops:
        b.m.ant_custom_dve_ops = sorted({*b.m.ant_custom_dve_ops, op.name})
    shape = bass_isa.CustomDveShape.TTSS
    isa_opcode = b.isa.Opcode[
        f"NEURON_ISA_TPB_OPCODE_CUSTOM_DVE_ANT_{shape.slot()}"
    ].value
    imm = lambda: mybir.ImmediateValue(dtype=mybir.dt.float32, value=0.0)
    return v.add_instruction(
        bass_isa.InstCustomDveAnt(
            name=b.get_next_instruction_name(),
            op_name=op.name,
            rd1_en=True,
            subdim=0,
            imm2=0.0,
            shape=shape,
            row=get_dve_sub_opcode(op.name),
            perf_max=perf_max,
            isa_opcode=isa_opcode,
            ins=[v.lower_ap(in0, for_isa=True),
                 v.lower_ap(in1, for_isa=True), imm(), imm()],
            outs=[v.lower_ap(out, for_isa=True)],
        )
    )


def _build_v3(R=8, final_gp_frac=0.65, n_vert_gp=1, in_bufs=2, mrg_bufs=1,
              x16_bufs=2, dve_tail=True, sliding2x=True, use_gp=False):
    """fp16 + custom sliding ops + ACT casts + GPSIMD final stage.

    Software-pipelined emission (engines execute their streams in order):
    per iteration k we emit  A(k+1) = load+cast of the NEXT chunk,
    D(k-1) = store of the PREVIOUS chunk, then B(k) = DVE work and
    C(k) = final stage.  This keeps the ACT in-cast ahead of the
    GPSIMD-dependent out-cast in the ACT/sync instruction streams.

    n_final_gp: how many of the 4 final med3 tensor_tensor ops run on
    GPSIMD (rest on DVE).  n_vert_gp: how many vertical ops on GPSIMD.
    dve_tail: run the last chunk's final stage on the DVE (shorter tail).
    """
    import concourse.bacc as bacc
    import concourse.mybir as mybir
    from concourse.tile import TileContext

    ops = _register_custom_ops()
    MIN = mybir.AluOpType.min
    MAX = mybir.AluOpType.max
    f32 = mybir.dt.float32
    f16 = mybir.dt.float16

    assert HH % R == 0 and R % 2 == 0
    K = HH // R
    Rh = R // 2
    NS = 2 + R * WP          # sliding stream length (2 prefix + rows)
    LSZ = R * WP + 4         # sliding tile: prefix 2 + rows + tail 2

    nc = bacc.Bacc("TRN2", name="median_pool2d_v3")
    x = nc.dram_tensor("x", [C, H, W], f32, kind="ExternalInput")
    out = nc.dram_tensor("out", [C, H, W], f32, kind="ExternalOutput")
    xg = x.ap()
    og = out.ap()

    with TileContext(nc) as tc:
        with (
            tc.tile_pool(name="in32", bufs=in_bufs) as in_pool,
            tc.tile_pool(name="x16p", bufs=x16_bufs) as x16_pool,
            tc.tile_pool(name="vert", bufs=1) as v_pool,
            tc.tile_pool(name="mrg", bufs=mrg_bufs) as m_pool,
            tc.tile_pool(name="fin", bufs=1) as f_pool,
            tc.tile_pool(name="ot16", bufs=2) as ot_pool,
            tc.tile_pool(name="out32", bufs=1) as o32_pool,
        ):
            # persistent sliding tiles: prefix/tail zeros written once
            def stile(name):
                t = v_pool.tile([128, LSZ], f16, name=name, tag=name)
                rows = t[:, 2:2 + R * WP].rearrange("p (r w) -> p r w", w=WP)
                return t, rows

            Lo_t, Lo3 = stile("Lo")
            Me_t, Me3 = stile("Me")
            Hi_t, Hi3 = stile("Hi")
            for t in (Lo_t, Me_t, Hi_t):
                nc.gpsimd.memset(t[:, 0:2], 0.0)
                nc.gpsimd.memset(t[:, 2 + R * WP:LSZ], 0.0)

            def vtile(name, rows):
                t = v_pool.tile([128, rows * WP], f16, name=name, tag=name)
                return t.rearrange("p (r w) -> p r w", w=WP)

            Pm = vtile("Pm", Rh)
            PM = vtile("PM", Rh)
            tEv = vtile("tE", Rh)
            tOv = vtile("tO", Rh)

            def stage_a(k):
                """DMA fp32 rows + ACT cast to padded fp16 rows."""
                r0 = k * R
                s32 = in_pool.tile([128, (R + 2) * W], f32, name="s32",
                                   tag="s32")
                s3 = s32.rearrange("p (r w) -> p r w", w=W)
                if k == 0:
                    nc.vector.memset(s3[0:64, 0:1], 0.0)
                    nc.sync.dma_start(out=s3[0:64, 1:R + 2],
                                      in_=xg[:, 0:R + 1, :])
                else:
                    nc.sync.dma_start(out=s3[0:64],
                                      in_=xg[:, r0 - 1:r0 + R + 1, :])
                if k == K - 1:
                    nc.vector.memset(s3[64:128, R + 1:R + 2], 0.0)
                    nc.sync.dma_start(out=s3[64:128, 0:R + 1],
                                      in_=xg[:, HH + r0 - 1:H, :])
                else:
                    nc.sync.dma_start(out=s3[64:128],
                                      in_=xg[:, HH + r0 - 1:HH + r0 + R + 1, :])
                x16 = x16_pool.tile([128, (R + 2) * WP], f16, name="x16",
                                    tag="x16")
                X = x16.rearrange("p (r w) -> p r w", w=WP)
                nc.gpsimd.memset(X[:, :, W:WP], 0.0)
                nc.scalar.copy(out=X[:, :, 0:W], in_=s3[:])
                return X

            def stage_bc(k, X):
                """DVE vertical + sliding, then final stage."""
                # ---- vertical sort3 (fp16 TT, 2x mode, pair-shared) ----
                vops = [
                    (Pm, X[:, 1:R + 1:2], X[:, 2:R + 2:2], MIN),
                    (PM, X[:, 1:R + 1:2], X[:, 2:R + 2:2], MAX),
                    (Lo3[:, 0:R:2], X[:, 0:R:2], Pm, MIN),
                    (Hi3[:, 0:R:2], X[:, 0:R:2], PM, MAX),
                    (tEv, X[:, 0:R:2], PM, MIN),
                    (Me3[:, 0:R:2], Pm, tEv, MAX),
                    (Lo3[:, 1:R:2], X[:, 3:R + 2:2], Pm, MIN),
                    (Hi3[:, 1:R:2], X[:, 3:R + 2:2], PM, MAX),
                    (tOv, X[:, 3:R + 2:2], PM, MIN),
                    (Me3[:, 1:R:2], Pm, tOv, MAX),
                ]
                for i, (o, a, b, alu) in enumerate(vops):
                    eng = (nc.gpsimd if i >= len(vops) - n_vert_gp
                           else nc.vector)
                    eng.tensor_tensor(out=o, in0=a, in1=b, op=alu)

                # ---- horizontal 3-tap merge: custom sliding DVE ops ----
                mA_t = m_pool.tile([128, LSZ], f16, name="mA", tag="mA")
                mB_t = m_pool.tile([128, LSZ], f16, name="mB", tag="mB")
                mC_t = m_pool.tile([128, LSZ], f16, name="mC", tag="mC")
                if sliding2x:
                    # window-start form at 2x: out[j] = f(T[j..j+2]);
                    # all APs even-offset fp16 step-1 -> RTL picks 2x_1p.
                    N2 = R * WP
                    for opname, src, dst in (
                        ("HMAX3W_ANT", Lo_t, mA_t),
                        ("HMED3W_ANT", Me_t, mB_t),
                        ("HMIN3W_ANT", Hi_t, mC_t),
                    ):
                        _emit_custom_dve(
                            nc.vector, ops[opname], out=dst[:, 0:N2],
                            in0=src[:, 0:N2], in1=src[:, 2:N2 + 2],
                            perf_max=1)
                    off = 0   # outputs shifted one left; final reads from 0
                else:
                    nc.vector._custom_dve(ops["HMAX3_ANT"],
                                          out=mA_t[:, 0:NS],
                                          in0=Lo_t[:, 0:NS],
                                          in1=Lo_t[:, 1:NS + 1])
                    nc.vector._custom_dve(ops["HMIN3_ANT"],
                                          out=mC_t[:, 0:NS],
                                          in0=Hi_t[:, 0:NS],
                                          in1=Hi_t[:, 1:NS + 1])
                    nc.vector._custom_dve(ops["HMED3_ANT"],
                                          out=mB_t[:, 0:NS],
                                          in0=Me_t[:, 0:NS],
                                          in1=Me_t[:, 1:NS + 1])
                    off = 2

                # ---- final med3(mA, mB, mC) ----
                mT = f_pool.tile([128, R * WP], f16, name="mT", tag="mT")
                mU = f_pool.tile([128, R * WP], f16, name="mU", tag="mU")
                mV = m_pool.tile([128, LSZ], f16, name="mV", tag="mA")
                ot = ot_pool.tile([128, R * WP + 2], f16, name="ot",
                                  tag="ot")
                Af = mA_t[:, off:off + R * WP]
                Bf = mB_t[:, off:off + R * WP]
                Cf = mC_t[:, off:off + R * WP]
                # split the 4-op chain by element range: each engine runs an
                # independent chain over its own slice (no cross-engine deps)
                g = 0.0 if (dve_tail and k == K - 1) else final_gp_frac
                S = 2 * int(R * WP * (1.0 - g) / 2)
                for eng, s0, s1 in ((nc.vector, 0, S),
                                    (nc.gpsimd, S, R * WP)):
                    if s1 <= s0:
                        continue
                    sl = slice(s0, s1)
                    osl = slice(off + s0, off + s1)
                    eng.tensor_tensor(out=mT[:, sl], in0=mA_t[:, osl],
                                      in1=mB_t[:, osl], op=MIN)
                    eng.tensor_tensor(out=mU[:, sl], in0=mA_t[:, osl],
                                      in1=mB_t[:, osl], op=MAX)
                    eng.tensor_tensor(out=mV[:, sl], in0=mU[:, sl],
                                      in1=mC_t[:, osl], op=MIN)
                    eng.tensor_tensor(out=ot[:, sl], in0=mT[:, sl],
                                      in1=mV[:, sl], op=MAX)
                return ot

            def stage_bc_dve(k, X):
                """All-DVE variant: GPSIMD tensor_tensor min/max is
                rejected by the walrus codegen, so everything runs on the
                DVE; dead Lo/Me/Hi buffers are re-used for the final
                temporaries (same engine, in-order, so aliasing is free)."""
                for o, a, b, alu in (
                    (Pm, X[:, 1:R + 1:2], X[:, 2:R + 2:2], MIN),
                    (PM, X[:, 1:R + 1:2], X[:, 2:R + 2:2], MAX),
                    (Lo3[:, 0:R:2], X[:, 0:R:2], Pm, MIN),
                    (Hi3[:, 0:R:2], X[:, 0:R:2], PM, MAX),
                    (tEv, X[:, 0:R:2], PM, MIN),
                    (Me3[:, 0:R:2], Pm, tEv, MAX),
                    (Lo3[:, 1:R:2], X[:, 3:R + 2:2], Pm, MIN),
                    (Hi3[:, 1:R:2], X[:, 3:R + 2:2], PM, MAX),
                    (tOv, X[:, 3:R + 2:2], PM, MIN),
                    (Me3[:, 1:R:2], Pm, tOv, MAX),
                ):
                    nc.vector.tensor_tensor(out=o, in0=a, in1=b, op=alu)

                mA_t = m_pool.tile([128, LSZ], f16, name="mA", tag="mA")
                mB_t = m_pool.tile([128, LSZ], f16, name="mB", tag="mB")
                mC_t = m_pool.tile([128, LSZ], f16, name="mC", tag="mC")
                N2 = R * WP
                for opname, src, dst in (
                    ("HMAX3W_ANT", Lo_t, mA_t),
                    ("HMED3W_ANT", Me_t, mB_t),
                    ("HMIN3W_ANT", Hi_t, mC_t),
                ):
                    _emit_custom_dve(
                        nc.vector, ops[opname], out=dst[:, 0:N2],
                        in0=src[:, 0:N2], in1=src[:, 2:N2 + 2], perf_max=1)

                # final med3 (all ops even-offset fp16 step-1 -> 2x mode)
                mT = f_pool.tile([128, N2], f16, name="mT", tag="mT")
                mU = f_pool.tile([128, N2], f16, name="mU", tag="mU")
                mV = f_pool.tile([128, N2], f16, name="mV", tag="mV")
                ot = ot_pool.tile([128, R * WP + 2], f16, name="ot",
                                  tag="ot")
                sl = slice(0, N2)
                nc.vector.tensor_tensor(out=mT[:], in0=mA_t[:, sl],
                                        in1=mB_t[:, sl], op=MIN)
                nc.vector.tensor_tensor(out=mU[:], in0=mA_t[:, sl],
                                        in1=mB_t[:, sl], op=MAX)
                nc.vector.tensor_tensor(out=mV[:], in0=mU[:],
                                        in1=mC_t[:, sl], op=MIN)
                nc.vector.tensor_tensor(out=ot[:, sl], in0=mT[:],
                                        in1=mV[:], op=MAX)
                return ot

            ot_off = 1 if sliding2x else 0

            def stage_d(k, ot):
                """ACT cast fp16 -> fp32 + DMA out."""
                r0 = k * R
                o32 = o32_pool.tile([128, R * W], f32, name="o32", tag="o32")
                o3 = o32.rearrange("p (r w) -> p r w", w=W)
                ot3 = ot[:, ot_off:ot_off + R * WP].rearrange(
                    "p (r w) -> p r w", w=WP)
                nc.scalar.copy(out=o3[:], in_=ot3[:, :, 0:W])
                # separate DGE queue from the input loads (sync) so stores
                # never head-of-line-block the next loads
                nc.scalar.dma_start(out=og[:, r0:r0 + R, :], in_=o3[0:64])
                nc.scalar.dma_start(out=og[:, HH + r0:HH + r0 + R, :],
                                    in_=o3[64:128])

            Xs = {0: stage_a(0)}
            ots = {}
            ddel = 1 if use_gp else 0   # store-delay (GPSIMD-final only)
            for k in range(K):
                if k + 1 < K:
                    Xs[k + 1] = stage_a(k + 1)
                if k - ddel in ots:
                    stage_d(k - ddel, ots.pop(k - ddel))
                ots[k] = (stage_bc(k, Xs.pop(k)) if use_gp
                          else stage_bc_dve(k, Xs.pop(k)))
                if ddel == 0:
                    stage_d(k, ots.pop(k))
            for kk in sorted(ots):
                stage_d(kk, ots.pop(kk))

    nc.compile()
    return nc


def _build_shared(R=16, gp_frac=0.0, dtype="float32", in_bufs=None,
                  out_bufs=None):
    """Fallback: 15-op/pixel fp32 TT network (previous working kernel)."""
    import concourse.bacc as bacc
    import concourse.mybir as mybir
    from concourse.tile import TileContext

    MIN = mybir.AluOpType.min
    MAX = mybir.AluOpType.max
    f32 = mybir.dt.float32
    cdt = getattr(mybir.dt, dtype)

    WPP = W + 2
    assert HH % R == 0 and R % 2 == 0
    K = HH // R
    Rh = R // 2

    nc = bacc.Bacc("TRN2", name="median_pool2d_s")
    x = nc.dram_tensor("x", [C, H, W], f32, kind="ExternalInput")
    out = nc.dram_tensor("out", [C, H, W], f32, kind="ExternalOutput")
    xg = x.ap()
    og = out.ap()

    def tt(out_ap, in0, in1, op):
        nc.vector.tensor_tensor(out=out_ap, in0=in0, in1=in1, op=op)

    if in_bufs is None:
        in_bufs = 3 if R <= 8 else 2
    if out_bufs is None:
        out_bufs = 3 if R <= 8 else 1
    with TileContext(nc) as tc:
        with (
            tc.tile_pool(name="io_in", bufs=in_bufs) as in_pool,
            tc.tile_pool(name="io_out", bufs=out_bufs) as out_pool,
            tc.tile_pool(name="work", bufs=1) as w_pool,
        ):
            def wtile(name, rows, width, tag=None):
                t = w_pool.tile([128, rows * width], cdt, name=name,
                                tag=tag or name)
                return t.rearrange("p (r w) -> p r w", w=width)

            for k in range(K):
                r0 = k * R
                it = in_pool.tile([128, (R + 2) * WPP], cdt, name="it",
                                  tag="it")
                it3 = it.rearrange("p (r w) -> p r w", w=WPP)
                nc.vector.memset(it3[:, :, 0:WPP:WPP - 1], 0.0)
                if k == 0:
                    nc.vector.memset(it3[0:64, 0:1, 1:W + 1], 0.0)
                    nc.sync.dma_start(out=it3[0:64, 1:R + 2, 1:W + 1],
                                      in_=xg[:, 0:R + 1, :])
                else:
                    nc.sync.dma_start(out=it3[0:64, :, 1:W + 1],
                                      in_=xg[:, r0 - 1:r0 + R + 1, :])
                if k == K - 1:
                    nc.vector.memset(it3[64:128, R + 1:R + 2, 1:W + 1], 0.0)
                    nc.sync.dma_start(out=it3[64:128, 0:R + 1, 1:W + 1],
                                      in_=xg[:, HH + r0 - 1:H, :])
                else:
                    nc.sync.dma_start(out=it3[64:128, :, 1:W + 1],
                                      in_=xg[:, HH + r0 - 1:HH + r0 + R + 1, :])

                Pm = wtile("Pm", Rh, WPP)
                PM = wtile("PM", Rh, WPP)
                tt(Pm, it3[:, 1:R + 1:2, :], it3[:, 2:R + 2:2, :], MIN)
                tt(PM, it3[:, 1:R + 1:2, :], it3[:, 2:R + 2:2, :], MAX)

                Lo3 = wtile("Lo", R, WPP)
                Me3 = wtile("Me", R, WPP)
                Hi3 = wtile("Hi", R, WPP)
                tE = wtile("tE", Rh, WPP)
                tO = wtile("tO", Rh, WPP)
                a_e = it3[:, 0:R:2, :]
                a_o = it3[:, 3:R + 2:2, :]
                tt(Lo3[:, 0:R:2], a_e, Pm, MIN)
                tt(Hi3[:, 0:R:2], a_e, PM, MAX)
                tt(tE, a_e, PM, MIN)
                tt(Me3[:, 0:R:2], Pm, tE, MAX)
                tt(Lo3[:, 1:R:2], a_o, Pm, MIN)
                tt(Hi3[:, 1:R:2], a_o, PM, MAX)
                tt(tO, a_o, PM, MIN)
                tt(Me3[:, 1:R:2], Pm, tO, MAX)

                NP = W // 2 + 1
                PA = wtile("PA", R, NP, tag="Pm")
                PC = wtile("PC", R, NP, tag="PM")
                Um = wtile("Um", R, NP, tag="tE")
                Vm = wtile("Vm", R, NP, tag="tO")
                tBe = wtile("tBe", R, W // 2, tag="Pm")
                tBo = wtile("tBo", R, W // 2, tag="PM")
                mA = wtile("mA", R, W)
                mB = wtile("mB", R, W)
                mC = wtile("mC", R, W)

                ev = slice(0, WPP, 2)
                od = slice(1, WPP, 2)
                tt(PA, Lo3[:, :, ev], Lo3[:, :, od], MAX)
                tt(mA[:, :, 0:W:2], PA[:, :, 0:NP - 1], Lo3[:, :, 2:WPP:2],
                   MAX)
                tt(mA[:, :, 1:W:2], PA[:, :, 1:NP], Lo3[:, :, 1:WPP - 2:2],
                   MAX)

                tt(PC, Hi3[:, :, ev], Hi3[:, :, od], MIN)
                tt(mC[:, :, 0:W:2], PC[:, :, 0:NP - 1], Hi3[:, :, 2:WPP:2],
                   MIN)
                tt(mC[:, :, 1:W:2], PC[:, :, 1:NP], Hi3[:, :, 1:WPP - 2:2],
                   MIN)

                tt(Um, Me3[:, :, ev], Me3[:, :, od], MIN)
                tt(Vm, Me3[:, :, ev], Me3[:, :, od], MAX)
                tt(tBe, Me3[:, :, 2:WPP:2], Vm[:, :, 0:NP - 1], MIN)
                tt(mB[:, :, 0:W:2], Um[:, :, 0:NP - 1], tBe, MAX)
                tt(tBo, Me3[:, :, 1:WPP - 2:2], Vm[:, :, 1:NP], MIN)
                tt(mB[:, :, 1:W:2], Um[:, :, 1:NP], tBo, MAX)

                mT = wtile("mT", R, W, tag="Lo")
                mU = wtile("mU", R, W, tag="Me")
                mV = wtile("mV", R, W, tag="Hi")
                ot = out_pool.tile([128, R * W], cdt, name="ot", tag="ot")
                ot3 = ot.rearrange("p (r w) -> p r w", w=W)
                tt(mT, mA, mB, MIN)
                tt(mU, mA, mB, MAX)
                tt(mV, mU, mC, MIN)
                tt(ot3, mT, mV, MAX)

                nc.sync.dma_start(out=og[:, r0:r0 + R, :], in_=ot3[0:64])
                nc.sync.dma_start(out=og[:, HH + r0:HH + r0 + R, :],
                                  in_=ot3[64:128])

    nc.compile()
    return nc


def _get_nc(variant="v3", **kw):
    key = (variant, tuple(sorted(kw.items())))
    if key not in _CACHE:
        if variant == "v3":
            _CACHE[key] = _build_v3(**kw)
        else:
            _CACHE[key] = _build_shared(**kw)
    return _CACHE[key]


_LAST_NC = None


def kernel(x: np.ndarray) -> np.ndarray:
    """MedianPool2d(3x3, s=1, p=1) on 8 NeuronCores."""
    global _LAST_NC
    from concourse.bass_utils import run_bass_kernel_spmd

    assert x.shape == (B, C, H, W), x.shape
    x = np.ascontiguousarray(x, dtype=np.float32)
    try:
        nc = _get_nc("v3")
    except Exception:
        nc = _get_nc("shared", R=16)
    _LAST_NC = nc
    in_maps = [{"x": x[i]} for i in range(NCORES)]
    res = run_bass_kernel_spmd(nc, in_maps, core_ids=list(range(NCORES)))
    return np.stack([r["out"] for r in res.results], axis=0)


# revision 30
# speedup vs baseline: 2.3406x; 1.0353x over previous
"""MedianPool2d (3x3, stride 1, zero-pad 1) Trainium2 Bass kernel.

Full input x: (8, 64, 256, 256) fp32.  Sharding: pure data parallel over
batch -> core i processes x[i] (64, 256, 256).

Design (v4: fp16 + custom packed-2x sliding DVE ops):
  - Per-core layout: 128 SBUF partitions = (h, c), h in {0,1} = top/bottom
    128-row half, c = channel.  Heterogeneous chunks of output rows per
    partition (small first/last chunks shorten pipeline ramp/tail).
  - DMA loads fp32 rows into a staging tile; the Activation engine casts
    fp32 -> fp16 into a padded row layout (WP=258: 256 cols + 2 zero pad
    cols).  fp16 is exact for the median network itself (pure min/max
    selection); only the input cast rounds (~2.1e-4 l2 rel, gate 2e-2).
  - Vertical sort3 (Lo/Me/Hi per output row) as fp16 tensor_tensor
    min/max on the DVE with row-pair sharing: all ops full-row step-1
    4B-aligned -> genuine 2x DVE mode (0.52 ns/elem).
  - Horizontal 3-tap merge via three hand-authored custom DVE micro-op
    programs (HMAX3W/HMED3W/HMIN3W, registered into the per-NEFF DVE
    table at runtime): ONE streaming pass each computes
    out[j] = f(T[j], T[j+1], T[j+2]) over the flat row stream.  The APs
    (src0 = T[0:N], src1 = T[2:N+2], both fp16 step-1 4B-aligned) make
    the RTL select 2x_1p packed mode, where the crossbar exposes four
    consecutive taps per cycle (SRC_0, SRC_0_HI, SRC_1, SRC_1_HI) - both
    parities' windows are pure spatial, even result -> WR0_LO, odd ->
    WR0_HI, 2 elem/cycle.  Each row carries 2 trailing zero pad cols so
    the previous row's tail doubles as the next row's left pad; a
    2-element zero prefix seeds row 0.  HW-validated bit-exact.
  - Final med3(mA, mB, mC) as 4 more fp16 2x TTs on the DVE.
    (GPSIMD tensor_tensor min/max and DMA-CCE min/max accumulate are both
    rejected by the walrus codegen, so GPSIMD only does small memsets.)
  - Output cast fp16 -> fp32 on the Activation engine, then DMA out on a
    separate DGE queue from the loads.

Effective cost ~12 TT-op-equivalents/pixel at 2x (vs 15 at 1x fp32 for
the baseline) -> 233.6us/core vs 546.7us baseline (2.34x).
"""

import numpy as np

B, C, H, W = 8, 64, 256, 256
NCORES = 8
HH = H // 2          # rows per half-strip
WP = W + 2           # padded row width (2 trailing zero cols)

_CACHE = {}
_OPS = None


def _register_custom_ops():
    """Register the three sliding-window custom DVE ops (idempotent).

    Each op streams src0 = T[j], src1 = T[j+1] and computes a 3-tap
    window f(T[j-1], T[j], T[j+1]) per output element:
      - T[j], T[j+1] arrive spatially on the two source streams;
      - T[j-1] = src1 two elements back, read via NEXT_ALU_OUT_A from a
        pipeline stage that latches raw src1 into its a-flop each cycle.
    Outputs j=0,1 use pre-instruction flop state (garbage); callers must
    treat the first two output elements as scrap.
    """
    global _OPS
    if _OPS is not None:
        return _OPS

    import concourse.dve_ops as dve_ops
    from concourse.dve_spec import Spec, Src0, Src1, minn
    from concourse.dve_uop import (
        ENABLE,
        AluInp,
        AluOp,
        DelayInp,
        DveOpSpec,
        InpSel,
        OutPath,
        OutSel,
        Trigger,
        UopConfig,
    )

    def _shift2(in1):
        z = np.zeros_like(in1[..., :2])
        return np.concatenate([z, in1[..., :-2]], axis=-1)

    def _hmax3_ref(in0, in1, c0, c1, c2):
        return np.maximum(np.maximum(in0, in1), _shift2(in1))

    def _hmin3_ref(in0, in1, c0, c1, c2):
        return np.minimum(np.minimum(in0, in1), _shift2(in1))

    def _hmed3_ref(in0, in1, c0, c1, c2):
        c = _shift2(in1)
        return np.maximum(np.minimum(in0, in1),
                          np.minimum(np.maximum(in0, in1), c))

    def _mm3_uops(op3):
        """out[j] = op3(src0[j], src1[j], src1[j-2]) single-uop program."""
        u = UopConfig()
        u.enable_input(InpSel.SRC_0, 1)     # -> PREV_DELAY_0 at blk0
        u.enable_input(InpSel.SRC_1, 2)     # -> PREV_DELAY_1 at blk0
        u.require_inp0 = ENABLE
        u.require_inp1 = ENABLE
        u.trigger = (Trigger.SRC_TENSOR_DONE, Trigger.NONE, Trigger.NONE)
        dp = u.datapath_config
        # blk0: m = op3(src0, src1)
        dp[0].enable_alu(op3, AluInp.PREV_DELAY_0, AluInp.PREV_DELAY_1)
        dp[0].pass_through_delay(1)
        # blk1: out = op3(m, src1[j-2])  (blk2's a-flop, prev cycle)
        dp[1].enable_alu(op3, AluInp.PREV_ALU_OUT, AluInp.NEXT_ALU_OUT_A)
        dp[1].pass_through_delay(1)
        # blk2: latch raw src1 into the a-flop; stash out in delay chain 0
        dp[2].enable_alu(AluOp.BYPASS, AluInp.PREV_DELAY_1)
        dp[2].alu_out_a_enable = ENABLE
        dp[2].enable_delay_from_src(DelayInp.PREV_ALU_OUT, 0)
        for kblk in range(3, 8):
            dp[kblk].pass_through_alu().pass_through_delay(0)
        u.enable_output(OutSel.DELAY_0, OutPath.WR0_LO)
        return [u]

    def _med3_uops():
        """out[j] = med3(src0[j], src1[j], src1[j-2]) single-uop program."""
        u = UopConfig()
        u.enable_input(InpSel.SRC_0, 1)     # chain 0
        u.enable_input(InpSel.SRC_1, 2)     # chain 1
        u.require_inp0 = ENABLE
        u.require_inp1 = ENABLE
        u.trigger = (Trigger.SRC_TENSOR_DONE, Trigger.NONE, Trigger.NONE)
        dp = u.datapath_config
        # blk0: p = min(a, b)
        dp[0].enable_alu(AluOp.MIN, AluInp.PREV_DELAY_0, AluInp.PREV_DELAY_1)
        dp[0].pass_through_delay(0, 1)
        # blk1: q = max(a, b); stash p in chain 2
        dp[1].enable_alu(AluOp.MAX, AluInp.PREV_DELAY_0, AluInp.PREV_DELAY_1)
        dp[1].enable_delay_from_src(DelayInp.PREV_ALU_OUT, 2)
        dp[1].pass_through_delay(1)
        # blk2: t = min(q, c) with c = src1[j-2] via blk3's a-flop
        dp[2].enable_alu(AluOp.MIN, AluInp.PREV_ALU_OUT, AluInp.NEXT_ALU_OUT_A)
        dp[2].pass_through_delay(1, 2)
        # blk3: latch raw src1 into a-flop; stash t in chain 3
        dp[3].enable_alu(AluOp.BYPASS, AluInp.PREV_DELAY_1)
        dp[3].alu_out_a_enable = ENABLE
        dp[3].enable_delay_from_src(DelayInp.PREV_ALU_OUT, 3)
        dp[3].pass_through_delay(2)
        # blk4: out = max(p, t)
        dp[4].enable_alu(AluOp.MAX, AluInp.PREV_DELAY_2, AluInp.PREV_DELAY_3)
        for kblk in range(5, 8):
            dp[kblk].pass_through_alu()
        u.enable_output(OutSel.ALU_OUT, OutPath.WR0_LO)
        return [u]

    # ---- 2x (packed fp16) window-start variants -------------------------
    # Semantics: out[j] = f(T[j], T[j+1], T[j+2]) with src0 = T[0:N],
    # src1 = T[2:N+2], all APs fp16 step-1 4B-aligned so the RTL always
    # selects 2x_1p.  Per pair-cycle the crossbar exposes 4 consecutive
    # taps (SRC_0, SRC_0_HI, SRC_1, SRC_1_HI); both parities' windows are
    # pure spatial.  Even result -> WR0_LO, odd -> WR0_HI.

    def _shiftcat(in0, in1):
        # full stream T (len N+2) from the two views
        return np.concatenate([in0, in1[..., -2:]], axis=-1)

    def _hmax3w_ref(in0, in1, c0, c1, c2):
        T = _shiftcat(in0, in1)
        return np.maximum(np.maximum(T[..., :-2], T[..., 1:-1]), T[..., 2:])

    def _hmin3w_ref(in0, in1, c0, c1, c2):
        T = _shiftcat(in0, in1)
        return np.minimum(np.minimum(T[..., :-2], T[..., 1:-1]), T[..., 2:])

    def _hmed3w_ref(in0, in1, c0, c1, c2):
        T = _shiftcat(in0, in1)
        a, b, c = T[..., :-2], T[..., 1:-1], T[..., 2:]
        return np.maximum(np.minimum(a, b),
                          np.minimum(np.maximum(a, b), c))

    def _dummy_1x(op3):
        """Placeholder REGULAR-mode program (never selected: the APs
        always satisfy the 2x_1p conditions)."""
        u = UopConfig()
        u.enable_input(InpSel.SRC_0, 1)
        u.enable_input(InpSel.SRC_1, 2)
        u.require_inp0 = ENABLE
        u.require_inp1 = ENABLE
        u.trigger = (Trigger.SRC_TENSOR_DONE, Trigger.NONE, Trigger.NONE)
        dp = u.datapath_config
        dp[0].enable_alu(op3, AluInp.PREV_DELAY_0, AluInp.PREV_DELAY_1)
        for kblk in range(1, 8):
            dp[kblk].pass_through_alu()
        u.enable_output(OutSel.ALU_OUT, OutPath.WR0_LO)
        return [u]

    def _mm3_w2x_uops(op3):
        """2x program: E = op3(S0, S0H, S1) -> WR0_LO, O = op3(S0H, S1,
        S1H) -> WR0_HI."""
        u = UopConfig()
        u.enable_input(InpSel.SRC_0, 1)      # chain 0
        u.enable_input(InpSel.SRC_0_HI, 2)   # chain 1
        u.enable_input(InpSel.SRC_1, 3)      # chain 2
        u.enable_input(InpSel.SRC_1_HI, 4)   # chain 3
        u.require_inp0 = ENABLE
        u.require_inp1 = ENABLE
        u.trigger = (Trigger.SRC_TENSOR_DONE, Trigger.NONE, Trigger.NONE)
        dp = u.datapath_config
        dp[0].enable_alu(op3, AluInp.PREV_DELAY_0, AluInp.PREV_DELAY_1)
        dp[0].pass_through_delay(1, 2, 3)
        dp[1].enable_alu(op3, AluInp.PREV_ALU_OUT, AluInp.PREV_DELAY_2)
        dp[1].pass_through_delay(1, 2, 3)
        dp[2].enable_alu(op3, AluInp.PREV_DELAY_1, AluInp.PREV_DELAY_2)
        dp[2].enable_delay_from_src(DelayInp.PREV_ALU_OUT, 4)   # E
        dp[2].pass_through_delay(3)
        dp[3].enable_alu(op3, AluInp.PREV_ALU_OUT, AluInp.PREV_DELAY_3)
        dp[3].pass_through_delay(4)
        for kblk in range(4, 8):
            dp[kblk].pass_through_alu().pass_through_delay(4)
        u.enable_output(OutSel.DELAY_4, OutPath.WR0_LO)
        u.enable_output(OutSel.ALU_OUT, OutPath.WR0_HI)
        return [u]

    def _med3_w2x_uops():
        """2x program: E = med3(S0, S0H, S1) -> WR0_LO, O = med3(S0H, S1,
        S1H) -> WR0_HI.  Exactly 8 ALU blocks."""
        u = UopConfig()
        u.enable_input(InpSel.SRC_0, 1)      # chain 0
        u.enable_input(InpSel.SRC_0_HI, 2)   # chain 1
        u.enable_input(InpSel.SRC_1, 3)      # chain 2
        u.enable_input(InpSel.SRC_1_HI, 4)   # chain 3
        u.require_inp0 = ENABLE
        u.require_inp1 = ENABLE
        u.trigger = (Trigger.SRC_TENSOR_DONE, Trigger.NONE, Trigger.NONE)
        dp = u.datapath_config
        # E-half: med3(d0, d1, d2)
        dp[0].enable_alu(AluOp.MIN, AluInp.PREV_DELAY_0, AluInp.PREV_DELAY_1)
        dp[0].pass_through_delay(0, 1, 2, 3)
        dp[1].enable_alu(AluOp.MAX, AluInp.PREV_DELAY_0, AluInp.PREV_DELAY_1)
        dp[1].enable_delay_from_src(DelayInp.PREV_ALU_OUT, 4)   # pE
        dp[1].pass_through_delay(1, 2, 3)
        dp[2].enable_alu(AluOp.MIN, AluInp.PREV_ALU_OUT, AluInp.PREV_DELAY_2)
        dp[2].pass_through_delay(1, 2, 3, 4)
        dp[3].enable_alu(AluOp.MAX, AluInp.PREV_DELAY_4, AluInp.PREV_ALU_OUT)
        dp[3].pass_through_delay(1, 2, 3)
        # O-half: med3(d1, d2, d3)
        dp[4].enable_alu(AluOp.MIN, AluInp.PREV_DELAY_1, AluInp.PREV_DELAY_2)
        dp[4].enable_delay_from_src(DelayInp.PREV_ALU_OUT, 4)   # E
        dp[4].pass_through_delay(1, 2, 3)
        dp[5].enable_alu(AluOp.MAX, AluInp.PREV_DELAY_1, AluInp.PREV_DELAY_2)
        dp[5].enable_delay_from_src(DelayInp.PREV_ALU_OUT, 5)   # pO
        dp[5].pass_through_delay(3, 4)
        dp[6].enable_alu(AluOp.MIN, AluInp.PREV_ALU_OUT, AluInp.PREV_DELAY_3)
        dp[6].pass_through_delay(4, 5)
        dp[7].enable_alu(AluOp.MAX, AluInp.PREV_DELAY_5, AluInp.PREV_ALU_OUT)
        dp[7].pass_through_delay(4)
        u.enable_output(OutSel.DELAY_4, OutPath.WR0_LO)
        u.enable_output(OutSel.ALU_OUT, OutPath.WR0_HI)
        return [u]

    class _SlidingOp:
        """Duck-typed DveOp: hand-authored uops instead of lower(spec)."""

        def __init__(self, name, reference, uops_builder, uops_2x_builder=None):
            # body is a placeholder that reads Src0+Src1 (rd1_en) and no
            # C2/C3; only `reference` is ever evaluated (bass_interp).
            self.name = name
            self.spec = Spec(body=minn(Src0, Src1), reference=reference)
            self.subdim = False
            self.perf_en = {}
            self._builder = uops_builder
            self._builder_2x = uops_2x_builder
            self._cache = {}

        def compile(self, ver):
            if ver not in self._cache:
                s = DveOpSpec(
                    name=self.name,
                    opcode=dve_ops.get_dve_sub_opcode(self.name),
                    uops=self._builder(),
                    uops_2x=(self._builder_2x() if self._builder_2x
                             else None),
                    perf_max=1 if self._builder_2x else 0,
                    rd1_en=True,
                )
                s.validate(ver)
                self._cache[ver] = s
            return self._cache[ver]

    defs = [
        ("HMAX3_ANT", _hmax3_ref, lambda: _mm3_uops(AluOp.MAX), None),
        ("HMIN3_ANT", _hmin3_ref, lambda: _mm3_uops(AluOp.MIN), None),
        ("HMED3_ANT", _hmed3_ref, _med3_uops, None),
        ("HMAX3W_ANT", _hmax3w_ref, lambda: _dummy_1x(AluOp.MAX),
         lambda: _mm3_w2x_uops(AluOp.MAX)),
        ("HMIN3W_ANT", _hmin3w_ref, lambda: _dummy_1x(AluOp.MIN),
         lambda: _mm3_w2x_uops(AluOp.MIN)),
        ("HMED3W_ANT", _hmed3w_ref, lambda: _dummy_1x(AluOp.MIN),
         _med3_w2x_uops),
    ]
    ops = {}
    for name, ref, builder, builder2x in defs:
        if name not in dve_ops._SUB_OPCODE_FOR_NAME:
            row = max(dve_ops._SUB_OPCODE_FOR_NAME.values()) + 1
            assert row < 0x20, "custom DVE opcode rows exhausted"
            dve_ops._SUB_OPCODE_FOR_NAME[name] = row
        op = _SlidingOp(name, ref, builder, builder2x)
        # replace any previous registration (idempotent across reloads)
        dve_ops.OPS[:] = [o for o in dve_ops.OPS if o.name != name] + [op]
        dve_ops.CUSTOM_DVE_SPECS[name] = op.spec
        ops[name] = op
    _OPS = ops
    return ops


def _emit_custom_dve(v, op, out, in0, in1, perf_max=0):
    """Like bass.Vector._custom_dve but with perf_max set at construction
    (the Tile scheduler drops post-hoc attribute edits)."""
    import concourse.bass_isa as bass_isa
    import concourse.mybir as mybir
    from concourse.dve_ops import get_dve_sub_opcode

    b = v.bass
    if op.name not in b.m.ant_custom_dve_ops:
        b.m.ant_custom_dve_ops = sorted({*b.m.ant_custom_dve_ops, op.name})
    shape = bass_isa.CustomDveShape.TTSS
    isa_opcode = b.isa.Opcode[
        f"NEURON_ISA_TPB_OPCODE_CUSTOM_DVE_ANT_{shape.slot()}"
    ].value
    imm = lambda: mybir.ImmediateValue(dtype=mybir.dt.float32, value=0.0)
    return v.add_instruction(
        bass_isa.InstCustomDveAnt(
            name=b.get_next_instruction_name(),
            op_name=op.name,
            rd1_en=True,
            subdim=0,
            imm2=0.0,
            shape=shape,
            row=get_dve_sub_opcode(op.name),
            perf_max=perf_max,
            isa_opcode=isa_opcode,
            ins=[v.lower_ap(in0, for_isa=True),
                 v.lower_ap(in1, for_isa=True), imm(), imm()],
            outs=[v.lower_ap(out, for_isa=True)],
        )
    )


def _build_v3(R=8, final_gp_frac=0.65, n_vert_gp=1, in_bufs=2, mrg_bufs=1,
              x16_bufs=2, dve_tail=True, sliding2x=True, use_gp=False):
    """fp16 + custom sliding ops + ACT casts + GPSIMD final stage.

    Software-pipelined emission (engines execute their streams in order):
    per iteration k we emit  A(k+1) = load+cast of the NEXT chunk,
    D(k-1) = store of the PREVIOUS chunk, then B(k) = DVE work and
    C(k) = final stage.  This keeps the ACT in-cast ahead of the
    GPSIMD-dependent out-cast in the ACT/sync instruction streams.

    n_final_gp: how many of the 4 final med3 tensor_tensor ops run on
    GPSIMD (rest on DVE).  n_vert_gp: how many vertical ops on GPSIMD.
    dve_tail: run the last chunk's final stage on the DVE (shorter tail).
    """
    import concourse.bacc as bacc
    import concourse.mybir as mybir
    from concourse.tile import TileContext

    ops = _register_custom_ops()
    MIN = mybir.AluOpType.min
    MAX = mybir.AluOpType.max
    f32 = mybir.dt.float32
    f16 = mybir.dt.float16

    assert HH % R == 0 and R % 2 == 0
    K = HH // R
    Rh = R // 2
    NS = 2 + R * WP          # sliding stream length (2 prefix + rows)
    LSZ = R * WP + 4         # sliding tile: prefix 2 + rows + tail 2

    nc = bacc.Bacc("TRN2", name="median_pool2d_v3")
    x = nc.dram_tensor("x", [C, H, W], f32, kind="ExternalInput")
    out = nc.dram_tensor("out", [C, H, W], f32, kind="ExternalOutput")
    xg = x.ap()
    og = out.ap()

    with TileContext(nc) as tc:
        with (
            tc.tile_pool(name="in32", bufs=in_bufs) as in_pool,
            tc.tile_pool(name="x16p", bufs=x16_bufs) as x16_pool,
            tc.tile_pool(name="vert", bufs=1) as v_pool,
            tc.tile_pool(name="mrg", bufs=mrg_bufs) as m_pool,
            tc.tile_pool(name="fin", bufs=1) as f_pool,
            tc.tile_pool(name="ot16", bufs=2) as ot_pool,
            tc.tile_pool(name="out32", bufs=1) as o32_pool,
        ):
            # persistent sliding tiles: prefix/tail zeros written once
            def stile(name):
                t = v_pool.tile([128, LSZ], f16, name=name, tag=name)
                rows = t[:, 2:2 + R * WP].rearrange("p (r w) -> p r w", w=WP)
                return t, rows

            Lo_t, Lo3 = stile("Lo")
            Me_t, Me3 = stile("Me")
            Hi_t, Hi3 = stile("Hi")
            for t in (Lo_t, Me_t, Hi_t):
                nc.gpsimd.memset(t[:, 0:2], 0.0)
                nc.gpsimd.memset(t[:, 2 + R * WP:LSZ], 0.0)

            def vtile(name, rows):
                t = v_pool.tile([128, rows * WP], f16, name=name, tag=name)
                return t.rearrange("p (r w) -> p r w", w=WP)

            Pm = vtile("Pm", Rh)
            PM = vtile("PM", Rh)
            tEv = vtile("tE", Rh)
            tOv = vtile("tO", Rh)

            def stage_a(k):
                """DMA fp32 rows + ACT cast to padded fp16 rows."""
                r0 = k * R
                s32 = in_pool.tile([128, (R + 2) * W], f32, name="s32",
                                   tag="s32")
                s3 = s32.rearrange("p (r w) -> p r w", w=W)
                if k == 0:
                    nc.vector.memset(s3[0:64, 0:1], 0.0)
                    nc.sync.dma_start(out=s3[0:64, 1:R + 2],
                                      in_=xg[:, 0:R + 1, :])
                else:
                    nc.sync.dma_start(out=s3[0:64],
                                      in_=xg[:, r0 - 1:r0 + R + 1, :])
                if k == K - 1:
                    nc.vector.memset(s3[64:128, R + 1:R + 2], 0.0)
                    nc.sync.dma_start(out=s3[64:128, 0:R + 1],
                                      in_=xg[:, HH + r0 - 1:H, :])
                else:
                    nc.sync.dma_start(out=s3[64:128],
                                      in_=xg[:, HH + r0 - 1:HH + r0 + R + 1, :])
                x16 = x16_pool.tile([128, (R + 2) * WP], f16, name="x16",
                                    tag="x16")
                X = x16.rearrange("p (r w) -> p r w", w=WP)
                nc.gpsimd.memset(X[:, :, W:WP], 0.0)
                nc.scalar.copy(out=X[:, :, 0:W], in_=s3[:])
                return X

            def stage_bc(k, X):
                """DVE vertical + sliding, then final stage."""
                # ---- vertical sort3 (fp16 TT, 2x mode, pair-shared) ----
                vops = [
                    (Pm, X[:, 1:R + 1:2], X[:, 2:R + 2:2], MIN),
                    (PM, X[:, 1:R + 1:2], X[:, 2:R + 2:2], MAX),
                    (Lo3[:, 0:R:2], X[:, 0:R:2], Pm, MIN),
                    (Hi3[:, 0:R:2], X[:, 0:R:2], PM, MAX),
                    (tEv, X[:, 0:R:2], PM, MIN),
                    (Me3[:, 0:R:2], Pm, tEv, MAX),
                    (Lo3[:, 1:R:2], X[:, 3:R + 2:2], Pm, MIN),
                    (Hi3[:, 1:R:2], X[:, 3:R + 2:2], PM, MAX),
                    (tOv, X[:, 3:R + 2:2], PM, MIN),
                    (Me3[:, 1:R:2], Pm, tOv, MAX),
                ]
                for i, (o, a, b, alu) in enumerate(vops):
                    eng = (nc.gpsimd if i >= len(vops) - n_vert_gp
                           else nc.vector)
                    eng.tensor_tensor(out=o, in0=a, in1=b, op=alu)

                # ---- horizontal 3-tap merge: custom sliding DVE ops ----
                mA_t = m_pool.tile([128, LSZ], f16, name="mA", tag="mA")
                mB_t = m_pool.tile([128, LSZ], f16, name="mB", tag="mB")
                mC_t = m_pool.tile([128, LSZ], f16, name="mC", tag="mC")
                if sliding2x:
                    # window-start form at 2x: out[j] = f(T[j..j+2]);
                    # all APs even-offset fp16 step-1 -> RTL picks 2x_1p.
                    N2 = R * WP
                    for opname, src, dst in (
                        ("HMAX3W_ANT", Lo_t, mA_t),
                        ("HMED3W_ANT", Me_t, mB_t),
                        ("HMIN3W_ANT", Hi_t, mC_t),
                    ):
                        _emit_custom_dve(
                            nc.vector, ops[opname], out=dst[:, 0:N2],
                            in0=src[:, 0:N2], in1=src[:, 2:N2 + 2],
                            perf_max=1)
                    off = 0   # outputs shifted one left; final reads from 0
                else:
                    nc.vector._custom_dve(ops["HMAX3_ANT"],
                                          out=mA_t[:, 0:NS],
                                          in0=Lo_t[:, 0:NS],
                                          in1=Lo_t[:, 1:NS + 1])
                    nc.vector._custom_dve(ops["HMIN3_ANT"],
                                          out=mC_t[:, 0:NS],
                                          in0=Hi_t[:, 0:NS],
                                          in1=Hi_t[:, 1:NS + 1])
                    nc.vector._custom_dve(ops["HMED3_ANT"],
                                          out=mB_t[:, 0:NS],
                                          in0=Me_t[:, 0:NS],
                                          in1=Me_t[:, 1:NS + 1])
                    off = 2

                # ---- final med3(mA, mB, mC) ----
                mT = f_pool.tile([128, R * WP], f16, name="mT", tag="mT")
                mU = f_pool.tile([128, R * WP], f16, name="mU", tag="mU")
                mV = m_pool.tile([128, LSZ], f16, name="mV", tag="mA")
                ot = ot_pool.tile([128, R * WP + 2], f16, name="ot",
                                  tag="ot")
                Af = mA_t[:, off:off + R * WP]
                Bf = mB_t[:, off:off + R * WP]
                Cf = mC_t[:, off:off + R * WP]
                # split the 4-op chain by element range: each engine runs an
                # independent chain over its own slice (no cross-engine deps)
                g = 0.0 if (dve_tail and k == K - 1) else final_gp_frac
                S = 2 * int(R * WP * (1.0 - g) / 2)
                for eng, s0, s1 in ((nc.vector, 0, S),
                                    (nc.gpsimd, S, R * WP)):
                    if s1 <= s0:
                        continue
                    sl = slice(s0, s1)
                    osl = slice(off + s0, off + s1)
                    eng.tensor_tensor(out=mT[:, sl], in0=mA_t[:, osl],
                                      in1=mB_t[:, osl], op=MIN)
                    eng.tensor_tensor(out=mU[:, sl], in0=mA_t[:, osl],
                                      in1=mB_t[:, osl], op=MAX)
                    eng.tensor_tensor(out=mV[:, sl], in0=mU[:, sl],
                                      in1=mC_t[:, osl], op=MIN)
                    eng.tensor_tensor(out=ot[:, sl], in0=mT[:, sl],
                                      in1=mV[:, sl], op=MAX)
                return ot

            def stage_bc_dve(k, X):
                """All-DVE variant: GPSIMD tensor_tensor min/max is
                rejected by the walrus codegen, so everything runs on the
                DVE; dead Lo/Me/Hi buffers are re-used for the final
                temporaries (same engine, in-order, so aliasing is free)."""
                for o, a, b, alu in (
                    (Pm, X[:, 1:R + 1:2], X[:, 2:R + 2:2], MIN),
                    (PM, X[:, 1:R + 1:2], X[:, 2:R + 2:2], MAX),
                    (Lo3[:, 0:R:2], X[:, 0:R:2], Pm, MIN),
                    (Hi3[:, 0:R:2], X[:, 0:R:2], PM, MAX),
                    (tEv, X[:, 0:R:2], PM, MIN),
                    (Me3[:, 0:R:2], Pm, tEv, MAX),
                    (Lo3[:, 1:R:2], X[:, 3:R + 2:2], Pm, MIN),
                    (Hi3[:, 1:R:2], X[:, 3:R + 2:2], PM, MAX),
                    (tOv, X[:, 3:R + 2:2], PM, MIN),
                    (Me3[:, 1:R:2], Pm, tOv, MAX),
                ):
                    nc.vector.tensor_tensor(out=o, in0=a, in1=b, op=alu)

                mA_t = m_pool.tile([128, LSZ], f16, name="mA", tag="mA")
                mB_t = m_pool.tile([128, LSZ], f16, name="mB", tag="mB")
                mC_t = m_pool.tile([128, LSZ], f16, name="mC", tag="mC")
                N2 = R * WP
                for opname, src, dst in (
                    ("HMAX3W_ANT", Lo_t, mA_t),
                    ("HMED3W_ANT", Me_t, mB_t),
                    ("HMIN3W_ANT", Hi_t, mC_t),
                ):
                    _emit_custom_dve(
                        nc.vector, ops[opname], out=dst[:, 0:N2],
                        in0=src[:, 0:N2], in1=src[:, 2:N2 + 2], perf_max=1)

                # final med3 (all ops even-offset fp16 step-1 -> 2x mode)
                mT = f_pool.tile([128, N2], f16, name="mT", tag="mT")
                mU = f_pool.tile([128, N2], f16, name="mU", tag="mU")
                mV = f_pool.tile([128, N2], f16, name="mV", tag="mV")
                ot = ot_pool.tile([128, R * WP + 2], f16, name="ot",
                                  tag="ot")
                sl = slice(0, N2)
                nc.vector.tensor_tensor(out=mT[:], in0=mA_t[:, sl],
                                        in1=mB_t[:, sl], op=MIN)
                nc.vector.tensor_tensor(out=mU[:], in0=mA_t[:, sl],
                                        in1=mB_t[:, sl], op=MAX)
                nc.vector.tensor_tensor(out=mV[:], in0=mU[:],
                                        in1=mC_t[:, sl], op=MIN)
                nc.vector.tensor_tensor(out=ot[:, sl], in0=mT[:],
                                        in1=mV[:], op=MAX)
                return ot

            ot_off = 1 if sliding2x else 0

            def stage_d(k, ot):
                """ACT cast fp16 -> fp32 + DMA out."""
                r0 = k * R
                o32 = o32_pool.tile([128, R * W], f32, name="o32", tag="o32")
                o3 = o32.rearrange("p (r w) -> p r w", w=W)
                ot3 = ot[:, ot_off:ot_off + R * WP].rearrange(
                    "p (r w) -> p r w", w=WP)
                nc.scalar.copy(out=o3[:], in_=ot3[:, :, 0:W])
                # separate DGE queue from the input loads (sync) so stores
                # never head-of-line-block the next loads
                nc.scalar.dma_start(out=og[:, r0:r0 + R, :], in_=o3[0:64])
                nc.scalar.dma_start(out=og[:, HH + r0:HH + r0 + R, :],
                                    in_=o3[64:128])

            Xs = {0: stage_a(0)}
            ots = {}
            ddel = 1 if use_gp else 0   # store-delay (GPSIMD-final only)
            for k in range(K):
                if k + 1 < K:
                    Xs[k + 1] = stage_a(k + 1)
                if k - ddel in ots:
                    stage_d(k - ddel, ots.pop(k - ddel))
                ots[k] = (stage_bc(k, Xs.pop(k)) if use_gp
                          else stage_bc_dve(k, Xs.pop(k)))
                if ddel == 0:
                    stage_d(k, ots.pop(k))
            for kk in sorted(ots):
                stage_d(kk, ots.pop(kk))

    nc.compile()
    return nc


def _build_v4(sizes=(4, 8, 16, 16, 16, 16, 16, 16, 12, 8), in_bufs=2,
              x16_bufs=2):
    """All-DVE variant with heterogeneous chunk sizes: small first/last
    chunks shorten the pipeline ramp/tail, large middle chunks amortize
    per-instruction overhead."""
    import concourse.bacc as bacc
    import concourse.mybir as mybir
    from concourse.tile import TileContext

    ops = _register_custom_ops()
    MIN = mybir.AluOpType.min
    MAX = mybir.AluOpType.max
    f32 = mybir.dt.float32
    f16 = mybir.dt.float16

    sizes = list(sizes)
    assert sum(sizes) == HH and all(s % 2 == 0 for s in sizes)
    K = len(sizes)
    r0s = [sum(sizes[:i]) for i in range(K)]
    RM = max(sizes)
    LSZ = RM * WP + 4

    nc = bacc.Bacc("TRN2", name="median_pool2d_v4")
    x = nc.dram_tensor("x", [C, H, W], f32, kind="ExternalInput")
    out = nc.dram_tensor("out", [C, H, W], f32, kind="ExternalOutput")
    xg = x.ap()
    og = out.ap()

    with TileContext(nc) as tc:
        with (
            tc.tile_pool(name="in32", bufs=in_bufs) as in_pool,
            tc.tile_pool(name="x16p", bufs=x16_bufs) as x16_pool,
            tc.tile_pool(name="vert", bufs=1) as v_pool,
            tc.tile_pool(name="mrg", bufs=1) as m_pool,
            tc.tile_pool(name="fin", bufs=1) as f_pool,
            tc.tile_pool(name="ot16", bufs=2) as ot_pool,
            tc.tile_pool(name="out32", bufs=1) as o32_pool,
        ):
            def stile(name):
                return v_pool.tile([128, LSZ], f16, name=name, tag=name)

            Lo_t = stile("Lo")
            Me_t = stile("Me")
            Hi_t = stile("Hi")
            for t in (Lo_t, Me_t, Hi_t):
                nc.gpsimd.memset(t[:, 0:2], 0.0)
                # zero every distinct chunk-size tail once; stale row data
                # at a smaller-chunk tail only feeds pad-column junk
                for rc in sorted(set(sizes)):
                    nc.gpsimd.memset(t[:, 2 + rc * WP:2 + rc * WP + 2], 0.0)

            def vtile(name):
                return v_pool.tile([128, (RM // 2) * WP], f16, name=name,
                                   tag=name)

            Pm_t, PM_t, tE_t, tO_t = (vtile(n) for n in
                                      ("Pm", "PM", "tE", "tO"))

            def stage_a(k):
                rc = sizes[k]
                r0 = r0s[k]
                s32 = in_pool.tile([128, (RM + 2) * W], f32, name="s32",
                                   tag="s32")
                s3 = s32[:, 0:(rc + 2) * W].rearrange("p (r w) -> p r w",
                                                      w=W)
                if k == 0:
                    nc.gpsimd.memset(s3[0:64, 0:1], 0.0)
                    nc.sync.dma_start(out=s3[0:64, 1:rc + 2],
                                      in_=xg[:, 0:rc + 1, :])
                else:
                    nc.sync.dma_start(out=s3[0:64],
                                      in_=xg[:, r0 - 1:r0 + rc + 1, :])
                if k == K - 1:
                    nc.gpsimd.memset(s3[64:128, rc + 1:rc + 2], 0.0)
                    nc.sync.dma_start(out=s3[64:128, 0:rc + 1],
                                      in_=xg[:, HH + r0 - 1:H, :])
                else:
                    nc.sync.dma_start(
                        out=s3[64:128],
                        in_=xg[:, HH + r0 - 1:HH + r0 + rc + 1, :])
                x16 = x16_pool.tile([128, (RM + 2) * WP], f16, name="x16",
                                    tag="x16")
                X = x16[:, 0:(rc + 2) * WP].rearrange("p (r w) -> p r w",
                                                      w=WP)
                nc.gpsimd.memset(X[:, :, W:WP], 0.0)
                nc.scalar.copy(out=X[:, :, 0:W], in_=s3[:])
                return X

            def stage_b(k, X):
                rc = sizes[k]
                N2 = rc * WP
                Lo3, Me3, Hi3 = (
                    t[:, 2:2 + N2].rearrange("p (r w) -> p r w", w=WP)
                    for t in (Lo_t, Me_t, Hi_t))
                Pm, PM, tEv, tOv = (
                    t[:, 0:(rc // 2) * WP].rearrange("p (r w) -> p r w",
                                                     w=WP)
                    for t in (Pm_t, PM_t, tE_t, tO_t))
                for o, a, b, alu in (
                    (Pm, X[:, 1:rc + 1:2], X[:, 2:rc + 2:2], MIN),
                    (PM, X[:, 1:rc + 1:2], X[:, 2:rc + 2:2], MAX),
                    (Lo3[:, 0:rc:2], X[:, 0:rc:2], Pm, MIN),
                    (Hi3[:, 0:rc:2], X[:, 0:rc:2], PM, MAX),
                    (tEv, X[:, 0:rc:2], PM, MIN),
                    (Me3[:, 0:rc:2], Pm, tEv, MAX),
                    (Lo3[:, 1:rc:2], X[:, 3:rc + 2:2], Pm, MIN),
                    (Hi3[:, 1:rc:2], X[:, 3:rc + 2:2], PM, MAX),
                    (tOv, X[:, 3:rc + 2:2], PM, MIN),
                    (Me3[:, 1:rc:2], Pm, tOv, MAX),
                ):
                    nc.vector.tensor_tensor(out=o, in0=a, in1=b, op=alu)

                mA_t = m_pool.tile([128, LSZ], f16, name="mA", tag="mA")
                mB_t = m_pool.tile([128, LSZ], f16, name="mB", tag="mB")
                mC_t = m_pool.tile([128, LSZ], f16, name="mC", tag="mC")
                for opname, src, dst in (
                    ("HMAX3W_ANT", Lo_t, mA_t),
                    ("HMED3W_ANT", Me_t, mB_t),
                    ("HMIN3W_ANT", Hi_t, mC_t),
                ):
                    _emit_custom_dve(
                        nc.vector, ops[opname], out=dst[:, 0:N2],
                        in0=src[:, 0:N2], in1=src[:, 2:N2 + 2], perf_max=1)

                mT = f_pool.tile([128, RM * WP], f16, name="mT", tag="mT")
                mU = f_pool.tile([128, RM * WP], f16, name="mU", tag="mU")
                mV = f_pool.tile([128, RM * WP], f16, name="mV", tag="mV")
                ot = ot_pool.tile([128, RM * WP + 2], f16, name="ot",
                                  tag="ot")
                sl = slice(0, N2)
                nc.vector.tensor_tensor(out=mT[:, sl], in0=mA_t[:, sl],
                                        in1=mB_t[:, sl], op=MIN)
                nc.vector.tensor_tensor(out=mU[:, sl], in0=mA_t[:, sl],
                                        in1=mB_t[:, sl], op=MAX)
                nc.vector.tensor_tensor(out=mV[:, sl], in0=mU[:, sl],
                                        in1=mC_t[:, sl], op=MIN)
                nc.vector.tensor_tensor(out=ot[:, sl], in0=mT[:, sl],
                                        in1=mV[:, sl], op=MAX)
                return ot

            def stage_d(k, ot):
                rc = sizes[k]
                r0 = r0s[k]
                o32 = o32_pool.tile([128, RM * W], f32, name="o32",
                                    tag="o32")
                o3 = o32[:, 0:rc * W].rearrange("p (r w) -> p r w", w=W)
                ot3 = ot[:, 1:1 + rc * WP].rearrange("p (r w) -> p r w",
                                                     w=WP)
                nc.scalar.copy(out=o3[:], in_=ot3[:, :, 0:W])
                nc.scalar.dma_start(out=og[:, r0:r0 + rc, :], in_=o3[0:64])
                nc.scalar.dma_start(out=og[:, HH + r0:HH + r0 + rc, :],
                                    in_=o3[64:128])

            Xs = {0: stage_a(0)}
            for k in range(K):
                if k + 1 < K:
                    Xs[k + 1] = stage_a(k + 1)
                ot = stage_b(k, Xs.pop(k))
                stage_d(k, ot)

    nc.compile()
    return nc


def _build_shared(R=16, gp_frac=0.0, dtype="float32", in_bufs=None,
                  out_bufs=None):
    """Fallback: 15-op/pixel fp32 TT network (previous working kernel)."""
    import concourse.bacc as bacc
    import concourse.mybir as mybir
    from concourse.tile import TileContext

    MIN = mybir.AluOpType.min
    MAX = mybir.AluOpType.max
    f32 = mybir.dt.float32
    cdt = getattr(mybir.dt, dtype)

    WPP = W + 2
    assert HH % R == 0 and R % 2 == 0
    K = HH // R
    Rh = R // 2

    nc = bacc.Bacc("TRN2", name="median_pool2d_s")
    x = nc.dram_tensor("x", [C, H, W], f32, kind="ExternalInput")
    out = nc.dram_tensor("out", [C, H, W], f32, kind="ExternalOutput")
    xg = x.ap()
    og = out.ap()

    def tt(out_ap, in0, in1, op):
        nc.vector.tensor_tensor(out=out_ap, in0=in0, in1=in1, op=op)

    if in_bufs is None:
        in_bufs = 3 if R <= 8 else 2
    if out_bufs is None:
        out_bufs = 3 if R <= 8 else 1
    with TileContext(nc) as tc:
        with (
            tc.tile_pool(name="io_in", bufs=in_bufs) as in_pool,
            tc.tile_pool(name="io_out", bufs=out_bufs) as out_pool,
            tc.tile_pool(name="work", bufs=1) as w_pool,
        ):
            def wtile(name, rows, width, tag=None):
                t = w_pool.tile([128, rows * width], cdt, name=name,
                                tag=tag or name)
                return t.rearrange("p (r w) -> p r w", w=width)

            for k in range(K):
                r0 = k * R
                it = in_pool.tile([128, (R + 2) * WPP], cdt, name="it",
                                  tag="it")
                it3 = it.rearrange("p (r w) -> p r w", w=WPP)
                nc.vector.memset(it3[:, :, 0:WPP:WPP - 1], 0.0)
                if k == 0:
                    nc.vector.memset(it3[0:64, 0:1, 1:W + 1], 0.0)
                    nc.sync.dma_start(out=it3[0:64, 1:R + 2, 1:W + 1],
                                      in_=xg[:, 0:R + 1, :])
                else:
                    nc.sync.dma_start(out=it3[0:64, :, 1:W + 1],
                                      in_=xg[:, r0 - 1:r0 + R + 1, :])
                if k == K - 1:
                    nc.vector.memset(it3[64:128, R + 1:R + 2, 1:W + 1], 0.0)
                    nc.sync.dma_start(out=it3[64:128, 0:R + 1, 1:W + 1],
                                      in_=xg[:, HH + r0 - 1:H, :])
                else:
                    nc.sync.dma_start(out=it3[64:128, :, 1:W + 1],
                                      in_=xg[:, HH + r0 - 1:HH + r0 + R + 1, :])

                Pm = wtile("Pm", Rh, WPP)
                PM = wtile("PM", Rh, WPP)
                tt(Pm, it3[:, 1:R + 1:2, :], it3[:, 2:R + 2:2, :], MIN)
                tt(PM, it3[:, 1:R + 1:2, :], it3[:, 2:R + 2:2, :], MAX)

                Lo3 = wtile("Lo", R, WPP)
                Me3 = wtile("Me", R, WPP)
                Hi3 = wtile("Hi", R, WPP)
                tE = wtile("tE", Rh, WPP)
                tO = wtile("tO", Rh, WPP)
                a_e = it3[:, 0:R:2, :]
                a_o = it3[:, 3:R + 2:2, :]
                tt(Lo3[:, 0:R:2], a_e, Pm, MIN)
                tt(Hi3[:, 0:R:2], a_e, PM, MAX)
                tt(tE, a_e, PM, MIN)
                tt(Me3[:, 0:R:2], Pm, tE, MAX)
                tt(Lo3[:, 1:R:2], a_o, Pm, MIN)
                tt(Hi3[:, 1:R:2], a_o, PM, MAX)
                tt(tO, a_o, PM, MIN)
                tt(Me3[:, 1:R:2], Pm, tO, MAX)

                NP = W // 2 + 1
                PA = wtile("PA", R, NP, tag="Pm")
                PC = wtile("PC", R, NP, tag="PM")
                Um = wtile("Um", R, NP, tag="tE")
                Vm = wtile("Vm", R, NP, tag="tO")
                tBe = wtile("tBe", R, W // 2, tag="Pm")
                tBo = wtile("tBo", R, W // 2, tag="PM")
                mA = wtile("mA", R, W)
                mB = wtile("mB", R, W)
                mC = wtile("mC", R, W)

                ev = slice(0, WPP, 2)
                od = slice(1, WPP, 2)
                tt(PA, Lo3[:, :, ev], Lo3[:, :, od], MAX)
                tt(mA[:, :, 0:W:2], PA[:, :, 0:NP - 1], Lo3[:, :, 2:WPP:2],
                   MAX)
                tt(mA[:, :, 1:W:2], PA[:, :, 1:NP], Lo3[:, :, 1:WPP - 2:2],
                   MAX)

                tt(PC, Hi3[:, :, ev], Hi3[:, :, od], MIN)
                tt(mC[:, :, 0:W:2], PC[:, :, 0:NP - 1], Hi3[:, :, 2:WPP:2],
                   MIN)
                tt(mC[:, :, 1:W:2], PC[:, :, 1:NP], Hi3[:, :, 1:WPP - 2:2],
                   MIN)

                tt(Um, Me3[:, :, ev], Me3[:, :, od], MIN)
                tt(Vm, Me3[:, :, ev], Me3[:, :, od], MAX)
                tt(tBe, Me3[:, :, 2:WPP:2], Vm[:, :, 0:NP - 1], MIN)
                tt(mB[:, :, 0:W:2], Um[:, :, 0:NP - 1], tBe, MAX)
                tt(tBo, Me3[:, :, 1:WPP - 2:2], Vm[:, :, 1:NP], MIN)
                tt(mB[:, :, 1:W:2], Um[:, :, 1:NP], tBo, MAX)

                mT = wtile("mT", R, W, tag="Lo")
                mU = wtile("mU", R, W, tag="Me")
                mV = wtile("mV", R, W, tag="Hi")
                ot = out_pool.tile([128, R * W], cdt, name="ot", tag="ot")
                ot3 = ot.rearrange("p (r w) -> p r w", w=W)
                tt(mT, mA, mB, MIN)
                tt(mU, mA, mB, MAX)
                tt(mV, mU, mC, MIN)
                tt(ot3, mT, mV, MAX)

                nc.sync.dma_start(out=og[:, r0:r0 + R, :], in_=ot3[0:64])
                nc.sync.dma_start(out=og[:, HH + r0:HH + r0 + R, :],
                                  in_=ot3[64:128])

    nc.compile()
    return nc


def _get_nc(variant="v3", **kw):
    key = (variant, tuple(sorted(kw.items())))
    if key not in _CACHE:
        if variant == "v4":
            _CACHE[key] = _build_v4(**kw)
        elif variant == "v3":
            _CACHE[key] = _build_v3(**kw)
        else:
            _CACHE[key] = _build_shared(**kw)
    return _CACHE[key]


_LAST_NC = None


def kernel(x: np.ndarray) -> np.ndarray:
    """MedianPool2d(3x3, s=1, p=1) on 8 NeuronCores."""
    global _LAST_NC
    from concourse.bass_utils import run_bass_kernel_spmd

    assert x.shape == (B, C, H, W), x.shape
    x = np.ascontiguousarray(x, dtype=np.float32)
    try:
        nc = _get_nc("v4")
    except Exception:
        try:
            nc = _get_nc("v3")
        except Exception:
            nc = _get_nc("shared", R=16)
    _LAST_NC = nc
    in_maps = [{"x": x[i]} for i in range(NCORES)]
    res = run_bass_kernel_spmd(nc, in_maps, core_ids=list(range(NCORES)))
    return np.stack([r["out"] for r in res.results], axis=0)


# revision 33
# speedup vs baseline: 2.3415x; 1.0004x over previous
"""MedianPool2d (3x3, stride 1, zero-pad 1) Trainium2 Bass kernel.

Full input x: (8, 64, 256, 256) fp32.  Sharding: pure data parallel over
batch -> core i processes x[i] (64, 256, 256).

Design (v4: fp16 + custom packed-2x sliding DVE ops):
  - Per-core layout: 128 SBUF partitions = (h, c), h in {0,1} = top/bottom
    128-row half, c = channel.  Heterogeneous chunks of output rows per
    partition (small first/last chunks shorten pipeline ramp/tail).
  - DMA loads fp32 rows into a staging tile; the Activation engine casts
    fp32 -> fp16 into a padded row layout (WP=258: 256 cols + 2 zero pad
    cols).  fp16 is exact for the median network itself (pure min/max
    selection); only the input cast rounds (~2.1e-4 l2 rel, gate 2e-2).
  - Vertical sort3 (Lo/Me/Hi per output row) as fp16 tensor_tensor
    min/max on the DVE with row-pair sharing: all ops full-row step-1
    4B-aligned -> genuine 2x DVE mode (0.52 ns/elem).
  - Horizontal 3-tap merge via three hand-authored custom DVE micro-op
    programs (HMAX3W/HMED3W/HMIN3W, registered into the per-NEFF DVE
    table at runtime): ONE streaming pass each computes
    out[j] = f(T[j], T[j+1], T[j+2]) over the flat row stream.  The APs
    (src0 = T[0:N], src1 = T[2:N+2], both fp16 step-1 4B-aligned) make
    the RTL select 2x_1p packed mode, where the crossbar exposes four
    consecutive taps per cycle (SRC_0, SRC_0_HI, SRC_1, SRC_1_HI) - both
    parities' windows are pure spatial, even result -> WR0_LO, odd ->
    WR0_HI, 2 elem/cycle.  Each row carries 2 trailing zero pad cols so
    the previous row's tail doubles as the next row's left pad; a
    2-element zero prefix seeds row 0.  HW-validated bit-exact.
  - Final med3(mA, mB, mC) as 4 more fp16 2x TTs on the DVE.
    (GPSIMD tensor_tensor min/max and DMA-CCE min/max accumulate are both
    rejected by the walrus codegen, so GPSIMD only does small memsets.)
  - Output cast fp16 -> fp32 on the Activation engine, then DMA out on a
    separate DGE queue from the loads.

Effective cost ~12 TT-op-equivalents/pixel at 2x (vs 15 at 1x fp32 for
the baseline) -> 233.5us/core vs 546.7us baseline (2.34x).
"""

import numpy as np

B, C, H, W = 8, 64, 256, 256
NCORES = 8
HH = H // 2          # rows per half-strip
WP = W + 2           # padded row width (2 trailing zero cols)

_CACHE = {}
_OPS = None


def _register_custom_ops():
    """Register the three sliding-window custom DVE ops (idempotent).

    Each op streams src0 = T[j], src1 = T[j+1] and computes a 3-tap
    window f(T[j-1], T[j], T[j+1]) per output element:
      - T[j], T[j+1] arrive spatially on the two source streams;
      - T[j-1] = src1 two elements back, read via NEXT_ALU_OUT_A from a
        pipeline stage that latches raw src1 into its a-flop each cycle.
    Outputs j=0,1 use pre-instruction flop state (garbage); callers must
    treat the first two output elements as scrap.
    """
    global _OPS
    if _OPS is not None:
        return _OPS

    import concourse.dve_ops as dve_ops
    from concourse.dve_spec import Spec, Src0, Src1, minn
    from concourse.dve_uop import (
        ENABLE,
        AluInp,
        AluOp,
        DelayInp,
        DveOpSpec,
        InpSel,
        OutPath,
        OutSel,
        Trigger,
        UopConfig,
    )

    def _shift2(in1):
        z = np.zeros_like(in1[..., :2])
        return np.concatenate([z, in1[..., :-2]], axis=-1)

    def _hmax3_ref(in0, in1, c0, c1, c2):
        return np.maximum(np.maximum(in0, in1), _shift2(in1))

    def _hmin3_ref(in0, in1, c0, c1, c2):
        return np.minimum(np.minimum(in0, in1), _shift2(in1))

    def _hmed3_ref(in0, in1, c0, c1, c2):
        c = _shift2(in1)
        return np.maximum(np.minimum(in0, in1),
                          np.minimum(np.maximum(in0, in1), c))

    def _mm3_uops(op3):
        """out[j] = op3(src0[j], src1[j], src1[j-2]) single-uop program."""
        u = UopConfig()
        u.enable_input(InpSel.SRC_0, 1)     # -> PREV_DELAY_0 at blk0
        u.enable_input(InpSel.SRC_1, 2)     # -> PREV_DELAY_1 at blk0
        u.require_inp0 = ENABLE
        u.require_inp1 = ENABLE
        u.trigger = (Trigger.SRC_TENSOR_DONE, Trigger.NONE, Trigger.NONE)
        dp = u.datapath_config
        # blk0: m = op3(src0, src1)
        dp[0].enable_alu(op3, AluInp.PREV_DELAY_0, AluInp.PREV_DELAY_1)
        dp[0].pass_through_delay(1)
        # blk1: out = op3(m, src1[j-2])  (blk2's a-flop, prev cycle)
        dp[1].enable_alu(op3, AluInp.PREV_ALU_OUT, AluInp.NEXT_ALU_OUT_A)
        dp[1].pass_through_delay(1)
        # blk2: latch raw src1 into the a-flop; stash out in delay chain 0
        dp[2].enable_alu(AluOp.BYPASS, AluInp.PREV_DELAY_1)
        dp[2].alu_out_a_enable = ENABLE
        dp[2].enable_delay_from_src(DelayInp.PREV_ALU_OUT, 0)
        for kblk in range(3, 8):
            dp[kblk].pass_through_alu().pass_through_delay(0)
        u.enable_output(OutSel.DELAY_0, OutPath.WR0_LO)
        return [u]

    def _med3_uops():
        """out[j] = med3(src0[j], src1[j], src1[j-2]) single-uop program."""
        u = UopConfig()
        u.enable_input(InpSel.SRC_0, 1)     # chain 0
        u.enable_input(InpSel.SRC_1, 2)     # chain 1
        u.require_inp0 = ENABLE
        u.require_inp1 = ENABLE
        u.trigger = (Trigger.SRC_TENSOR_DONE, Trigger.NONE, Trigger.NONE)
        dp = u.datapath_config
        # blk0: p = min(a, b)
        dp[0].enable_alu(AluOp.MIN, AluInp.PREV_DELAY_0, AluInp.PREV_DELAY_1)
        dp[0].pass_through_delay(0, 1)
        # blk1: q = max(a, b); stash p in chain 2
        dp[1].enable_alu(AluOp.MAX, AluInp.PREV_DELAY_0, AluInp.PREV_DELAY_1)
        dp[1].enable_delay_from_src(DelayInp.PREV_ALU_OUT, 2)
        dp[1].pass_through_delay(1)
        # blk2: t = min(q, c) with c = src1[j-2] via blk3's a-flop
        dp[2].enable_alu(AluOp.MIN, AluInp.PREV_ALU_OUT, AluInp.NEXT_ALU_OUT_A)
        dp[2].pass_through_delay(1, 2)
        # blk3: latch raw src1 into a-flop; stash t in chain 3
        dp[3].enable_alu(AluOp.BYPASS, AluInp.PREV_DELAY_1)
        dp[3].alu_out_a_enable = ENABLE
        dp[3].enable_delay_from_src(DelayInp.PREV_ALU_OUT, 3)
        dp[3].pass_through_delay(2)
        # blk4: out = max(p, t)
        dp[4].enable_alu(AluOp.MAX, AluInp.PREV_DELAY_2, AluInp.PREV_DELAY_3)
        for kblk in range(5, 8):
            dp[kblk].pass_through_alu()
        u.enable_output(OutSel.ALU_OUT, OutPath.WR0_LO)
        return [u]

    # ---- 2x (packed fp16) window-start variants -------------------------
    # Semantics: out[j] = f(T[j], T[j+1], T[j+2]) with src0 = T[0:N],
    # src1 = T[2:N+2], all APs fp16 step-1 4B-aligned so the RTL always
    # selects 2x_1p.  Per pair-cycle the crossbar exposes 4 consecutive
    # taps (SRC_0, SRC_0_HI, SRC_1, SRC_1_HI); both parities' windows are
    # pure spatial.  Even result -> WR0_LO, odd -> WR0_HI.

    def _shiftcat(in0, in1):
        # full stream T (len N+2) from the two views
        return np.concatenate([in0, in1[..., -2:]], axis=-1)

    def _hmax3w_ref(in0, in1, c0, c1, c2):
        T = _shiftcat(in0, in1)
        return np.maximum(np.maximum(T[..., :-2], T[..., 1:-1]), T[..., 2:])

    def _hmin3w_ref(in0, in1, c0, c1, c2):
        T = _shiftcat(in0, in1)
        return np.minimum(np.minimum(T[..., :-2], T[..., 1:-1]), T[..., 2:])

    def _hmed3w_ref(in0, in1, c0, c1, c2):
        T = _shiftcat(in0, in1)
        a, b, c = T[..., :-2], T[..., 1:-1], T[..., 2:]
        return np.maximum(np.minimum(a, b),
                          np.minimum(np.maximum(a, b), c))

    def _dummy_1x(op3):
        """Placeholder REGULAR-mode program (never selected: the APs
        always satisfy the 2x_1p conditions)."""
        u = UopConfig()
        u.enable_input(InpSel.SRC_0, 1)
        u.enable_input(InpSel.SRC_1, 2)
        u.require_inp0 = ENABLE
        u.require_inp1 = ENABLE
        u.trigger = (Trigger.SRC_TENSOR_DONE, Trigger.NONE, Trigger.NONE)
        dp = u.datapath_config
        dp[0].enable_alu(op3, AluInp.PREV_DELAY_0, AluInp.PREV_DELAY_1)
        for kblk in range(1, 8):
            dp[kblk].pass_through_alu()
        u.enable_output(OutSel.ALU_OUT, OutPath.WR0_LO)
        return [u]

    def _mm3_w2x_uops(op3):
        """2x program: E = op3(S0, S0H, S1) -> WR0_LO, O = op3(S0H, S1,
        S1H) -> WR0_HI."""
        u = UopConfig()
        u.enable_input(InpSel.SRC_0, 1)      # chain 0
        u.enable_input(InpSel.SRC_0_HI, 2)   # chain 1
        u.enable_input(InpSel.SRC_1, 3)      # chain 2
        u.enable_input(InpSel.SRC_1_HI, 4)   # chain 3
        u.require_inp0 = ENABLE
        u.require_inp1 = ENABLE
        u.trigger = (Trigger.SRC_TENSOR_DONE, Trigger.NONE, Trigger.NONE)
        dp = u.datapath_config
        dp[0].enable_alu(op3, AluInp.PREV_DELAY_0, AluInp.PREV_DELAY_1)
        dp[0].pass_through_delay(1, 2, 3)
        dp[1].enable_alu(op3, AluInp.PREV_ALU_OUT, AluInp.PREV_DELAY_2)
        dp[1].pass_through_delay(1, 2, 3)
        dp[2].enable_alu(op3, AluInp.PREV_DELAY_1, AluInp.PREV_DELAY_2)
        dp[2].enable_delay_from_src(DelayInp.PREV_ALU_OUT, 4)   # E
        dp[2].pass_through_delay(3)
        dp[3].enable_alu(op3, AluInp.PREV_ALU_OUT, AluInp.PREV_DELAY_3)
        dp[3].pass_through_delay(4)
        for kblk in range(4, 8):
            dp[kblk].pass_through_alu().pass_through_delay(4)
        u.enable_output(OutSel.DELAY_4, OutPath.WR0_LO)
        u.enable_output(OutSel.ALU_OUT, OutPath.WR0_HI)
        return [u]

    def _med3_w2x_uops():
        """2x program: E = med3(S0, S0H, S1) -> WR0_LO, O = med3(S0H, S1,
        S1H) -> WR0_HI.  Exactly 8 ALU blocks."""
        u = UopConfig()
        u.enable_input(InpSel.SRC_0, 1)      # chain 0
        u.enable_input(InpSel.SRC_0_HI, 2)   # chain 1
        u.enable_input(InpSel.SRC_1, 3)      # chain 2
        u.enable_input(InpSel.SRC_1_HI, 4)   # chain 3
        u.require_inp0 = ENABLE
        u.require_inp1 = ENABLE
        u.trigger = (Trigger.SRC_TENSOR_DONE, Trigger.NONE, Trigger.NONE)
        dp = u.datapath_config
        # E-half: med3(d0, d1, d2)
        dp[0].enable_alu(AluOp.MIN, AluInp.PREV_DELAY_0, AluInp.PREV_DELAY_1)
        dp[0].pass_through_delay(0, 1, 2, 3)
        dp[1].enable_alu(AluOp.MAX, AluInp.PREV_DELAY_0, AluInp.PREV_DELAY_1)
        dp[1].enable_delay_from_src(DelayInp.PREV_ALU_OUT, 4)   # pE
        dp[1].pass_through_delay(1, 2, 3)
        dp[2].enable_alu(AluOp.MIN, AluInp.PREV_ALU_OUT, AluInp.PREV_DELAY_2)
        dp[2].pass_through_delay(1, 2, 3, 4)
        dp[3].enable_alu(AluOp.MAX, AluInp.PREV_DELAY_4, AluInp.PREV_ALU_OUT)
        dp[3].pass_through_delay(1, 2, 3)
        # O-half: med3(d1, d2, d3)
        dp[4].enable_alu(AluOp.MIN, AluInp.PREV_DELAY_1, AluInp.PREV_DELAY_2)
        dp[4].enable_delay_from_src(DelayInp.PREV_ALU_OUT, 4)   # E
        dp[4].pass_through_delay(1, 2, 3)
        dp[5].enable_alu(AluOp.MAX, AluInp.PREV_DELAY_1, AluInp.PREV_DELAY_2)
        dp[5].enable_delay_from_src(DelayInp.PREV_ALU_OUT, 5)   # pO
        dp[5].pass_through_delay(3, 4)
        dp[6].enable_alu(AluOp.MIN, AluInp.PREV_ALU_OUT, AluInp.PREV_DELAY_3)
        dp[6].pass_through_delay(4, 5)
        dp[7].enable_alu(AluOp.MAX, AluInp.PREV_DELAY_5, AluInp.PREV_ALU_OUT)
        dp[7].pass_through_delay(4)
        u.enable_output(OutSel.DELAY_4, OutPath.WR0_LO)
        u.enable_output(OutSel.ALU_OUT, OutPath.WR0_HI)
        return [u]

    class _SlidingOp:
        """Duck-typed DveOp: hand-authored uops instead of lower(spec)."""

        def __init__(self, name, reference, uops_builder, uops_2x_builder=None):
            # body is a placeholder that reads Src0+Src1 (rd1_en) and no
            # C2/C3; only `reference` is ever evaluated (bass_interp).
            self.name = name
            self.spec = Spec(body=minn(Src0, Src1), reference=reference)
            self.subdim = False
            self.perf_en = {}
            self._builder = uops_builder
            self._builder_2x = uops_2x_builder
            self._cache = {}

        def compile(self, ver):
            if ver not in self._cache:
                s = DveOpSpec(
                    name=self.name,
                    opcode=dve_ops.get_dve_sub_opcode(self.name),
                    uops=self._builder(),
                    uops_2x=(self._builder_2x() if self._builder_2x
                             else None),
                    perf_max=1 if self._builder_2x else 0,
                    rd1_en=True,
                )
                s.validate(ver)
                self._cache[ver] = s
            return self._cache[ver]

    defs = [
        ("HMAX3_ANT", _hmax3_ref, lambda: _mm3_uops(AluOp.MAX), None),
        ("HMIN3_ANT", _hmin3_ref, lambda: _mm3_uops(AluOp.MIN), None),
        ("HMED3_ANT", _hmed3_ref, _med3_uops, None),
        ("HMAX3W_ANT", _hmax3w_ref, lambda: _dummy_1x(AluOp.MAX),
         lambda: _mm3_w2x_uops(AluOp.MAX)),
        ("HMIN3W_ANT", _hmin3w_ref, lambda: _dummy_1x(AluOp.MIN),
         lambda: _mm3_w2x_uops(AluOp.MIN)),
        ("HMED3W_ANT", _hmed3w_ref, lambda: _dummy_1x(AluOp.MIN),
         _med3_w2x_uops),
    ]
    ops = {}
    for name, ref, builder, builder2x in defs:
        if name not in dve_ops._SUB_OPCODE_FOR_NAME:
            row = max(dve_ops._SUB_OPCODE_FOR_NAME.values()) + 1
            assert row < 0x20, "custom DVE opcode rows exhausted"
            dve_ops._SUB_OPCODE_FOR_NAME[name] = row
        op = _SlidingOp(name, ref, builder, builder2x)
        # replace any previous registration (idempotent across reloads)
        dve_ops.OPS[:] = [o for o in dve_ops.OPS if o.name != name] + [op]
        dve_ops.CUSTOM_DVE_SPECS[name] = op.spec
        ops[name] = op
    _OPS = ops
    return ops


def _emit_custom_dve(v, op, out, in0, in1, perf_max=0):
    """Like bass.Vector._custom_dve but with perf_max set at construction
    (the Tile scheduler drops post-hoc attribute edits)."""
    import concourse.bass_isa as bass_isa
    import concourse.mybir as mybir
    from concourse.dve_ops import get_dve_sub_opcode

    b = v.bass
    if op.name not in b.m.ant_custom_dve_ops:
        b.m.ant_custom_dve_ops = sorted({*b.m.ant_custom_dve_ops, op.name})
    shape = bass_isa.CustomDveShape.TTSS
    isa_opcode = b.isa.Opcode[
        f"NEURON_ISA_TPB_OPCODE_CUSTOM_DVE_ANT_{shape.slot()}"
    ].value
    imm = lambda: mybir.ImmediateValue(dtype=mybir.dt.float32, value=0.0)
    return v.add_instruction(
        bass_isa.InstCustomDveAnt(
            name=b.get_next_instruction_name(),
            op_name=op.name,
            rd1_en=True,
            subdim=0,
            imm2=0.0,
            shape=shape,
            row=get_dve_sub_opcode(op.name),
            perf_max=perf_max,
            isa_opcode=isa_opcode,
            ins=[v.lower_ap(in0, for_isa=True),
                 v.lower_ap(in1, for_isa=True), imm(), imm()],
            outs=[v.lower_ap(out, for_isa=True)],
        )
    )


def _build_v3(R=8, final_gp_frac=0.65, n_vert_gp=1, in_bufs=2, mrg_bufs=1,
              x16_bufs=2, dve_tail=True, sliding2x=True, use_gp=False):
    """fp16 + custom sliding ops + ACT casts + GPSIMD final stage.

    Software-pipelined emission (engines execute their streams in order):
    per iteration k we emit  A(k+1) = load+cast of the NEXT chunk,
    D(k-1) = store of the PREVIOUS chunk, then B(k) = DVE work and
    C(k) = final stage.  This keeps the ACT in-cast ahead of the
    GPSIMD-dependent out-cast in the ACT/sync instruction streams.

    n_final_gp: how many of the 4 final med3 tensor_tensor ops run on
    GPSIMD (rest on DVE).  n_vert_gp: how many vertical ops on GPSIMD.
    dve_tail: run the last chunk's final stage on the DVE (shorter tail).
    """
    import concourse.bacc as bacc
    import concourse.mybir as mybir
    from concourse.tile import TileContext

    ops = _register_custom_ops()
    MIN = mybir.AluOpType.min
    MAX = mybir.AluOpType.max
    f32 = mybir.dt.float32
    f16 = mybir.dt.float16

    assert HH % R == 0 and R % 2 == 0
    K = HH // R
    Rh = R // 2
    NS = 2 + R * WP          # sliding stream length (2 prefix + rows)
    LSZ = R * WP + 4         # sliding tile: prefix 2 + rows + tail 2

    nc = bacc.Bacc("TRN2", name="median_pool2d_v3")
    x = nc.dram_tensor("x", [C, H, W], f32, kind="ExternalInput")
    out = nc.dram_tensor("out", [C, H, W], f32, kind="ExternalOutput")
    xg = x.ap()
    og = out.ap()

    with TileContext(nc) as tc:
        with (
            tc.tile_pool(name="in32", bufs=in_bufs) as in_pool,
            tc.tile_pool(name="x16p", bufs=x16_bufs) as x16_pool,
            tc.tile_pool(name="vert", bufs=1) as v_pool,
            tc.tile_pool(name="mrg", bufs=mrg_bufs) as m_pool,
            tc.tile_pool(name="fin", bufs=1) as f_pool,
            tc.tile_pool(name="ot16", bufs=2) as ot_pool,
            tc.tile_pool(name="out32", bufs=1) as o32_pool,
        ):
            # persistent sliding tiles: prefix/tail zeros written once
            def stile(name):
                t = v_pool.tile([128, LSZ], f16, name=name, tag=name)
                rows = t[:, 2:2 + R * WP].rearrange("p (r w) -> p r w", w=WP)
                return t, rows

            Lo_t, Lo3 = stile("Lo")
            Me_t, Me3 = stile("Me")
            Hi_t, Hi3 = stile("Hi")
            for t in (Lo_t, Me_t, Hi_t):
                nc.gpsimd.memset(t[:, 0:2], 0.0)
                nc.gpsimd.memset(t[:, 2 + R * WP:LSZ], 0.0)

            def vtile(name, rows):
                t = v_pool.tile([128, rows * WP], f16, name=name, tag=name)
                return t.rearrange("p (r w) -> p r w", w=WP)

            Pm = vtile("Pm", Rh)
            PM = vtile("PM", Rh)
            tEv = vtile("tE", Rh)
            tOv = vtile("tO", Rh)

            def stage_a(k):
                """DMA fp32 rows + ACT cast to padded fp16 rows."""
                r0 = k * R
                s32 = in_pool.tile([128, (R + 2) * W], f32, name="s32",
                                   tag="s32")
                s3 = s32.rearrange("p (r w) -> p r w", w=W)
                if k == 0:
                    nc.vector.memset(s3[0:64, 0:1], 0.0)
                    nc.sync.dma_start(out=s3[0:64, 1:R + 2],
                                      in_=xg[:, 0:R + 1, :])
                else:
                    nc.sync.dma_start(out=s3[0:64],
                                      in_=xg[:, r0 - 1:r0 + R + 1, :])
                if k == K - 1:
                    nc.vector.memset(s3[64:128, R + 1:R + 2], 0.0)
                    nc.sync.dma_start(out=s3[64:128, 0:R + 1],
                                      in_=xg[:, HH + r0 - 1:H, :])
                else:
                    nc.sync.dma_start(out=s3[64:128],
                                      in_=xg[:, HH + r0 - 1:HH + r0 + R + 1, :])
                x16 = x16_pool.tile([128, (R + 2) * WP], f16, name="x16",
                                    tag="x16")
                X = x16.rearrange("p (r w) -> p r w", w=WP)
                nc.gpsimd.memset(X[:, :, W:WP], 0.0)
                nc.scalar.copy(out=X[:, :, 0:W], in_=s3[:])
                return X

            def stage_bc(k, X):
                """DVE vertical + sliding, then final stage."""
                # ---- vertical sort3 (fp16 TT, 2x mode, pair-shared) ----
                vops = [
                    (Pm, X[:, 1:R + 1:2], X[:, 2:R + 2:2], MIN),
                    (PM, X[:, 1:R + 1:2], X[:, 2:R + 2:2], MAX),
                    (Lo3[:, 0:R:2], X[:, 0:R:2], Pm, MIN),
                    (Hi3[:, 0:R:2], X[:, 0:R:2], PM, MAX),
                    (tEv, X[:, 0:R:2], PM, MIN),
                    (Me3[:, 0:R:2], Pm, tEv, MAX),
                    (Lo3[:, 1:R:2], X[:, 3:R + 2:2], Pm, MIN),
                    (Hi3[:, 1:R:2], X[:, 3:R + 2:2], PM, MAX),
                    (tOv, X[:, 3:R + 2:2], PM, MIN),
                    (Me3[:, 1:R:2], Pm, tOv, MAX),
                ]
                for i, (o, a, b, alu) in enumerate(vops):
                    eng = (nc.gpsimd if i >= len(vops) - n_vert_gp
                           else nc.vector)
                    eng.tensor_tensor(out=o, in0=a, in1=b, op=alu)

                # ---- horizontal 3-tap merge: custom sliding DVE ops ----
                mA_t = m_pool.tile([128, LSZ], f16, name="mA", tag="mA")
                mB_t = m_pool.tile([128, LSZ], f16, name="mB", tag="mB")
                mC_t = m_pool.tile([128, LSZ], f16, name="mC", tag="mC")
                if sliding2x:
                    # window-start form at 2x: out[j] = f(T[j..j+2]);
                    # all APs even-offset fp16 step-1 -> RTL picks 2x_1p.
                    N2 = R * WP
                    for opname, src, dst in (
                        ("HMAX3W_ANT", Lo_t, mA_t),
                        ("HMED3W_ANT", Me_t, mB_t),
                        ("HMIN3W_ANT", Hi_t, mC_t),
                    ):
                        _emit_custom_dve(
                            nc.vector, ops[opname], out=dst[:, 0:N2],
                            in0=src[:, 0:N2], in1=src[:, 2:N2 + 2],
                            perf_max=1)
                    off = 0   # outputs shifted one left; final reads from 0
                else:
                    nc.vector._custom_dve(ops["HMAX3_ANT"],
                                          out=mA_t[:, 0:NS],
                                          in0=Lo_t[:, 0:NS],
                                          in1=Lo_t[:, 1:NS + 1])
                    nc.vector._custom_dve(ops["HMIN3_ANT"],
                                          out=mC_t[:, 0:NS],
                                          in0=Hi_t[:, 0:NS],
                                          in1=Hi_t[:, 1:NS + 1])
                    nc.vector._custom_dve(ops["HMED3_ANT"],
                                          out=mB_t[:, 0:NS],
                                          in0=Me_t[:, 0:NS],
                                          in1=Me_t[:, 1:NS + 1])
                    off = 2

                # ---- final med3(mA, mB, mC) ----
                mT = f_pool.tile([128, R * WP], f16, name="mT", tag="mT")
                mU = f_pool.tile([128, R * WP], f16, name="mU", tag="mU")
                mV = m_pool.tile([128, LSZ], f16, name="mV", tag="mA")
                ot = ot_pool.tile([128, R * WP + 2], f16, name="ot",
                                  tag="ot")
                Af = mA_t[:, off:off + R * WP]
                Bf = mB_t[:, off:off + R * WP]
                Cf = mC_t[:, off:off + R * WP]
                # split the 4-op chain by element range: each engine runs an
                # independent chain over its own slice (no cross-engine deps)
                g = 0.0 if (dve_tail and k == K - 1) else final_gp_frac
                S = 2 * int(R * WP * (1.0 - g) / 2)
                for eng, s0, s1 in ((nc.vector, 0, S),
                                    (nc.gpsimd, S, R * WP)):
                    if s1 <= s0:
                        continue
                    sl = slice(s0, s1)
                    osl = slice(off + s0, off + s1)
                    eng.tensor_tensor(out=mT[:, sl], in0=mA_t[:, osl],
                                      in1=mB_t[:, osl], op=MIN)
                    eng.tensor_tensor(out=mU[:, sl], in0=mA_t[:, osl],
                                      in1=mB_t[:, osl], op=MAX)
                    eng.tensor_tensor(out=mV[:, sl], in0=mU[:, sl],
                                      in1=mC_t[:, osl], op=MIN)
                    eng.tensor_tensor(out=ot[:, sl], in0=mT[:, sl],
                                      in1=mV[:, sl], op=MAX)
                return ot

            def stage_bc_dve(k, X):
                """All-DVE variant: GPSIMD tensor_tensor min/max is
                rejected by the walrus codegen, so everything runs on the
                DVE; dead Lo/Me/Hi buffers are re-used for the final
                temporaries (same engine, in-order, so aliasing is free)."""
                for o, a, b, alu in (
                    (Pm, X[:, 1:R + 1:2], X[:, 2:R + 2:2], MIN),
                    (PM, X[:, 1:R + 1:2], X[:, 2:R + 2:2], MAX),
                    (Lo3[:, 0:R:2], X[:, 0:R:2], Pm, MIN),
                    (Hi3[:, 0:R:2], X[:, 0:R:2], PM, MAX),
                    (tEv, X[:, 0:R:2], PM, MIN),
                    (Me3[:, 0:R:2], Pm, tEv, MAX),
                    (Lo3[:, 1:R:2], X[:, 3:R + 2:2], Pm, MIN),
                    (Hi3[:, 1:R:2], X[:, 3:R + 2:2], PM, MAX),
                    (tOv, X[:, 3:R + 2:2], PM, MIN),
                    (Me3[:, 1:R:2], Pm, tOv, MAX),
                ):
                    nc.vector.tensor_tensor(out=o, in0=a, in1=b, op=alu)

                mA_t = m_pool.tile([128, LSZ], f16, name="mA", tag="mA")
                mB_t = m_pool.tile([128, LSZ], f16, name="mB", tag="mB")
                mC_t = m_pool.tile([128, LSZ], f16, name="mC", tag="mC")
                N2 = R * WP
                for opname, src, dst in (
                    ("HMAX3W_ANT", Lo_t, mA_t),
                    ("HMED3W_ANT", Me_t, mB_t),
                    ("HMIN3W_ANT", Hi_t, mC_t),
                ):
                    _emit_custom_dve(
                        nc.vector, ops[opname], out=dst[:, 0:N2],
                        in0=src[:, 0:N2], in1=src[:, 2:N2 + 2], perf_max=1)

                # final med3 (all ops even-offset fp16 step-1 -> 2x mode)
                mT = f_pool.tile([128, N2], f16, name="mT", tag="mT")
                mU = f_pool.tile([128, N2], f16, name="mU", tag="mU")
                mV = f_pool.tile([128, N2], f16, name="mV", tag="mV")
                ot = ot_pool.tile([128, R * WP + 2], f16, name="ot",
                                  tag="ot")
                sl = slice(0, N2)
                nc.vector.tensor_tensor(out=mT[:], in0=mA_t[:, sl],
                                        in1=mB_t[:, sl], op=MIN)
                nc.vector.tensor_tensor(out=mU[:], in0=mA_t[:, sl],
                                        in1=mB_t[:, sl], op=MAX)
                nc.vector.tensor_tensor(out=mV[:], in0=mU[:],
                                        in1=mC_t[:, sl], op=MIN)
                nc.vector.tensor_tensor(out=ot[:, sl], in0=mT[:],
                                        in1=mV[:], op=MAX)
                return ot

            ot_off = 1 if sliding2x else 0

            def stage_d(k, ot):
                """ACT cast fp16 -> fp32 + DMA out."""
                r0 = k * R
                o32 = o32_pool.tile([128, R * W], f32, name="o32", tag="o32")
                o3 = o32.rearrange("p (r w) -> p r w", w=W)
                ot3 = ot[:, ot_off:ot_off + R * WP].rearrange(
                    "p (r w) -> p r w", w=WP)
                nc.scalar.copy(out=o3[:], in_=ot3[:, :, 0:W])
                # separate DGE queue from the input loads (sync) so stores
                # never head-of-line-block the next loads
                nc.scalar.dma_start(out=og[:, r0:r0 + R, :], in_=o3[0:64])
                nc.scalar.dma_start(out=og[:, HH + r0:HH + r0 + R, :],
                                    in_=o3[64:128])

            Xs = {0: stage_a(0)}
            ots = {}
            ddel = 1 if use_gp else 0   # store-delay (GPSIMD-final only)
            for k in range(K):
                if k + 1 < K:
                    Xs[k + 1] = stage_a(k + 1)
                if k - ddel in ots:
                    stage_d(k - ddel, ots.pop(k - ddel))
                ots[k] = (stage_bc(k, Xs.pop(k)) if use_gp
                          else stage_bc_dve(k, Xs.pop(k)))
                if ddel == 0:
                    stage_d(k, ots.pop(k))
            for kk in sorted(ots):
                stage_d(kk, ots.pop(kk))

    nc.compile()
    return nc


def _build_v4(sizes=(4, 8, 12, 16, 16, 16, 16, 16, 16, 8), in_bufs=2,
              x16_bufs=2):
    """All-DVE variant with heterogeneous chunk sizes: small first/last
    chunks shorten the pipeline ramp/tail, large middle chunks amortize
    per-instruction overhead."""
    import concourse.bacc as bacc
    import concourse.mybir as mybir
    from concourse.tile import TileContext

    ops = _register_custom_ops()
    MIN = mybir.AluOpType.min
    MAX = mybir.AluOpType.max
    f32 = mybir.dt.float32
    f16 = mybir.dt.float16

    sizes = list(sizes)
    assert sum(sizes) == HH and all(s % 2 == 0 for s in sizes)
    K = len(sizes)
    r0s = [sum(sizes[:i]) for i in range(K)]
    RM = max(sizes)
    LSZ = RM * WP + 4

    nc = bacc.Bacc("TRN2", name="median_pool2d_v4")
    x = nc.dram_tensor("x", [C, H, W], f32, kind="ExternalInput")
    out = nc.dram_tensor("out", [C, H, W], f32, kind="ExternalOutput")
    xg = x.ap()
    og = out.ap()

    with TileContext(nc) as tc:
        with (
            tc.tile_pool(name="in32", bufs=in_bufs) as in_pool,
            tc.tile_pool(name="x16p", bufs=x16_bufs) as x16_pool,
            tc.tile_pool(name="vert", bufs=1) as v_pool,
            tc.tile_pool(name="mrg", bufs=1) as m_pool,
            tc.tile_pool(name="fin", bufs=1) as f_pool,
            tc.tile_pool(name="ot16", bufs=2) as ot_pool,
            tc.tile_pool(name="out32", bufs=1) as o32_pool,
        ):
            def stile(name):
                return v_pool.tile([128, LSZ], f16, name=name, tag=name)

            Lo_t = stile("Lo")
            Me_t = stile("Me")
            Hi_t = stile("Hi")
            for t in (Lo_t, Me_t, Hi_t):
                nc.gpsimd.memset(t[:, 0:2], 0.0)
                # zero every distinct chunk-size tail once; stale row data
                # at a smaller-chunk tail only feeds pad-column junk
                for rc in sorted(set(sizes)):
                    nc.gpsimd.memset(t[:, 2 + rc * WP:2 + rc * WP + 2], 0.0)

            def vtile(name):
                return v_pool.tile([128, (RM // 2) * WP], f16, name=name,
                                   tag=name)

            Pm_t, PM_t, tE_t, tO_t = (vtile(n) for n in
                                      ("Pm", "PM", "tE", "tO"))

            def stage_a(k):
                rc = sizes[k]
                r0 = r0s[k]
                s32 = in_pool.tile([128, (RM + 2) * W], f32, name="s32",
                                   tag="s32")
                s3 = s32[:, 0:(rc + 2) * W].rearrange("p (r w) -> p r w",
                                                      w=W)
                if k == 0:
                    nc.gpsimd.memset(s3[0:64, 0:1], 0.0)
                    nc.sync.dma_start(out=s3[0:64, 1:rc + 2],
                                      in_=xg[:, 0:rc + 1, :])
                else:
                    nc.sync.dma_start(out=s3[0:64],
                                      in_=xg[:, r0 - 1:r0 + rc + 1, :])
                if k == K - 1:
                    nc.gpsimd.memset(s3[64:128, rc + 1:rc + 2], 0.0)
                    nc.sync.dma_start(out=s3[64:128, 0:rc + 1],
                                      in_=xg[:, HH + r0 - 1:H, :])
                else:
                    nc.sync.dma_start(
                        out=s3[64:128],
                        in_=xg[:, HH + r0 - 1:HH + r0 + rc + 1, :])
                x16 = x16_pool.tile([128, (RM + 2) * WP], f16, name="x16",
                                    tag="x16")
                X = x16[:, 0:(rc + 2) * WP].rearrange("p (r w) -> p r w",
                                                      w=WP)
                nc.gpsimd.memset(X[:, :, W:WP], 0.0)
                nc.scalar.copy(out=X[:, :, 0:W], in_=s3[:])
                return X

            def stage_b(k, X):
                rc = sizes[k]
                N2 = rc * WP
                Lo3, Me3, Hi3 = (
                    t[:, 2:2 + N2].rearrange("p (r w) -> p r w", w=WP)
                    for t in (Lo_t, Me_t, Hi_t))
                Pm, PM, tEv, tOv = (
                    t[:, 0:(rc // 2) * WP].rearrange("p (r w) -> p r w",
                                                     w=WP)
                    for t in (Pm_t, PM_t, tE_t, tO_t))
                for o, a, b, alu in (
                    (Pm, X[:, 1:rc + 1:2], X[:, 2:rc + 2:2], MIN),
                    (PM, X[:, 1:rc + 1:2], X[:, 2:rc + 2:2], MAX),
                    (Lo3[:, 0:rc:2], X[:, 0:rc:2], Pm, MIN),
                    (Hi3[:, 0:rc:2], X[:, 0:rc:2], PM, MAX),
                    (tEv, X[:, 0:rc:2], PM, MIN),
                    (Me3[:, 0:rc:2], Pm, tEv, MAX),
                    (Lo3[:, 1:rc:2], X[:, 3:rc + 2:2], Pm, MIN),
                    (Hi3[:, 1:rc:2], X[:, 3:rc + 2:2], PM, MAX),
                    (tOv, X[:, 3:rc + 2:2], PM, MIN),
                    (Me3[:, 1:rc:2], Pm, tOv, MAX),
                ):
                    nc.vector.tensor_tensor(out=o, in0=a, in1=b, op=alu)

                mA_t = m_pool.tile([128, LSZ], f16, name="mA", tag="mA")
                mB_t = m_pool.tile([128, LSZ], f16, name="mB", tag="mB")
                mC_t = m_pool.tile([128, LSZ], f16, name="mC", tag="mC")
                for opname, src, dst in (
                    ("HMAX3W_ANT", Lo_t, mA_t),
                    ("HMED3W_ANT", Me_t, mB_t),
                    ("HMIN3W_ANT", Hi_t, mC_t),
                ):
                    _emit_custom_dve(
                        nc.vector, ops[opname], out=dst[:, 0:N2],
                        in0=src[:, 0:N2], in1=src[:, 2:N2 + 2], perf_max=1)

                mT = f_pool.tile([128, RM * WP], f16, name="mT", tag="mT")
                mU = f_pool.tile([128, RM * WP], f16, name="mU", tag="mU")
                # mB is dead once mT/mU are done; reuse its buffer (all
                # ops on the DVE, in-order, so aliasing is dependency-free)
                mV = m_pool.tile([128, LSZ], f16, name="mV", tag="mB")
                ot = ot_pool.tile([128, RM * WP + 2], f16, name="ot",
                                  tag="ot")
                sl = slice(0, N2)
                nc.vector.tensor_tensor(out=mT[:, sl], in0=mA_t[:, sl],
                                        in1=mB_t[:, sl], op=MIN)
                nc.vector.tensor_tensor(out=mU[:, sl], in0=mA_t[:, sl],
                                        in1=mB_t[:, sl], op=MAX)
                nc.vector.tensor_tensor(out=mV[:, sl], in0=mU[:, sl],
                                        in1=mC_t[:, sl], op=MIN)
                nc.vector.tensor_tensor(out=ot[:, sl], in0=mT[:, sl],
                                        in1=mV[:, sl], op=MAX)
                return ot

            def stage_d(k, ot):
                rc = sizes[k]
                r0 = r0s[k]
                o32 = o32_pool.tile([128, RM * W], f32, name="o32",
                                    tag="o32")
                o3 = o32[:, 0:rc * W].rearrange("p (r w) -> p r w", w=W)
                ot3 = ot[:, 1:1 + rc * WP].rearrange("p (r w) -> p r w",
                                                     w=WP)
                nc.scalar.copy(out=o3[:], in_=ot3[:, :, 0:W])
                nc.scalar.dma_start(out=og[:, r0:r0 + rc, :], in_=o3[0:64])
                nc.scalar.dma_start(out=og[:, HH + r0:HH + r0 + rc, :],
                                    in_=o3[64:128])

            Xs = {0: stage_a(0)}
            for k in range(K):
                if k + 1 < K:
                    Xs[k + 1] = stage_a(k + 1)
                ot = stage_b(k, Xs.pop(k))
                stage_d(k, ot)

    nc.compile()
    return nc


def _build_shared(R=16, gp_frac=0.0, dtype="float32", in_bufs=None,
                  out_bufs=None):
    """Fallback: 15-op/pixel fp32 TT network (previous working kernel)."""
    import concourse.bacc as bacc
    import concourse.mybir as mybir
    from concourse.tile import TileContext

    MIN = mybir.AluOpType.min
    MAX = mybir.AluOpType.max
    f32 = mybir.dt.float32
    cdt = getattr(mybir.dt, dtype)

    WPP = W + 2
    assert HH % R == 0 and R % 2 == 0
    K = HH // R
    Rh = R // 2

    nc = bacc.Bacc("TRN2", name="median_pool2d_s")
    x = nc.dram_tensor("x", [C, H, W], f32, kind="ExternalInput")
    out = nc.dram_tensor("out", [C, H, W], f32, kind="ExternalOutput")
    xg = x.ap()
    og = out.ap()

    def tt(out_ap, in0, in1, op):
        nc.vector.tensor_tensor(out=out_ap, in0=in0, in1=in1, op=op)

    if in_bufs is None:
        in_bufs = 3 if R <= 8 else 2
    if out_bufs is None:
        out_bufs = 3 if R <= 8 else 1
    with TileContext(nc) as tc:
        with (
            tc.tile_pool(name="io_in", bufs=in_bufs) as in_pool,
            tc.tile_pool(name="io_out", bufs=out_bufs) as out_pool,
            tc.tile_pool(name="work", bufs=1) as w_pool,
        ):
            def wtile(name, rows, width, tag=None):
                t = w_pool.tile([128, rows * width], cdt, name=name,
                                tag=tag or name)
                return t.rearrange("p (r w) -> p r w", w=width)

            for k in range(K):
                r0 = k * R
                it = in_pool.tile([128, (R + 2) * WPP], cdt, name="it",
                                  tag="it")
                it3 = it.rearrange("p (r w) -> p r w", w=WPP)
                nc.vector.memset(it3[:, :, 0:WPP:WPP - 1], 0.0)
                if k == 0:
                    nc.vector.memset(it3[0:64, 0:1, 1:W + 1], 0.0)
                    nc.sync.dma_start(out=it3[0:64, 1:R + 2, 1:W + 1],
                                      in_=xg[:, 0:R + 1, :])
                else:
                    nc.sync.dma_start(out=it3[0:64, :, 1:W + 1],
                                      in_=xg[:, r0 - 1:r0 + R + 1, :])
                if k == K - 1:
                    nc.vector.memset(it3[64:128, R + 1:R + 2, 1:W + 1], 0.0)
                    nc.sync.dma_start(out=it3[64:128, 0:R + 1, 1:W + 1],
                                      in_=xg[:, HH + r0 - 1:H, :])
                else:
                    nc.sync.dma_start(out=it3[64:128, :, 1:W + 1],
                                      in_=xg[:, HH + r0 - 1:HH + r0 + R + 1, :])

                Pm = wtile("Pm", Rh, WPP)
                PM = wtile("PM", Rh, WPP)
                tt(Pm, it3[:, 1:R + 1:2, :], it3[:, 2:R + 2:2, :], MIN)
                tt(PM, it3[:, 1:R + 1:2, :], it3[:, 2:R + 2:2, :], MAX)

                Lo3 = wtile("Lo", R, WPP)
                Me3 = wtile("Me", R, WPP)
                Hi3 = wtile("Hi", R, WPP)
                tE = wtile("tE", Rh, WPP)
                tO = wtile("tO", Rh, WPP)
                a_e = it3[:, 0:R:2, :]
                a_o = it3[:, 3:R + 2:2, :]
                tt(Lo3[:, 0:R:2], a_e, Pm, MIN)
                tt(Hi3[:, 0:R:2], a_e, PM, MAX)
                tt(tE, a_e, PM, MIN)
                tt(Me3[:, 0:R:2], Pm, tE, MAX)
                tt(Lo3[:, 1:R:2], a_o, Pm, MIN)
                tt(Hi3[:, 1:R:2], a_o, PM, MAX)
                tt(tO, a_o, PM, MIN)
                tt(Me3[:, 1:R:2], Pm, tO, MAX)

                NP = W // 2 + 1
                PA = wtile("PA", R, NP, tag="Pm")
                PC = wtile("PC", R, NP, tag="PM")
                Um = wtile("Um", R, NP, tag="tE")
                Vm = wtile("Vm", R, NP, tag="tO")
                tBe = wtile("tBe", R, W // 2, tag="Pm")
                tBo = wtile("tBo", R, W // 2, tag="PM")
                mA = wtile("mA", R, W)
                mB = wtile("mB", R, W)
                mC = wtile("mC", R, W)

                ev = slice(0, WPP, 2)
                od = slice(1, WPP, 2)
                tt(PA, Lo3[:, :, ev], Lo3[:, :, od], MAX)
                tt(mA[:, :, 0:W:2], PA[:, :, 0:NP - 1], Lo3[:, :, 2:WPP:2],
                   MAX)
                tt(mA[:, :, 1:W:2], PA[:, :, 1:NP], Lo3[:, :, 1:WPP - 2:2],
                   MAX)

                tt(PC, Hi3[:, :, ev], Hi3[:, :, od], MIN)
                tt(mC[:, :, 0:W:2], PC[:, :, 0:NP - 1], Hi3[:, :, 2:WPP:2],
                   MIN)
                tt(mC[:, :, 1:W:2], PC[:, :, 1:NP], Hi3[:, :, 1:WPP - 2:2],
                   MIN)

                tt(Um, Me3[:, :, ev], Me3[:, :, od], MIN)
                tt(Vm, Me3[:, :, ev], Me3[:, :, od], MAX)
                tt(tBe, Me3[:, :, 2:WPP:2], Vm[:, :, 0:NP - 1], MIN)
                tt(mB[:, :, 0:W:2], Um[:, :, 0:NP - 1], tBe, MAX)
                tt(tBo, Me3[:, :, 1:WPP - 2:2], Vm[:, :, 1:NP], MIN)
                tt(mB[:, :, 1:W:2], Um[:, :, 1:NP], tBo, MAX)

                mT = wtile("mT", R, W, tag="Lo")
                mU = wtile("mU", R, W, tag="Me")
                mV = wtile("mV", R, W, tag="Hi")
                ot = out_pool.tile([128, R * W], cdt, name="ot", tag="ot")
                ot3 = ot.rearrange("p (r w) -> p r w", w=W)
                tt(mT, mA, mB, MIN)
                tt(mU, mA, mB, MAX)
                tt(mV, mU, mC, MIN)
                tt(ot3, mT, mV, MAX)

                nc.sync.dma_start(out=og[:, r0:r0 + R, :], in_=ot3[0:64])
                nc.sync.dma_start(out=og[:, HH + r0:HH + r0 + R, :],
                                  in_=ot3[64:128])

    nc.compile()
    return nc


def _get_nc(variant="v3", **kw):
    key = (variant, tuple(sorted(kw.items())))
    if key not in _CACHE:
        if variant == "v4":
            _CACHE[key] = _build_v4(**kw)
        elif variant == "v3":
            _CACHE[key] = _build_v3(**kw)
        else:
            _CACHE[key] = _build_shared(**kw)
    return _CACHE[key]


_LAST_NC = None


def kernel(x: np.ndarray) -> np.ndarray:
    """MedianPool2d(3x3, s=1, p=1) on 8 NeuronCores."""
    global _LAST_NC
    from concourse.bass_utils import run_bass_kernel_spmd

    assert x.shape == (B, C, H, W), x.shape
    x = np.ascontiguousarray(x, dtype=np.float32)
    try:
        nc = _get_nc("v4")
    except Exception:
        try:
            nc = _get_nc("v3")
        except Exception:
            nc = _get_nc("shared", R=16)
    _LAST_NC = nc
    in_maps = [{"x": x[i]} for i in range(NCORES)]
    res = run_bass_kernel_spmd(nc, in_maps, core_ids=list(range(NCORES)))
    return np.stack([r["out"] for r in res.results], axis=0)


# revision 36
# speedup vs baseline: 2.3650x; 1.0100x over previous
"""MedianPool2d (3x3, stride 1, zero-pad 1) Trainium2 Bass kernel.

Full input x: (8, 64, 256, 256) fp32.  Sharding: pure data parallel over
batch -> core i processes x[i] (64, 256, 256).

Design (v4: fp16 + custom packed-2x sliding DVE ops):
  - Per-core layout: 128 SBUF partitions = (h, c), h in {0,1} = top/bottom
    128-row half, c = channel.  Heterogeneous chunks of output rows per
    partition (small first/last chunks shorten pipeline ramp/tail).
  - DMA loads fp32 rows into a staging tile; the Activation engine casts
    fp32 -> fp16 into a padded row layout (WP=258: 256 cols + 2 zero pad
    cols).  fp16 is exact for the median network itself (pure min/max
    selection); only the input cast rounds (~2.1e-4 l2 rel, gate 2e-2).
  - Vertical sort3 (Lo/Me/Hi per output row) as fp16 tensor_tensor
    min/max on the DVE with row-pair sharing: all ops full-row step-1
    4B-aligned -> genuine 2x DVE mode (0.52 ns/elem).
  - Horizontal 3-tap merge via three hand-authored custom DVE micro-op
    programs (HMAX3W/HMED3W/HMIN3W, registered into the per-NEFF DVE
    table at runtime): ONE streaming pass each computes
    out[j] = f(T[j], T[j+1], T[j+2]) over the flat row stream.  The APs
    (src0 = T[0:N], src1 = T[2:N+2], both fp16 step-1 4B-aligned) make
    the RTL select 2x_1p packed mode, where the crossbar exposes four
    consecutive taps per cycle (SRC_0, SRC_0_HI, SRC_1, SRC_1_HI) - both
    parities' windows are pure spatial, even result -> WR0_LO, odd ->
    WR0_HI, 2 elem/cycle.  Each row carries 2 trailing zero pad cols so
    the previous row's tail doubles as the next row's left pad; a
    2-element zero prefix seeds row 0.  HW-validated bit-exact.
  - Final med3(mA, mB, mC) as 4 more fp16 2x TTs on the DVE.
    (GPSIMD tensor_tensor min/max and DMA-CCE min/max accumulate are both
    rejected by the walrus codegen, so GPSIMD only does small memsets.)
  - Output cast fp16 -> fp32 on the Activation engine, then DMA out on a
    separate DGE queue from the loads.

Effective cost ~12 TT-op-equivalents/pixel at 2x (vs 15 at 1x fp32 for
the baseline) -> 233.5us/core vs 546.7us baseline (2.34x).
"""

import numpy as np

B, C, H, W = 8, 64, 256, 256
NCORES = 8
HH = H // 2          # rows per half-strip
WP = W + 2           # padded row width (2 trailing zero cols)

_CACHE = {}
_OPS = None


def _register_custom_ops():
    """Register the three sliding-window custom DVE ops (idempotent).

    Each op streams src0 = T[j], src1 = T[j+1] and computes a 3-tap
    window f(T[j-1], T[j], T[j+1]) per output element:
      - T[j], T[j+1] arrive spatially on the two source streams;
      - T[j-1] = src1 two elements back, read via NEXT_ALU_OUT_A from a
        pipeline stage that latches raw src1 into its a-flop each cycle.
    Outputs j=0,1 use pre-instruction flop state (garbage); callers must
    treat the first two output elements as scrap.
    """
    global _OPS
    if _OPS is not None:
        return _OPS

    import concourse.dve_ops as dve_ops
    from concourse.dve_spec import Spec, Src0, Src1, minn
    from concourse.dve_uop import (
        ENABLE,
        AluInp,
        AluOp,
        DelayInp,
        DveOpSpec,
        InpSel,
        OutPath,
        OutSel,
        Trigger,
        UopConfig,
    )

    def _shift2(in1):
        z = np.zeros_like(in1[..., :2])
        return np.concatenate([z, in1[..., :-2]], axis=-1)

    def _hmax3_ref(in0, in1, c0, c1, c2):
        return np.maximum(np.maximum(in0, in1), _shift2(in1))

    def _hmin3_ref(in0, in1, c0, c1, c2):
        return np.minimum(np.minimum(in0, in1), _shift2(in1))

    def _hmed3_ref(in0, in1, c0, c1, c2):
        c = _shift2(in1)
        return np.maximum(np.minimum(in0, in1),
                          np.minimum(np.maximum(in0, in1), c))

    def _mm3_uops(op3):
        """out[j] = op3(src0[j], src1[j], src1[j-2]) single-uop program."""
        u = UopConfig()
        u.enable_input(InpSel.SRC_0, 1)     # -> PREV_DELAY_0 at blk0
        u.enable_input(InpSel.SRC_1, 2)     # -> PREV_DELAY_1 at blk0
        u.require_inp0 = ENABLE
        u.require_inp1 = ENABLE
        u.trigger = (Trigger.SRC_TENSOR_DONE, Trigger.NONE, Trigger.NONE)
        dp = u.datapath_config
        # blk0: m = op3(src0, src1)
        dp[0].enable_alu(op3, AluInp.PREV_DELAY_0, AluInp.PREV_DELAY_1)
        dp[0].pass_through_delay(1)
        # blk1: out = op3(m, src1[j-2])  (blk2's a-flop, prev cycle)
        dp[1].enable_alu(op3, AluInp.PREV_ALU_OUT, AluInp.NEXT_ALU_OUT_A)
        dp[1].pass_through_delay(1)
        # blk2: latch raw src1 into the a-flop; stash out in delay chain 0
        dp[2].enable_alu(AluOp.BYPASS, AluInp.PREV_DELAY_1)
        dp[2].alu_out_a_enable = ENABLE
        dp[2].enable_delay_from_src(DelayInp.PREV_ALU_OUT, 0)
        for kblk in range(3, 8):
            dp[kblk].pass_through_alu().pass_through_delay(0)
        u.enable_output(OutSel.DELAY_0, OutPath.WR0_LO)
        return [u]

    def _med3_uops():
        """out[j] = med3(src0[j], src1[j], src1[j-2]) single-uop program."""
        u = UopConfig()
        u.enable_input(InpSel.SRC_0, 1)     # chain 0
        u.enable_input(InpSel.SRC_1, 2)     # chain 1
        u.require_inp0 = ENABLE
        u.require_inp1 = ENABLE
        u.trigger = (Trigger.SRC_TENSOR_DONE, Trigger.NONE, Trigger.NONE)
        dp = u.datapath_config
        # blk0: p = min(a, b)
        dp[0].enable_alu(AluOp.MIN, AluInp.PREV_DELAY_0, AluInp.PREV_DELAY_1)
        dp[0].pass_through_delay(0, 1)
        # blk1: q = max(a, b); stash p in chain 2
        dp[1].enable_alu(AluOp.MAX, AluInp.PREV_DELAY_0, AluInp.PREV_DELAY_1)
        dp[1].enable_delay_from_src(DelayInp.PREV_ALU_OUT, 2)
        dp[1].pass_through_delay(1)
        # blk2: t = min(q, c) with c = src1[j-2] via blk3's a-flop
        dp[2].enable_alu(AluOp.MIN, AluInp.PREV_ALU_OUT, AluInp.NEXT_ALU_OUT_A)
        dp[2].pass_through_delay(1, 2)
        # blk3: latch raw src1 into a-flop; stash t in chain 3
        dp[3].enable_alu(AluOp.BYPASS, AluInp.PREV_DELAY_1)
        dp[3].alu_out_a_enable = ENABLE
        dp[3].enable_delay_from_src(DelayInp.PREV_ALU_OUT, 3)
        dp[3].pass_through_delay(2)
        # blk4: out = max(p, t)
        dp[4].enable_alu(AluOp.MAX, AluInp.PREV_DELAY_2, AluInp.PREV_DELAY_3)
        for kblk in range(5, 8):
            dp[kblk].pass_through_alu()
        u.enable_output(OutSel.ALU_OUT, OutPath.WR0_LO)
        return [u]

    # ---- 2x (packed fp16) window-start variants -------------------------
    # Semantics: out[j] = f(T[j], T[j+1], T[j+2]) with src0 = T[0:N],
    # src1 = T[2:N+2], all APs fp16 step-1 4B-aligned so the RTL always
    # selects 2x_1p.  Per pair-cycle the crossbar exposes 4 consecutive
    # taps (SRC_0, SRC_0_HI, SRC_1, SRC_1_HI); both parities' windows are
    # pure spatial.  Even result -> WR0_LO, odd -> WR0_HI.

    def _shiftcat(in0, in1):
        # full stream T (len N+2) from the two views
        return np.concatenate([in0, in1[..., -2:]], axis=-1)

    def _hmax3w_ref(in0, in1, c0, c1, c2):
        T = _shiftcat(in0, in1)
        return np.maximum(np.maximum(T[..., :-2], T[..., 1:-1]), T[..., 2:])

    def _hmin3w_ref(in0, in1, c0, c1, c2):
        T = _shiftcat(in0, in1)
        return np.minimum(np.minimum(T[..., :-2], T[..., 1:-1]), T[..., 2:])

    def _hmed3w_ref(in0, in1, c0, c1, c2):
        T = _shiftcat(in0, in1)
        a, b, c = T[..., :-2], T[..., 1:-1], T[..., 2:]
        return np.maximum(np.minimum(a, b),
                          np.minimum(np.maximum(a, b), c))

    def _dummy_1x(op3):
        """Placeholder REGULAR-mode program (never selected: the APs
        always satisfy the 2x_1p conditions)."""
        u = UopConfig()
        u.enable_input(InpSel.SRC_0, 1)
        u.enable_input(InpSel.SRC_1, 2)
        u.require_inp0 = ENABLE
        u.require_inp1 = ENABLE
        u.trigger = (Trigger.SRC_TENSOR_DONE, Trigger.NONE, Trigger.NONE)
        dp = u.datapath_config
        dp[0].enable_alu(op3, AluInp.PREV_DELAY_0, AluInp.PREV_DELAY_1)
        for kblk in range(1, 8):
            dp[kblk].pass_through_alu()
        u.enable_output(OutSel.ALU_OUT, OutPath.WR0_LO)
        return [u]

    def _mm3_w2x_uops(op3):
        """2x program: E = op3(S0, S0H, S1) -> WR0_LO, O = op3(S0H, S1,
        S1H) -> WR0_HI."""
        u = UopConfig()
        u.enable_input(InpSel.SRC_0, 1)      # chain 0
        u.enable_input(InpSel.SRC_0_HI, 2)   # chain 1
        u.enable_input(InpSel.SRC_1, 3)      # chain 2
        u.enable_input(InpSel.SRC_1_HI, 4)   # chain 3
        u.require_inp0 = ENABLE
        u.require_inp1 = ENABLE
        u.trigger = (Trigger.SRC_TENSOR_DONE, Trigger.NONE, Trigger.NONE)
        dp = u.datapath_config
        dp[0].enable_alu(op3, AluInp.PREV_DELAY_0, AluInp.PREV_DELAY_1)
        dp[0].pass_through_delay(1, 2, 3)
        dp[1].enable_alu(op3, AluInp.PREV_ALU_OUT, AluInp.PREV_DELAY_2)
        dp[1].pass_through_delay(1, 2, 3)
        dp[2].enable_alu(op3, AluInp.PREV_DELAY_1, AluInp.PREV_DELAY_2)
        dp[2].enable_delay_from_src(DelayInp.PREV_ALU_OUT, 4)   # E
        dp[2].pass_through_delay(3)
        dp[3].enable_alu(op3, AluInp.PREV_ALU_OUT, AluInp.PREV_DELAY_3)
        dp[3].pass_through_delay(4)
        for kblk in range(4, 8):
            dp[kblk].pass_through_alu().pass_through_delay(4)
        u.enable_output(OutSel.DELAY_4, OutPath.WR0_LO)
        u.enable_output(OutSel.ALU_OUT, OutPath.WR0_HI)
        return [u]

    def _med3_w2x_uops():
        """2x program: E = med3(S0, S0H, S1) -> WR0_LO, O = med3(S0H, S1,
        S1H) -> WR0_HI.  Exactly 8 ALU blocks."""
        u = UopConfig()
        u.enable_input(InpSel.SRC_0, 1)      # chain 0
        u.enable_input(InpSel.SRC_0_HI, 2)   # chain 1
        u.enable_input(InpSel.SRC_1, 3)      # chain 2
        u.enable_input(InpSel.SRC_1_HI, 4)   # chain 3
        u.require_inp0 = ENABLE
        u.require_inp1 = ENABLE
        u.trigger = (Trigger.SRC_TENSOR_DONE, Trigger.NONE, Trigger.NONE)
        dp = u.datapath_config
        # E-half: med3(d0, d1, d2)
        dp[0].enable_alu(AluOp.MIN, AluInp.PREV_DELAY_0, AluInp.PREV_DELAY_1)
        dp[0].pass_through_delay(0, 1, 2, 3)
        dp[1].enable_alu(AluOp.MAX, AluInp.PREV_DELAY_0, AluInp.PREV_DELAY_1)
        dp[1].enable_delay_from_src(DelayInp.PREV_ALU_OUT, 4)   # pE
        dp[1].pass_through_delay(1, 2, 3)
        dp[2].enable_alu(AluOp.MIN, AluInp.PREV_ALU_OUT, AluInp.PREV_DELAY_2)
        dp[2].pass_through_delay(1, 2, 3, 4)
        dp[3].enable_alu(AluOp.MAX, AluInp.PREV_DELAY_4, AluInp.PREV_ALU_OUT)
        dp[3].pass_through_delay(1, 2, 3)
        # O-half: med3(d1, d2, d3)
        dp[4].enable_alu(AluOp.MIN, AluInp.PREV_DELAY_1, AluInp.PREV_DELAY_2)
        dp[4].enable_delay_from_src(DelayInp.PREV_ALU_OUT, 4)   # E
        dp[4].pass_through_delay(1, 2, 3)
        dp[5].enable_alu(AluOp.MAX, AluInp.PREV_DELAY_1, AluInp.PREV_DELAY_2)
        dp[5].enable_delay_from_src(DelayInp.PREV_ALU_OUT, 5)   # pO
        dp[5].pass_through_delay(3, 4)
        dp[6].enable_alu(AluOp.MIN, AluInp.PREV_ALU_OUT, AluInp.PREV_DELAY_3)
        dp[6].pass_through_delay(4, 5)
        dp[7].enable_alu(AluOp.MAX, AluInp.PREV_DELAY_5, AluInp.PREV_ALU_OUT)
        dp[7].pass_through_delay(4)
        u.enable_output(OutSel.DELAY_4, OutPath.WR0_LO)
        u.enable_output(OutSel.ALU_OUT, OutPath.WR0_HI)
        return [u]

    class _SlidingOp:
        """Duck-typed DveOp: hand-authored uops instead of lower(spec)."""

        def __init__(self, name, reference, uops_builder, uops_2x_builder=None):
            # body is a placeholder that reads Src0+Src1 (rd1_en) and no
            # C2/C3; only `reference` is ever evaluated (bass_interp).
            self.name = name
            self.spec = Spec(body=minn(Src0, Src1), reference=reference)
            self.subdim = False
            self.perf_en = {}
            self._builder = uops_builder
            self._builder_2x = uops_2x_builder
            self._cache = {}

        def compile(self, ver):
            if ver not in self._cache:
                s = DveOpSpec(
                    name=self.name,
                    opcode=dve_ops.get_dve_sub_opcode(self.name),
                    uops=self._builder(),
                    uops_2x=(self._builder_2x() if self._builder_2x
                             else None),
                    perf_max=1 if self._builder_2x else 0,
                    rd1_en=True,
                )
                s.validate(ver)
                self._cache[ver] = s
            return self._cache[ver]

    defs = [
        ("HMAX3_ANT", _hmax3_ref, lambda: _mm3_uops(AluOp.MAX), None),
        ("HMIN3_ANT", _hmin3_ref, lambda: _mm3_uops(AluOp.MIN), None),
        ("HMED3_ANT", _hmed3_ref, _med3_uops, None),
        ("HMAX3W_ANT", _hmax3w_ref, lambda: _dummy_1x(AluOp.MAX),
         lambda: _mm3_w2x_uops(AluOp.MAX)),
        ("HMIN3W_ANT", _hmin3w_ref, lambda: _dummy_1x(AluOp.MIN),
         lambda: _mm3_w2x_uops(AluOp.MIN)),
        ("HMED3W_ANT", _hmed3w_ref, lambda: _dummy_1x(AluOp.MIN),
         _med3_w2x_uops),
    ]
    ops = {}
    for name, ref, builder, builder2x in defs:
        if name not in dve_ops._SUB_OPCODE_FOR_NAME:
            row = max(dve_ops._SUB_OPCODE_FOR_NAME.values()) + 1
            assert row < 0x20, "custom DVE opcode rows exhausted"
            dve_ops._SUB_OPCODE_FOR_NAME[name] = row
        op = _SlidingOp(name, ref, builder, builder2x)
        # replace any previous registration (idempotent across reloads)
        dve_ops.OPS[:] = [o for o in dve_ops.OPS if o.name != name] + [op]
        dve_ops.CUSTOM_DVE_SPECS[name] = op.spec
        ops[name] = op
    _OPS = ops
    return ops


def _emit_custom_dve(v, op, out, in0, in1, perf_max=0):
    """Like bass.Vector._custom_dve but with perf_max set at construction
    (the Tile scheduler drops post-hoc attribute edits)."""
    import concourse.bass_isa as bass_isa
    import concourse.mybir as mybir
    from concourse.dve_ops import get_dve_sub_opcode

    b = v.bass
    if op.name not in b.m.ant_custom_dve_ops:
        b.m.ant_custom_dve_ops = sorted({*b.m.ant_custom_dve_ops, op.name})
    shape = bass_isa.CustomDveShape.TTSS
    isa_opcode = b.isa.Opcode[
        f"NEURON_ISA_TPB_OPCODE_CUSTOM_DVE_ANT_{shape.slot()}"
    ].value
    imm = lambda: mybir.ImmediateValue(dtype=mybir.dt.float32, value=0.0)
    return v.add_instruction(
        bass_isa.InstCustomDveAnt(
            name=b.get_next_instruction_name(),
            op_name=op.name,
            rd1_en=True,
            subdim=0,
            imm2=0.0,
            shape=shape,
            row=get_dve_sub_opcode(op.name),
            perf_max=perf_max,
            isa_opcode=isa_opcode,
            ins=[v.lower_ap(in0, for_isa=True),
                 v.lower_ap(in1, for_isa=True), imm(), imm()],
            outs=[v.lower_ap(out, for_isa=True)],
        )
    )


def _build_v3(R=8, final_gp_frac=0.65, n_vert_gp=1, in_bufs=2, mrg_bufs=1,
              x16_bufs=2, dve_tail=True, sliding2x=True, use_gp=False):
    """fp16 + custom sliding ops + ACT casts + GPSIMD final stage.

    Software-pipelined emission (engines execute their streams in order):
    per iteration k we emit  A(k+1) = load+cast of the NEXT chunk,
    D(k-1) = store of the PREVIOUS chunk, then B(k) = DVE work and
    C(k) = final stage.  This keeps the ACT in-cast ahead of the
    GPSIMD-dependent out-cast in the ACT/sync instruction streams.

    n_final_gp: how many of the 4 final med3 tensor_tensor ops run on
    GPSIMD (rest on DVE).  n_vert_gp: how many vertical ops on GPSIMD.
    dve_tail: run the last chunk's final stage on the DVE (shorter tail).
    """
    import concourse.bacc as bacc
    import concourse.mybir as mybir
    from concourse.tile import TileContext

    ops = _register_custom_ops()
    MIN = mybir.AluOpType.min
    MAX = mybir.AluOpType.max
    f32 = mybir.dt.float32
    f16 = mybir.dt.float16

    assert HH % R == 0 and R % 2 == 0
    K = HH // R
    Rh = R // 2
    NS = 2 + R * WP          # sliding stream length (2 prefix + rows)
    LSZ = R * WP + 4         # sliding tile: prefix 2 + rows + tail 2

    nc = bacc.Bacc("TRN2", name="median_pool2d_v3")
    x = nc.dram_tensor("x", [C, H, W], f32, kind="ExternalInput")
    out = nc.dram_tensor("out", [C, H, W], f32, kind="ExternalOutput")
    xg = x.ap()
    og = out.ap()

    with TileContext(nc) as tc:
        with (
            tc.tile_pool(name="in32", bufs=in_bufs) as in_pool,
            tc.tile_pool(name="x16p", bufs=x16_bufs) as x16_pool,
            tc.tile_pool(name="vert", bufs=1) as v_pool,
            tc.tile_pool(name="mrg", bufs=mrg_bufs) as m_pool,
            tc.tile_pool(name="fin", bufs=1) as f_pool,
            tc.tile_pool(name="ot16", bufs=2) as ot_pool,
            tc.tile_pool(name="out32", bufs=1) as o32_pool,
        ):
            # persistent sliding tiles: prefix/tail zeros written once
            def stile(name):
                t = v_pool.tile([128, LSZ], f16, name=name, tag=name)
                rows = t[:, 2:2 + R * WP].rearrange("p (r w) -> p r w", w=WP)
                return t, rows

            Lo_t, Lo3 = stile("Lo")
            Me_t, Me3 = stile("Me")
            Hi_t, Hi3 = stile("Hi")
            for t in (Lo_t, Me_t, Hi_t):
                nc.gpsimd.memset(t[:, 0:2], 0.0)
                nc.gpsimd.memset(t[:, 2 + R * WP:LSZ], 0.0)

            def vtile(name, rows):
                t = v_pool.tile([128, rows * WP], f16, name=name, tag=name)
                return t.rearrange("p (r w) -> p r w", w=WP)

            Pm = vtile("Pm", Rh)
            PM = vtile("PM", Rh)
            tEv = vtile("tE", Rh)
            tOv = vtile("tO", Rh)

            def stage_a(k):
                """DMA fp32 rows + ACT cast to padded fp16 rows."""
                r0 = k * R
                s32 = in_pool.tile([128, (R + 2) * W], f32, name="s32",
                                   tag="s32")
                s3 = s32.rearrange("p (r w) -> p r w", w=W)
                if k == 0:
                    nc.vector.memset(s3[0:64, 0:1], 0.0)
                    nc.sync.dma_start(out=s3[0:64, 1:R + 2],
                                      in_=xg[:, 0:R + 1, :])
                else:
                    nc.sync.dma_start(out=s3[0:64],
                                      in_=xg[:, r0 - 1:r0 + R + 1, :])
                if k == K - 1:
                    nc.vector.memset(s3[64:128, R + 1:R + 2], 0.0)
                    nc.sync.dma_start(out=s3[64:128, 0:R + 1],
                                      in_=xg[:, HH + r0 - 1:H, :])
                else:
                    nc.sync.dma_start(out=s3[64:128],
                                      in_=xg[:, HH + r0 - 1:HH + r0 + R + 1, :])
                x16 = x16_pool.tile([128, (R + 2) * WP], f16, name="x16",
                                    tag="x16")
                X = x16.rearrange("p (r w) -> p r w", w=WP)
                nc.gpsimd.memset(X[:, :, W:WP], 0.0)
                nc.scalar.copy(out=X[:, :, 0:W], in_=s3[:])
                return X

            def stage_bc(k, X):
                """DVE vertical + sliding, then final stage."""
                # ---- vertical sort3 (fp16 TT, 2x mode, pair-shared) ----
                vops = [
                    (Pm, X[:, 1:R + 1:2], X[:, 2:R + 2:2], MIN),
                    (PM, X[:, 1:R + 1:2], X[:, 2:R + 2:2], MAX),
                    (Lo3[:, 0:R:2], X[:, 0:R:2], Pm, MIN),
                    (Hi3[:, 0:R:2], X[:, 0:R:2], PM, MAX),
                    (tEv, X[:, 0:R:2], PM, MIN),
                    (Me3[:, 0:R:2], Pm, tEv, MAX),
                    (Lo3[:, 1:R:2], X[:, 3:R + 2:2], Pm, MIN),
                    (Hi3[:, 1:R:2], X[:, 3:R + 2:2], PM, MAX),
                    (tOv, X[:, 3:R + 2:2], PM, MIN),
                    (Me3[:, 1:R:2], Pm, tOv, MAX),
                ]
                for i, (o, a, b, alu) in enumerate(vops):
                    eng = (nc.gpsimd if i >= len(vops) - n_vert_gp
                           else nc.vector)
                    eng.tensor_tensor(out=o, in0=a, in1=b, op=alu)

                # ---- horizontal 3-tap merge: custom sliding DVE ops ----
                mA_t = m_pool.tile([128, LSZ], f16, name="mA", tag="mA")
                mB_t = m_pool.tile([128, LSZ], f16, name="mB", tag="mB")
                mC_t = m_pool.tile([128, LSZ], f16, name="mC", tag="mC")
                if sliding2x:
                    # window-start form at 2x: out[j] = f(T[j..j+2]);
                    # all APs even-offset fp16 step-1 -> RTL picks 2x_1p.
                    N2 = R * WP
                    for opname, src, dst in (
                        ("HMAX3W_ANT", Lo_t, mA_t),
                        ("HMED3W_ANT", Me_t, mB_t),
                        ("HMIN3W_ANT", Hi_t, mC_t),
                    ):
                        _emit_custom_dve(
                            nc.vector, ops[opname], out=dst[:, 0:N2],
                            in0=src[:, 0:N2], in1=src[:, 2:N2 + 2],
                            perf_max=1)
                    off = 0   # outputs shifted one left; final reads from 0
                else:
                    nc.vector._custom_dve(ops["HMAX3_ANT"],
                                          out=mA_t[:, 0:NS],
                                          in0=Lo_t[:, 0:NS],
                                          in1=Lo_t[:, 1:NS + 1])
                    nc.vector._custom_dve(ops["HMIN3_ANT"],
                                          out=mC_t[:, 0:NS],
                                          in0=Hi_t[:, 0:NS],
                                          in1=Hi_t[:, 1:NS + 1])
                    nc.vector._custom_dve(ops["HMED3_ANT"],
                                          out=mB_t[:, 0:NS],
                                          in0=Me_t[:, 0:NS],
                                          in1=Me_t[:, 1:NS + 1])
                    off = 2

                # ---- final med3(mA, mB, mC) ----
                mT = f_pool.tile([128, R * WP], f16, name="mT", tag="mT")
                mU = f_pool.tile([128, R * WP], f16, name="mU", tag="mU")
                mV = m_pool.tile([128, LSZ], f16, name="mV", tag="mA")
                ot = ot_pool.tile([128, R * WP + 2], f16, name="ot",
                                  tag="ot")
                Af = mA_t[:, off:off + R * WP]
                Bf = mB_t[:, off:off + R * WP]
                Cf = mC_t[:, off:off + R * WP]
                # split the 4-op chain by element range: each engine runs an
                # independent chain over its own slice (no cross-engine deps)
                g = 0.0 if (dve_tail and k == K - 1) else final_gp_frac
                S = 2 * int(R * WP * (1.0 - g) / 2)
                for eng, s0, s1 in ((nc.vector, 0, S),
                                    (nc.gpsimd, S, R * WP)):
                    if s1 <= s0:
                        continue
                    sl = slice(s0, s1)
                    osl = slice(off + s0, off + s1)
                    eng.tensor_tensor(out=mT[:, sl], in0=mA_t[:, osl],
                                      in1=mB_t[:, osl], op=MIN)
                    eng.tensor_tensor(out=mU[:, sl], in0=mA_t[:, osl],
                                      in1=mB_t[:, osl], op=MAX)
                    eng.tensor_tensor(out=mV[:, sl], in0=mU[:, sl],
                                      in1=mC_t[:, osl], op=MIN)
                    eng.tensor_tensor(out=ot[:, sl], in0=mT[:, sl],
                                      in1=mV[:, sl], op=MAX)
                return ot

            def stage_bc_dve(k, X):
                """All-DVE variant: GPSIMD tensor_tensor min/max is
                rejected by the walrus codegen, so everything runs on the
                DVE; dead Lo/Me/Hi buffers are re-used for the final
                temporaries (same engine, in-order, so aliasing is free)."""
                for o, a, b, alu in (
                    (Pm, X[:, 1:R + 1:2], X[:, 2:R + 2:2], MIN),
                    (PM, X[:, 1:R + 1:2], X[:, 2:R + 2:2], MAX),
                    (Lo3[:, 0:R:2], X[:, 0:R:2], Pm, MIN),
                    (Hi3[:, 0:R:2], X[:, 0:R:2], PM, MAX),
                    (tEv, X[:, 0:R:2], PM, MIN),
                    (Me3[:, 0:R:2], Pm, tEv, MAX),
                    (Lo3[:, 1:R:2], X[:, 3:R + 2:2], Pm, MIN),
                    (Hi3[:, 1:R:2], X[:, 3:R + 2:2], PM, MAX),
                    (tOv, X[:, 3:R + 2:2], PM, MIN),
                    (Me3[:, 1:R:2], Pm, tOv, MAX),
                ):
                    nc.vector.tensor_tensor(out=o, in0=a, in1=b, op=alu)

                mA_t = m_pool.tile([128, LSZ], f16, name="mA", tag="mA")
                mB_t = m_pool.tile([128, LSZ], f16, name="mB", tag="mB")
                mC_t = m_pool.tile([128, LSZ], f16, name="mC", tag="mC")
                N2 = R * WP
                for opname, src, dst in (
                    ("HMAX3W_ANT", Lo_t, mA_t),
                    ("HMED3W_ANT", Me_t, mB_t),
                    ("HMIN3W_ANT", Hi_t, mC_t),
                ):
                    _emit_custom_dve(
                        nc.vector, ops[opname], out=dst[:, 0:N2],
                        in0=src[:, 0:N2], in1=src[:, 2:N2 + 2], perf_max=1)

                # final med3 (all ops even-offset fp16 step-1 -> 2x mode)
                mT = f_pool.tile([128, N2], f16, name="mT", tag="mT")
                mU = f_pool.tile([128, N2], f16, name="mU", tag="mU")
                mV = f_pool.tile([128, N2], f16, name="mV", tag="mV")
                ot = ot_pool.tile([128, R * WP + 2], f16, name="ot",
                                  tag="ot")
                sl = slice(0, N2)
                nc.vector.tensor_tensor(out=mT[:], in0=mA_t[:, sl],
                                        in1=mB_t[:, sl], op=MIN)
                nc.vector.tensor_tensor(out=mU[:], in0=mA_t[:, sl],
                                        in1=mB_t[:, sl], op=MAX)
                nc.vector.tensor_tensor(out=mV[:], in0=mU[:],
                                        in1=mC_t[:, sl], op=MIN)
                nc.vector.tensor_tensor(out=ot[:, sl], in0=mT[:],
                                        in1=mV[:], op=MAX)
                return ot

            ot_off = 1 if sliding2x else 0

            def stage_d(k, ot):
                """ACT cast fp16 -> fp32 + DMA out."""
                r0 = k * R
                o32 = o32_pool.tile([128, R * W], f32, name="o32", tag="o32")
                o3 = o32.rearrange("p (r w) -> p r w", w=W)
                ot3 = ot[:, ot_off:ot_off + R * WP].rearrange(
                    "p (r w) -> p r w", w=WP)
                nc.scalar.copy(out=o3[:], in_=ot3[:, :, 0:W])
                # separate DGE queue from the input loads (sync) so stores
                # never head-of-line-block the next loads
                nc.scalar.dma_start(out=og[:, r0:r0 + R, :], in_=o3[0:64])
                nc.scalar.dma_start(out=og[:, HH + r0:HH + r0 + R, :],
                                    in_=o3[64:128])

            Xs = {0: stage_a(0)}
            ots = {}
            ddel = 1 if use_gp else 0   # store-delay (GPSIMD-final only)
            for k in range(K):
                if k + 1 < K:
                    Xs[k + 1] = stage_a(k + 1)
                if k - ddel in ots:
                    stage_d(k - ddel, ots.pop(k - ddel))
                ots[k] = (stage_bc(k, Xs.pop(k)) if use_gp
                          else stage_bc_dve(k, Xs.pop(k)))
                if ddel == 0:
                    stage_d(k, ots.pop(k))
            for kk in sorted(ots):
                stage_d(kk, ots.pop(kk))

    nc.compile()
    return nc


def _build_v4(sizes=(4, 8, 16, 16, 16, 16, 16, 16, 12, 8), in_bufs=2,
              x16_bufs=2):
    """All-DVE variant with heterogeneous chunk sizes: small first/last
    chunks shorten the pipeline ramp/tail, large middle chunks amortize
    per-instruction overhead."""
    import concourse.bacc as bacc
    import concourse.mybir as mybir
    from concourse.tile import TileContext

    ops = _register_custom_ops()
    MIN = mybir.AluOpType.min
    MAX = mybir.AluOpType.max
    f32 = mybir.dt.float32
    f16 = mybir.dt.float16

    sizes = list(sizes)
    assert sum(sizes) == HH and all(s % 2 == 0 for s in sizes)
    K = len(sizes)
    r0s = [sum(sizes[:i]) for i in range(K)]
    RM = max(sizes)
    LSZ = RM * WP + 4

    nc = bacc.Bacc("TRN2", name="median_pool2d_v4")
    x = nc.dram_tensor("x", [C, H, W], f32, kind="ExternalInput")
    out = nc.dram_tensor("out", [C, H, W], f32, kind="ExternalOutput")
    xg = x.ap()
    og = out.ap()

    with TileContext(nc) as tc:
        with (
            tc.tile_pool(name="in32", bufs=in_bufs) as in_pool,
            tc.tile_pool(name="x16p", bufs=x16_bufs) as x16_pool,
            tc.tile_pool(name="vert", bufs=1) as v_pool,
            tc.tile_pool(name="mrg", bufs=1) as m_pool,
            tc.tile_pool(name="fin", bufs=1) as f_pool,
            tc.tile_pool(name="ot16", bufs=2) as ot_pool,
            tc.tile_pool(name="out32", bufs=1) as o32_pool,
        ):
            def stile(name):
                return v_pool.tile([128, LSZ], f16, name=name, tag=name)

            Lo_t = stile("Lo")
            Me_t = stile("Me")
            Hi_t = stile("Hi")
            for t in (Lo_t, Me_t, Hi_t):
                nc.gpsimd.memset(t[:, 0:2], 0.0)
                # zero every distinct chunk-size tail once; stale row data
                # at a smaller-chunk tail only feeds pad-column junk
                for rc in sorted(set(sizes)):
                    nc.gpsimd.memset(t[:, 2 + rc * WP:2 + rc * WP + 2], 0.0)

            def vtile(name, rows):
                return v_pool.tile([128, rows * WP], f16, name=name,
                                   tag=name)

            Pm_t = vtile("Pm", RM // 2)
            PM_t = vtile("PM", RM // 2)
            tE_t = vtile("tE", RM)

            def stage_a(k):
                rc = sizes[k]
                r0 = r0s[k]
                s32 = in_pool.tile([128, (RM + 2) * W], f32, name="s32",
                                   tag="s32")
                s3 = s32[:, 0:(rc + 2) * W].rearrange("p (r w) -> p r w",
                                                      w=W)
                if k == 0:
                    nc.gpsimd.memset(s3[0:64, 0:1], 0.0)
                    nc.sync.dma_start(out=s3[0:64, 1:rc + 2],
                                      in_=xg[:, 0:rc + 1, :])
                else:
                    nc.sync.dma_start(out=s3[0:64],
                                      in_=xg[:, r0 - 1:r0 + rc + 1, :])
                if k == K - 1:
                    nc.gpsimd.memset(s3[64:128, rc + 1:rc + 2], 0.0)
                    nc.sync.dma_start(out=s3[64:128, 0:rc + 1],
                                      in_=xg[:, HH + r0 - 1:H, :])
                else:
                    nc.sync.dma_start(
                        out=s3[64:128],
                        in_=xg[:, HH + r0 - 1:HH + r0 + rc + 1, :])
                x16 = x16_pool.tile([128, (RM + 2) * WP], f16, name="x16",
                                    tag="x16")
                X = x16[:, 0:(rc + 2) * WP].rearrange("p (r w) -> p r w",
                                                      w=WP)
                nc.gpsimd.memset(X[:, :, W:WP], 0.0)
                nc.scalar.copy(out=X[:, :, 0:W], in_=s3[:])
                return x16

            def _ap(view, dims):
                """Hand-built access pattern over `view`'s tile (keeps the
                partition dim and base offset)."""
                w = view.copy()
                p0 = list(view.ap)[0]
                w.ap = mybir.VecI64Pair(
                    [[int(p0[0]), int(p0[1])]]
                    + [[int(s), int(c)] for s, c in dims])
                return w

            def stage_b(k, x16):
                rc = sizes[k]
                Rh = rc // 2
                N2 = rc * WP
                X = x16[:, 0:(rc + 2) * WP].rearrange("p (r w) -> p r w",
                                                      w=WP)
                Pm, PM = (
                    t[:, 0:Rh * WP].rearrange("p (r w) -> p r w", w=WP)
                    for t in (Pm_t, PM_t))
                nc.vector.tensor_tensor(out=Pm, in0=X[:, 1:rc + 1:2],
                                        in1=X[:, 2:rc + 2:2], op=MIN)
                nc.vector.tensor_tensor(out=PM, in0=X[:, 1:rc + 1:2],
                                        in1=X[:, 2:rc + 2:2], op=MAX)
                # merged even/odd completions: one op per output array.
                # iteration order (pair i, parity g, col w); out rows are
                # consecutive; the X operand taps rows (2i, 2i+3); Pm/PM
                # broadcast over the parity dim (0-stride).
                grp = [[2 * WP, Rh], [3 * WP, 2], [1, WP]]
                bcast = [[WP, Rh], [0, 2], [1, WP]]
                rows = [[2 * WP, Rh], [WP, 2], [1, WP]]
                a_g = _ap(x16[:, 0:(rc + 2) * WP], grp)
                lo_o = _ap(Lo_t[:, 2:2 + N2], rows)
                hi_o = _ap(Hi_t[:, 2:2 + N2], rows)
                me_o = _ap(Me_t[:, 2:2 + N2], rows)
                pm_b = _ap(Pm_t[:, 0:Rh * WP], bcast)
                pM_b = _ap(PM_t[:, 0:Rh * WP], bcast)
                tEO = _ap(tE_t[:, 0:N2], rows)
                nc.vector.tensor_tensor(out=lo_o, in0=a_g, in1=pm_b, op=MIN)
                nc.vector.tensor_tensor(out=hi_o, in0=a_g, in1=pM_b, op=MAX)
                nc.vector.tensor_tensor(out=tEO, in0=a_g, in1=pM_b, op=MIN)
                nc.vector.tensor_tensor(out=me_o, in0=pm_b, in1=tEO,
                                        op=MAX)

                mA_t = m_pool.tile([128, LSZ], f16, name="mA", tag="mA")
                mB_t = m_pool.tile([128, LSZ], f16, name="mB", tag="mB")
                mC_t = m_pool.tile([128, LSZ], f16, name="mC", tag="mC")
                for opname, src, dst in (
                    ("HMAX3W_ANT", Lo_t, mA_t),
                    ("HMED3W_ANT", Me_t, mB_t),
                    ("HMIN3W_ANT", Hi_t, mC_t),
                ):
                    _emit_custom_dve(
                        nc.vector, ops[opname], out=dst[:, 0:N2],
                        in0=src[:, 0:N2], in1=src[:, 2:N2 + 2], perf_max=1)

                mT = f_pool.tile([128, RM * WP], f16, name="mT", tag="mT")
                mU = f_pool.tile([128, RM * WP], f16, name="mU", tag="mU")
                # mB is dead once mT/mU are done; reuse its buffer (all
                # ops on the DVE, in-order, so aliasing is dependency-free)
                mV = m_pool.tile([128, LSZ], f16, name="mV", tag="mB")
                ot = ot_pool.tile([128, RM * WP + 2], f16, name="ot",
                                  tag="ot")
                sl = slice(0, N2)
                nc.vector.tensor_tensor(out=mT[:, sl], in0=mA_t[:, sl],
                                        in1=mB_t[:, sl], op=MIN)
                nc.vector.tensor_tensor(out=mU[:, sl], in0=mA_t[:, sl],
                                        in1=mB_t[:, sl], op=MAX)
                nc.vector.tensor_tensor(out=mV[:, sl], in0=mU[:, sl],
                                        in1=mC_t[:, sl], op=MIN)
                nc.vector.tensor_tensor(out=ot[:, sl], in0=mT[:, sl],
                                        in1=mV[:, sl], op=MAX)
                return ot

            def stage_d(k, ot):
                rc = sizes[k]
                r0 = r0s[k]
                o32 = o32_pool.tile([128, RM * W], f32, name="o32",
                                    tag="o32")
                o3 = o32[:, 0:rc * W].rearrange("p (r w) -> p r w", w=W)
                ot3 = ot[:, 1:1 + rc * WP].rearrange("p (r w) -> p r w",
                                                     w=WP)
                nc.scalar.copy(out=o3[:], in_=ot3[:, :, 0:W])
                nc.scalar.dma_start(out=og[:, r0:r0 + rc, :], in_=o3[0:64])
                nc.scalar.dma_start(out=og[:, HH + r0:HH + r0 + rc, :],
                                    in_=o3[64:128])

            Xs = {0: stage_a(0)}
            for k in range(K):
                if k + 1 < K:
                    Xs[k + 1] = stage_a(k + 1)
                ot = stage_b(k, Xs.pop(k))
                stage_d(k, ot)

    nc.compile()
    return nc


def _build_shared(R=16, gp_frac=0.0, dtype="float32", in_bufs=None,
                  out_bufs=None):
    """Fallback: 15-op/pixel fp32 TT network (previous working kernel)."""
    import concourse.bacc as bacc
    import concourse.mybir as mybir
    from concourse.tile import TileContext

    MIN = mybir.AluOpType.min
    MAX = mybir.AluOpType.max
    f32 = mybir.dt.float32
    cdt = getattr(mybir.dt, dtype)

    WPP = W + 2
    assert HH % R == 0 and R % 2 == 0
    K = HH // R
    Rh = R // 2

    nc = bacc.Bacc("TRN2", name="median_pool2d_s")
    x = nc.dram_tensor("x", [C, H, W], f32, kind="ExternalInput")
    out = nc.dram_tensor("out", [C, H, W], f32, kind="ExternalOutput")
    xg = x.ap()
    og = out.ap()

    def tt(out_ap, in0, in1, op):
        nc.vector.tensor_tensor(out=out_ap, in0=in0, in1=in1, op=op)

    if in_bufs is None:
        in_bufs = 3 if R <= 8 else 2
    if out_bufs is None:
        out_bufs = 3 if R <= 8 else 1
    with TileContext(nc) as tc:
        with (
            tc.tile_pool(name="io_in", bufs=in_bufs) as in_pool,
            tc.tile_pool(name="io_out", bufs=out_bufs) as out_pool,
            tc.tile_pool(name="work", bufs=1) as w_pool,
        ):
            def wtile(name, rows, width, tag=None):
                t = w_pool.tile([128, rows * width], cdt, name=name,
                                tag=tag or name)
                return t.rearrange("p (r w) -> p r w", w=width)

            for k in range(K):
                r0 = k * R
                it = in_pool.tile([128, (R + 2) * WPP], cdt, name="it",
                                  tag="it")
                it3 = it.rearrange("p (r w) -> p r w", w=WPP)
                nc.vector.memset(it3[:, :, 0:WPP:WPP - 1], 0.0)
                if k == 0:
                    nc.vector.memset(it3[0:64, 0:1, 1:W + 1], 0.0)
                    nc.sync.dma_start(out=it3[0:64, 1:R + 2, 1:W + 1],
                                      in_=xg[:, 0:R + 1, :])
                else:
                    nc.sync.dma_start(out=it3[0:64, :, 1:W + 1],
                                      in_=xg[:, r0 - 1:r0 + R + 1, :])
                if k == K - 1:
                    nc.vector.memset(it3[64:128, R + 1:R + 2, 1:W + 1], 0.0)
                    nc.sync.dma_start(out=it3[64:128, 0:R + 1, 1:W + 1],
                                      in_=xg[:, HH + r0 - 1:H, :])
                else:
                    nc.sync.dma_start(out=it3[64:128, :, 1:W + 1],
                                      in_=xg[:, HH + r0 - 1:HH + r0 + R + 1, :])

                Pm = wtile("Pm", Rh, WPP)
                PM = wtile("PM", Rh, WPP)
                tt(Pm, it3[:, 1:R + 1:2, :], it3[:, 2:R + 2:2, :], MIN)
                tt(PM, it3[:, 1:R + 1:2, :], it3[:, 2:R + 2:2, :], MAX)

                Lo3 = wtile("Lo", R, WPP)
                Me3 = wtile("Me", R, WPP)
                Hi3 = wtile("Hi", R, WPP)
                tE = wtile("tE", Rh, WPP)
                tO = wtile("tO", Rh, WPP)
                a_e = it3[:, 0:R:2, :]
                a_o = it3[:, 3:R + 2:2, :]
                tt(Lo3[:, 0:R:2], a_e, Pm, MIN)
                tt(Hi3[:, 0:R:2], a_e, PM, MAX)
                tt(tE, a_e, PM, MIN)
                tt(Me3[:, 0:R:2], Pm, tE, MAX)
                tt(Lo3[:, 1:R:2], a_o, Pm, MIN)
                tt(Hi3[:, 1:R:2], a_o, PM, MAX)
                tt(tO, a_o, PM, MIN)
                tt(Me3[:, 1:R:2], Pm, tO, MAX)

                NP = W // 2 + 1
                PA = wtile("PA", R, NP, tag="Pm")
                PC = wtile("PC", R, NP, tag="PM")
                Um = wtile("Um", R, NP, tag="tE")
                Vm = wtile("Vm", R, NP, tag="tO")
                tBe = wtile("tBe", R, W // 2, tag="Pm")
                tBo = wtile("tBo", R, W // 2, tag="PM")
                mA = wtile("mA", R, W)
                mB = wtile("mB", R, W)
                mC = wtile("mC", R, W)

                ev = slice(0, WPP, 2)
                od = slice(1, WPP, 2)
                tt(PA, Lo3[:, :, ev], Lo3[:, :, od], MAX)
                tt(mA[:, :, 0:W:2], PA[:, :, 0:NP - 1], Lo3[:, :, 2:WPP:2],
                   MAX)
                tt(mA[:, :, 1:W:2], PA[:, :, 1:NP], Lo3[:, :, 1:WPP - 2:2],
                   MAX)

                tt(PC, Hi3[:, :, ev], Hi3[:, :, od], MIN)
                tt(mC[:, :, 0:W:2], PC[:, :, 0:NP - 1], Hi3[:, :, 2:WPP:2],
                   MIN)
                tt(mC[:, :, 1:W:2], PC[:, :, 1:NP], Hi3[:, :, 1:WPP - 2:2],
                   MIN)

                tt(Um, Me3[:, :, ev], Me3[:, :, od], MIN)
                tt(Vm, Me3[:, :, ev], Me3[:, :, od], MAX)
                tt(tBe, Me3[:, :, 2:WPP:2], Vm[:, :, 0:NP - 1], MIN)
                tt(mB[:, :, 0:W:2], Um[:, :, 0:NP - 1], tBe, MAX)
                tt(tBo, Me3[:, :, 1:WPP - 2:2], Vm[:, :, 1:NP], MIN)
                tt(mB[:, :, 1:W:2], Um[:, :, 1:NP], tBo, MAX)

                mT = wtile("mT", R, W, tag="Lo")
                mU = wtile("mU", R, W, tag="Me")
                mV = wtile("mV", R, W, tag="Hi")
                ot = out_pool.tile([128, R * W], cdt, name="ot", tag="ot")
                ot3 = ot.rearrange("p (r w) -> p r w", w=W)
                tt(mT, mA, mB, MIN)
                tt(mU, mA, mB, MAX)
                tt(mV, mU, mC, MIN)
                tt(ot3, mT, mV, MAX)

                nc.sync.dma_start(out=og[:, r0:r0 + R, :], in_=ot3[0:64])
                nc.sync.dma_start(out=og[:, HH + r0:HH + r0 + R, :],
                                  in_=ot3[64:128])

    nc.compile()
    return nc


def _get_nc(variant="v3", **kw):
    key = (variant, tuple(sorted(kw.items())))
    if key not in _CACHE:
        if variant == "v4":
            _CACHE[key] = _build_v4(**kw)
        elif variant == "v3":
            _CACHE[key] = _build_v3(**kw)
        else:
            _CACHE[key] = _build_shared(**kw)
    return _CACHE[key]


_LAST_NC = None


def kernel(x: np.ndarray) -> np.ndarray:
    """MedianPool2d(3x3, s=1, p=1) on 8 NeuronCores."""
    global _LAST_NC
    from concourse.bass_utils import run_bass_kernel_spmd

    assert x.shape == (B, C, H, W), x.shape
    x = np.ascontiguousarray(x, dtype=np.float32)
    try:
        nc = _get_nc("v4")
    except Exception:
        try:
            nc = _get_nc("v3")
        except Exception:
            nc = _get_nc("shared", R=16)
    _LAST_NC = nc
    in_maps = [{"x": x[i]} for i in range(NCORES)]
    res = run_bass_kernel_spmd(nc, in_maps, core_ids=list(range(NCORES)))
    return np.stack([r["out"] for r in res.results], axis=0)


# revision 41
# speedup vs baseline: 2.3686x; 1.0015x over previous
"""MedianPool2d (3x3, stride 1, zero-pad 1) Trainium2 Bass kernel.

Full input x: (8, 64, 256, 256) fp32.  Sharding: pure data parallel over
batch -> core i processes x[i] (64, 256, 256).

Design (v4: fp16 + custom packed-2x sliding DVE ops):
  - Per-core layout: 128 SBUF partitions = (h, c), h in {0,1} = top/bottom
    128-row half, c = channel.  Heterogeneous chunks of output rows per
    partition (small first/last chunks shorten pipeline ramp/tail).
  - DMA loads fp32 rows into a staging tile; the Activation engine casts
    fp32 -> fp16 into a padded row layout (WP=258: 256 cols + 2 zero pad
    cols).  fp16 is exact for the median network itself (pure min/max
    selection); only the input cast rounds (~2.1e-4 l2 rel, gate 2e-2).
  - Vertical sort3 (Lo/Me/Hi per output row) as fp16 tensor_tensor
    min/max on the DVE with row-pair sharing: all ops full-row step-1
    4B-aligned -> genuine 2x DVE mode (0.52 ns/elem).  The even/odd
    completions are merged into single instructions via hand-built
    access patterns (a (pair, parity, col) iteration with a 3*WP-stride
    group dim on the X operand and a 0-stride parity broadcast on the
    pair operand) - 6 vertical instructions per chunk instead of 10.
  - Horizontal 3-tap merge via three hand-authored custom DVE micro-op
    programs (HMAX3W/HMED3W/HMIN3W, registered into the per-NEFF DVE
    table at runtime): ONE streaming pass each computes
    out[j] = f(T[j], T[j+1], T[j+2]) over the flat row stream.  The APs
    (src0 = T[0:N], src1 = T[2:N+2], both fp16 step-1 4B-aligned) make
    the RTL select 2x_1p packed mode, where the crossbar exposes four
    consecutive taps per cycle (SRC_0, SRC_0_HI, SRC_1, SRC_1_HI) - both
    parities' windows are pure spatial, even result -> WR0_LO, odd ->
    WR0_HI, 2 elem/cycle.  Each row carries 2 trailing zero pad cols so
    the previous row's tail doubles as the next row's left pad; a
    2-element zero prefix seeds row 0.  HW-validated bit-exact.
  - Final med3(mA, mB, mC) as 4 more fp16 2x TTs on the DVE.
    (GPSIMD tensor_tensor min/max and DMA-CCE min/max accumulate are both
    rejected by the walrus codegen, so GPSIMD only does small memsets.)
  - Output cast fp16 -> fp32 on the Activation engine, then DMA out on a
    separate DGE queue from the loads.

Effective cost ~12 TT-op-equivalents/pixel at 2x (vs 15 at 1x fp32 for
the baseline) -> 230.8us/core vs 546.7us baseline (2.37x).
"""

import numpy as np

B, C, H, W = 8, 64, 256, 256
NCORES = 8
HH = H // 2          # rows per half-strip
WP = W + 2           # padded row width (2 trailing zero cols)

_CACHE = {}
_OPS = None


def _register_custom_ops():
    """Register the three sliding-window custom DVE ops (idempotent).

    Each op streams src0 = T[j], src1 = T[j+1] and computes a 3-tap
    window f(T[j-1], T[j], T[j+1]) per output element:
      - T[j], T[j+1] arrive spatially on the two source streams;
      - T[j-1] = src1 two elements back, read via NEXT_ALU_OUT_A from a
        pipeline stage that latches raw src1 into its a-flop each cycle.
    Outputs j=0,1 use pre-instruction flop state (garbage); callers must
    treat the first two output elements as scrap.
    """
    global _OPS
    if _OPS is not None:
        return _OPS

    import concourse.dve_ops as dve_ops
    from concourse.dve_spec import Spec, Src0, Src1, minn
    from concourse.dve_uop import (
        ENABLE,
        AluInp,
        AluOp,
        DelayInp,
        DveOpSpec,
        InpSel,
        OutPath,
        OutSel,
        Trigger,
        UopConfig,
    )

    def _shift2(in1):
        z = np.zeros_like(in1[..., :2])
        return np.concatenate([z, in1[..., :-2]], axis=-1)

    def _hmax3_ref(in0, in1, c0, c1, c2):
        return np.maximum(np.maximum(in0, in1), _shift2(in1))

    def _hmin3_ref(in0, in1, c0, c1, c2):
        return np.minimum(np.minimum(in0, in1), _shift2(in1))

    def _hmed3_ref(in0, in1, c0, c1, c2):
        c = _shift2(in1)
        return np.maximum(np.minimum(in0, in1),
                          np.minimum(np.maximum(in0, in1), c))

    def _mm3_uops(op3):
        """out[j] = op3(src0[j], src1[j], src1[j-2]) single-uop program."""
        u = UopConfig()
        u.enable_input(InpSel.SRC_0, 1)     # -> PREV_DELAY_0 at blk0
        u.enable_input(InpSel.SRC_1, 2)     # -> PREV_DELAY_1 at blk0
        u.require_inp0 = ENABLE
        u.require_inp1 = ENABLE
        u.trigger = (Trigger.SRC_TENSOR_DONE, Trigger.NONE, Trigger.NONE)
        dp = u.datapath_config
        # blk0: m = op3(src0, src1)
        dp[0].enable_alu(op3, AluInp.PREV_DELAY_0, AluInp.PREV_DELAY_1)
        dp[0].pass_through_delay(1)
        # blk1: out = op3(m, src1[j-2])  (blk2's a-flop, prev cycle)
        dp[1].enable_alu(op3, AluInp.PREV_ALU_OUT, AluInp.NEXT_ALU_OUT_A)
        dp[1].pass_through_delay(1)
        # blk2: latch raw src1 into the a-flop; stash out in delay chain 0
        dp[2].enable_alu(AluOp.BYPASS, AluInp.PREV_DELAY_1)
        dp[2].alu_out_a_enable = ENABLE
        dp[2].enable_delay_from_src(DelayInp.PREV_ALU_OUT, 0)
        for kblk in range(3, 8):
            dp[kblk].pass_through_alu().pass_through_delay(0)
        u.enable_output(OutSel.DELAY_0, OutPath.WR0_LO)
        return [u]

    def _med3_uops():
        """out[j] = med3(src0[j], src1[j], src1[j-2]) single-uop program."""
        u = UopConfig()
        u.enable_input(InpSel.SRC_0, 1)     # chain 0
        u.enable_input(InpSel.SRC_1, 2)     # chain 1
        u.require_inp0 = ENABLE
        u.require_inp1 = ENABLE
        u.trigger = (Trigger.SRC_TENSOR_DONE, Trigger.NONE, Trigger.NONE)
        dp = u.datapath_config
        # blk0: p = min(a, b)
        dp[0].enable_alu(AluOp.MIN, AluInp.PREV_DELAY_0, AluInp.PREV_DELAY_1)
        dp[0].pass_through_delay(0, 1)
        # blk1: q = max(a, b); stash p in chain 2
        dp[1].enable_alu(AluOp.MAX, AluInp.PREV_DELAY_0, AluInp.PREV_DELAY_1)
        dp[1].enable_delay_from_src(DelayInp.PREV_ALU_OUT, 2)
        dp[1].pass_through_delay(1)
        # blk2: t = min(q, c) with c = src1[j-2] via blk3's a-flop
        dp[2].enable_alu(AluOp.MIN, AluInp.PREV_ALU_OUT, AluInp.NEXT_ALU_OUT_A)
        dp[2].pass_through_delay(1, 2)
        # blk3: latch raw src1 into a-flop; stash t in chain 3
        dp[3].enable_alu(AluOp.BYPASS, AluInp.PREV_DELAY_1)
        dp[3].alu_out_a_enable = ENABLE
        dp[3].enable_delay_from_src(DelayInp.PREV_ALU_OUT, 3)
        dp[3].pass_through_delay(2)
        # blk4: out = max(p, t)
        dp[4].enable_alu(AluOp.MAX, AluInp.PREV_DELAY_2, AluInp.PREV_DELAY_3)
        for kblk in range(5, 8):
            dp[kblk].pass_through_alu()
        u.enable_output(OutSel.ALU_OUT, OutPath.WR0_LO)
        return [u]

    # ---- 2x (packed fp16) window-start variants -------------------------
    # Semantics: out[j] = f(T[j], T[j+1], T[j+2]) with src0 = T[0:N],
    # src1 = T[2:N+2], all APs fp16 step-1 4B-aligned so the RTL always
    # selects 2x_1p.  Per pair-cycle the crossbar exposes 4 consecutive
    # taps (SRC_0, SRC_0_HI, SRC_1, SRC_1_HI); both parities' windows are
    # pure spatial.  Even result -> WR0_LO, odd -> WR0_HI.

    def _shiftcat(in0, in1):
        # full stream T (len N+2) from the two views
        return np.concatenate([in0, in1[..., -2:]], axis=-1)

    def _hmax3w_ref(in0, in1, c0, c1, c2):
        T = _shiftcat(in0, in1)
        return np.maximum(np.maximum(T[..., :-2], T[..., 1:-1]), T[..., 2:])

    def _hmin3w_ref(in0, in1, c0, c1, c2):
        T = _shiftcat(in0, in1)
        return np.minimum(np.minimum(T[..., :-2], T[..., 1:-1]), T[..., 2:])

    def _hmed3w_ref(in0, in1, c0, c1, c2):
        T = _shiftcat(in0, in1)
        a, b, c = T[..., :-2], T[..., 1:-1], T[..., 2:]
        return np.maximum(np.minimum(a, b),
                          np.minimum(np.maximum(a, b), c))

    def _dummy_1x(op3):
        """Placeholder REGULAR-mode program (never selected: the APs
        always satisfy the 2x_1p conditions)."""
        u = UopConfig()
        u.enable_input(InpSel.SRC_0, 1)
        u.enable_input(InpSel.SRC_1, 2)
        u.require_inp0 = ENABLE
        u.require_inp1 = ENABLE
        u.trigger = (Trigger.SRC_TENSOR_DONE, Trigger.NONE, Trigger.NONE)
        dp = u.datapath_config
        dp[0].enable_alu(op3, AluInp.PREV_DELAY_0, AluInp.PREV_DELAY_1)
        for kblk in range(1, 8):
            dp[kblk].pass_through_alu()
        u.enable_output(OutSel.ALU_OUT, OutPath.WR0_LO)
        return [u]

    def _mm3_w2x_uops(op3):
        """2x program: E = op3(S0, S0H, S1) -> WR0_LO, O = op3(S0H, S1,
        S1H) -> WR0_HI."""
        u = UopConfig()
        u.enable_input(InpSel.SRC_0, 1)      # chain 0
        u.enable_input(InpSel.SRC_0_HI, 2)   # chain 1
        u.enable_input(InpSel.SRC_1, 3)      # chain 2
        u.enable_input(InpSel.SRC_1_HI, 4)   # chain 3
        u.require_inp0 = ENABLE
        u.require_inp1 = ENABLE
        u.trigger = (Trigger.SRC_TENSOR_DONE, Trigger.NONE, Trigger.NONE)
        dp = u.datapath_config
        dp[0].enable_alu(op3, AluInp.PREV_DELAY_0, AluInp.PREV_DELAY_1)
        dp[0].pass_through_delay(1, 2, 3)
        dp[1].enable_alu(op3, AluInp.PREV_ALU_OUT, AluInp.PREV_DELAY_2)
        dp[1].pass_through_delay(1, 2, 3)
        dp[2].enable_alu(op3, AluInp.PREV_DELAY_1, AluInp.PREV_DELAY_2)
        dp[2].enable_delay_from_src(DelayInp.PREV_ALU_OUT, 4)   # E
        dp[2].pass_through_delay(3)
        dp[3].enable_alu(op3, AluInp.PREV_ALU_OUT, AluInp.PREV_DELAY_3)
        dp[3].pass_through_delay(4)
        for kblk in range(4, 8):
            dp[kblk].pass_through_alu().pass_through_delay(4)
        u.enable_output(OutSel.DELAY_4, OutPath.WR0_LO)
        u.enable_output(OutSel.ALU_OUT, OutPath.WR0_HI)
        return [u]

    def _med3_w2x_uops():
        """2x program: E = med3(S0, S0H, S1) -> WR0_LO, O = med3(S0H, S1,
        S1H) -> WR0_HI.  Exactly 8 ALU blocks."""
        u = UopConfig()
        u.enable_input(InpSel.SRC_0, 1)      # chain 0
        u.enable_input(InpSel.SRC_0_HI, 2)   # chain 1
        u.enable_input(InpSel.SRC_1, 3)      # chain 2
        u.enable_input(InpSel.SRC_1_HI, 4)   # chain 3
        u.require_inp0 = ENABLE
        u.require_inp1 = ENABLE
        u.trigger = (Trigger.SRC_TENSOR_DONE, Trigger.NONE, Trigger.NONE)
        dp = u.datapath_config
        # E-half: med3(d0, d1, d2)
        dp[0].enable_alu(AluOp.MIN, AluInp.PREV_DELAY_0, AluInp.PREV_DELAY_1)
        dp[0].pass_through_delay(0, 1, 2, 3)
        dp[1].enable_alu(AluOp.MAX, AluInp.PREV_DELAY_0, AluInp.PREV_DELAY_1)
        dp[1].enable_delay_from_src(DelayInp.PREV_ALU_OUT, 4)   # pE
        dp[1].pass_through_delay(1, 2, 3)
        dp[2].enable_alu(AluOp.MIN, AluInp.PREV_ALU_OUT, AluInp.PREV_DELAY_2)
        dp[2].pass_through_delay(1, 2, 3, 4)
        dp[3].enable_alu(AluOp.MAX, AluInp.PREV_DELAY_4, AluInp.PREV_ALU_OUT)
        dp[3].pass_through_delay(1, 2, 3)
        # O-half: med3(d1, d2, d3)
        dp[4].enable_alu(AluOp.MIN, AluInp.PREV_DELAY_1, AluInp.PREV_DELAY_2)
        dp[4].enable_delay_from_src(DelayInp.PREV_ALU_OUT, 4)   # E
        dp[4].pass_through_delay(1, 2, 3)
        dp[5].enable_alu(AluOp.MAX, AluInp.PREV_DELAY_1, AluInp.PREV_DELAY_2)
        dp[5].enable_delay_from_src(DelayInp.PREV_ALU_OUT, 5)   # pO
        dp[5].pass_through_delay(3, 4)
        dp[6].enable_alu(AluOp.MIN, AluInp.PREV_ALU_OUT, AluInp.PREV_DELAY_3)
        dp[6].pass_through_delay(4, 5)
        dp[7].enable_alu(AluOp.MAX, AluInp.PREV_DELAY_5, AluInp.PREV_ALU_OUT)
        dp[7].pass_through_delay(4)
        u.enable_output(OutSel.DELAY_4, OutPath.WR0_LO)
        u.enable_output(OutSel.ALU_OUT, OutPath.WR0_HI)
        return [u]

    class _SlidingOp:
        """Duck-typed DveOp: hand-authored uops instead of lower(spec)."""

        def __init__(self, name, reference, uops_builder, uops_2x_builder=None):
            # body is a placeholder that reads Src0+Src1 (rd1_en) and no
            # C2/C3; only `reference` is ever evaluated (bass_interp).
            self.name = name
            self.spec = Spec(body=minn(Src0, Src1), reference=reference)
            self.subdim = False
            self.perf_en = {}
            self._builder = uops_builder
            self._builder_2x = uops_2x_builder
            self._cache = {}

        def compile(self, ver):
            if ver not in self._cache:
                s = DveOpSpec(
                    name=self.name,
                    opcode=dve_ops.get_dve_sub_opcode(self.name),
                    uops=self._builder(),
                    uops_2x=(self._builder_2x() if self._builder_2x
                             else None),
                    perf_max=1 if self._builder_2x else 0,
                    rd1_en=True,
                )
                s.validate(ver)
                self._cache[ver] = s
            return self._cache[ver]

    defs = [
        ("HMAX3_ANT", _hmax3_ref, lambda: _mm3_uops(AluOp.MAX), None),
        ("HMIN3_ANT", _hmin3_ref, lambda: _mm3_uops(AluOp.MIN), None),
        ("HMED3_ANT", _hmed3_ref, _med3_uops, None),
        ("HMAX3W_ANT", _hmax3w_ref, lambda: _dummy_1x(AluOp.MAX),
         lambda: _mm3_w2x_uops(AluOp.MAX)),
        ("HMIN3W_ANT", _hmin3w_ref, lambda: _dummy_1x(AluOp.MIN),
         lambda: _mm3_w2x_uops(AluOp.MIN)),
        ("HMED3W_ANT", _hmed3w_ref, lambda: _dummy_1x(AluOp.MIN),
         _med3_w2x_uops),
    ]
    ops = {}
    for name, ref, builder, builder2x in defs:
        if name not in dve_ops._SUB_OPCODE_FOR_NAME:
            row = max(dve_ops._SUB_OPCODE_FOR_NAME.values()) + 1
            assert row < 0x20, "custom DVE opcode rows exhausted"
            dve_ops._SUB_OPCODE_FOR_NAME[name] = row
        op = _SlidingOp(name, ref, builder, builder2x)
        # replace any previous registration (idempotent across reloads)
        dve_ops.OPS[:] = [o for o in dve_ops.OPS if o.name != name] + [op]
        dve_ops.CUSTOM_DVE_SPECS[name] = op.spec
        ops[name] = op
    _OPS = ops
    return ops


def _emit_custom_dve(v, op, out, in0, in1, perf_max=0):
    """Like bass.Vector._custom_dve but with perf_max set at construction
    (the Tile scheduler drops post-hoc attribute edits)."""
    import concourse.bass_isa as bass_isa
    import concourse.mybir as mybir
    from concourse.dve_ops import get_dve_sub_opcode

    b = v.bass
    if op.name not in b.m.ant_custom_dve_ops:
        b.m.ant_custom_dve_ops = sorted({*b.m.ant_custom_dve_ops, op.name})
    shape = bass_isa.CustomDveShape.TTSS
    isa_opcode = b.isa.Opcode[
        f"NEURON_ISA_TPB_OPCODE_CUSTOM_DVE_ANT_{shape.slot()}"
    ].value
    imm = lambda: mybir.ImmediateValue(dtype=mybir.dt.float32, value=0.0)
    return v.add_instruction(
        bass_isa.InstCustomDveAnt(
            name=b.get_next_instruction_name(),
            op_name=op.name,
            rd1_en=True,
            subdim=0,
            imm2=0.0,
            shape=shape,
            row=get_dve_sub_opcode(op.name),
            perf_max=perf_max,
            isa_opcode=isa_opcode,
            ins=[v.lower_ap(in0, for_isa=True),
                 v.lower_ap(in1, for_isa=True), imm(), imm()],
            outs=[v.lower_ap(out, for_isa=True)],
        )
    )


def _build_v3(R=8, final_gp_frac=0.65, n_vert_gp=1, in_bufs=2, mrg_bufs=1,
              x16_bufs=2, dve_tail=True, sliding2x=True, use_gp=False):
    """fp16 + custom sliding ops + ACT casts + GPSIMD final stage.

    Software-pipelined emission (engines execute their streams in order):
    per iteration k we emit  A(k+1) = load+cast of the NEXT chunk,
    D(k-1) = store of the PREVIOUS chunk, then B(k) = DVE work and
    C(k) = final stage.  This keeps the ACT in-cast ahead of the
    GPSIMD-dependent out-cast in the ACT/sync instruction streams.

    n_final_gp: how many of the 4 final med3 tensor_tensor ops run on
    GPSIMD (rest on DVE).  n_vert_gp: how many vertical ops on GPSIMD.
    dve_tail: run the last chunk's final stage on the DVE (shorter tail).
    """
    import concourse.bacc as bacc
    import concourse.mybir as mybir
    from concourse.tile import TileContext

    ops = _register_custom_ops()
    MIN = mybir.AluOpType.min
    MAX = mybir.AluOpType.max
    f32 = mybir.dt.float32
    f16 = mybir.dt.float16

    assert HH % R == 0 and R % 2 == 0
    K = HH // R
    Rh = R // 2
    NS = 2 + R * WP          # sliding stream length (2 prefix + rows)
    LSZ = R * WP + 4         # sliding tile: prefix 2 + rows + tail 2

    nc = bacc.Bacc("TRN2", name="median_pool2d_v3")
    x = nc.dram_tensor("x", [C, H, W], f32, kind="ExternalInput")
    out = nc.dram_tensor("out", [C, H, W], f32, kind="ExternalOutput")
    xg = x.ap()
    og = out.ap()

    with TileContext(nc) as tc:
        with (
            tc.tile_pool(name="in32", bufs=in_bufs) as in_pool,
            tc.tile_pool(name="x16p", bufs=x16_bufs) as x16_pool,
            tc.tile_pool(name="vert", bufs=1) as v_pool,
            tc.tile_pool(name="mrg", bufs=mrg_bufs) as m_pool,
            tc.tile_pool(name="fin", bufs=1) as f_pool,
            tc.tile_pool(name="ot16", bufs=2) as ot_pool,
            tc.tile_pool(name="out32", bufs=1) as o32_pool,
        ):
            # persistent sliding tiles: prefix/tail zeros written once
            def stile(name):
                t = v_pool.tile([128, LSZ], f16, name=name, tag=name)
                rows = t[:, 2:2 + R * WP].rearrange("p (r w) -> p r w", w=WP)
                return t, rows

            Lo_t, Lo3 = stile("Lo")
            Me_t, Me3 = stile("Me")
            Hi_t, Hi3 = stile("Hi")
            for t in (Lo_t, Me_t, Hi_t):
                nc.gpsimd.memset(t[:, 0:2], 0.0)
                nc.gpsimd.memset(t[:, 2 + R * WP:LSZ], 0.0)

            def vtile(name, rows):
                t = v_pool.tile([128, rows * WP], f16, name=name, tag=name)
                return t.rearrange("p (r w) -> p r w", w=WP)

            Pm = vtile("Pm", Rh)
            PM = vtile("PM", Rh)
            tEv = vtile("tE", Rh)
            tOv = vtile("tO", Rh)

            def stage_a(k):
                """DMA fp32 rows + ACT cast to padded fp16 rows."""
                r0 = k * R
                s32 = in_pool.tile([128, (R + 2) * W], f32, name="s32",
                                   tag="s32")
                s3 = s32.rearrange("p (r w) -> p r w", w=W)
                if k == 0:
                    nc.vector.memset(s3[0:64, 0:1], 0.0)
                    nc.sync.dma_start(out=s3[0:64, 1:R + 2],
                                      in_=xg[:, 0:R + 1, :])
                else:
                    nc.sync.dma_start(out=s3[0:64],
                                      in_=xg[:, r0 - 1:r0 + R + 1, :])
                if k == K - 1:
                    nc.vector.memset(s3[64:128, R + 1:R + 2], 0.0)
                    nc.sync.dma_start(out=s3[64:128, 0:R + 1],
                                      in_=xg[:, HH + r0 - 1:H, :])
                else:
                    nc.sync.dma_start(out=s3[64:128],
                                      in_=xg[:, HH + r0 - 1:HH + r0 + R + 1, :])
                x16 = x16_pool.tile([128, (R + 2) * WP], f16, name="x16",
                                    tag="x16")
                X = x16.rearrange("p (r w) -> p r w", w=WP)
                nc.gpsimd.memset(X[:, :, W:WP], 0.0)
                nc.scalar.copy(out=X[:, :, 0:W], in_=s3[:])
                return X

            def stage_bc(k, X):
                """DVE vertical + sliding, then final stage."""
                # ---- vertical sort3 (fp16 TT, 2x mode, pair-shared) ----
                vops = [
                    (Pm, X[:, 1:R + 1:2], X[:, 2:R + 2:2], MIN),
                    (PM, X[:, 1:R + 1:2], X[:, 2:R + 2:2], MAX),
                    (Lo3[:, 0:R:2], X[:, 0:R:2], Pm, MIN),
                    (Hi3[:, 0:R:2], X[:, 0:R:2], PM, MAX),
                    (tEv, X[:, 0:R:2], PM, MIN),
                    (Me3[:, 0:R:2], Pm, tEv, MAX),
                    (Lo3[:, 1:R:2], X[:, 3:R + 2:2], Pm, MIN),
                    (Hi3[:, 1:R:2], X[:, 3:R + 2:2], PM, MAX),
                    (tOv, X[:, 3:R + 2:2], PM, MIN),
                    (Me3[:, 1:R:2], Pm, tOv, MAX),
                ]
                for i, (o, a, b, alu) in enumerate(vops):
                    eng = (nc.gpsimd if i >= len(vops) - n_vert_gp
                           else nc.vector)
                    eng.tensor_tensor(out=o, in0=a, in1=b, op=alu)

                # ---- horizontal 3-tap merge: custom sliding DVE ops ----
                mA_t = m_pool.tile([128, LSZ], f16, name="mA", tag="mA")
                mB_t = m_pool.tile([128, LSZ], f16, name="mB", tag="mB")
                mC_t = m_pool.tile([128, LSZ], f16, name="mC", tag="mC")
                if sliding2x:
                    # window-start form at 2x: out[j] = f(T[j..j+2]);
                    # all APs even-offset fp16 step-1 -> RTL picks 2x_1p.
                    N2 = R * WP
                    for opname, src, dst in (
                        ("HMAX3W_ANT", Lo_t, mA_t),
                        ("HMED3W_ANT", Me_t, mB_t),
                        ("HMIN3W_ANT", Hi_t, mC_t),
                    ):
                        _emit_custom_dve(
                            nc.vector, ops[opname], out=dst[:, 0:N2],
                            in0=src[:, 0:N2], in1=src[:, 2:N2 + 2],
                            perf_max=1)
                    off = 0   # outputs shifted one left; final reads from 0
                else:
                    nc.vector._custom_dve(ops["HMAX3_ANT"],
                                          out=mA_t[:, 0:NS],
                                          in0=Lo_t[:, 0:NS],
                                          in1=Lo_t[:, 1:NS + 1])
                    nc.vector._custom_dve(ops["HMIN3_ANT"],
                                          out=mC_t[:, 0:NS],
                                          in0=Hi_t[:, 0:NS],
                                          in1=Hi_t[:, 1:NS + 1])
                    nc.vector._custom_dve(ops["HMED3_ANT"],
                                          out=mB_t[:, 0:NS],
                                          in0=Me_t[:, 0:NS],
                                          in1=Me_t[:, 1:NS + 1])
                    off = 2

                # ---- final med3(mA, mB, mC) ----
                mT = f_pool.tile([128, R * WP], f16, name="mT", tag="mT")
                mU = f_pool.tile([128, R * WP], f16, name="mU", tag="mU")
                mV = m_pool.tile([128, LSZ], f16, name="mV", tag="mA")
                ot = ot_pool.tile([128, R * WP + 2], f16, name="ot",
                                  tag="ot")
                Af = mA_t[:, off:off + R * WP]
                Bf = mB_t[:, off:off + R * WP]
                Cf = mC_t[:, off:off + R * WP]
                # split the 4-op chain by element range: each engine runs an
                # independent chain over its own slice (no cross-engine deps)
                g = 0.0 if (dve_tail and k == K - 1) else final_gp_frac
                S = 2 * int(R * WP * (1.0 - g) / 2)
                for eng, s0, s1 in ((nc.vector, 0, S),
                                    (nc.gpsimd, S, R * WP)):
                    if s1 <= s0:
                        continue
                    sl = slice(s0, s1)
                    osl = slice(off + s0, off + s1)
                    eng.tensor_tensor(out=mT[:, sl], in0=mA_t[:, osl],
                                      in1=mB_t[:, osl], op=MIN)
                    eng.tensor_tensor(out=mU[:, sl], in0=mA_t[:, osl],
                                      in1=mB_t[:, osl], op=MAX)
                    eng.tensor_tensor(out=mV[:, sl], in0=mU[:, sl],
                                      in1=mC_t[:, osl], op=MIN)
                    eng.tensor_tensor(out=ot[:, sl], in0=mT[:, sl],
                                      in1=mV[:, sl], op=MAX)
                return ot

            def stage_bc_dve(k, X):
                """All-DVE variant: GPSIMD tensor_tensor min/max is
                rejected by the walrus codegen, so everything runs on the
                DVE; dead Lo/Me/Hi buffers are re-used for the final
                temporaries (same engine, in-order, so aliasing is free)."""
                for o, a, b, alu in (
                    (Pm, X[:, 1:R + 1:2], X[:, 2:R + 2:2], MIN),
                    (PM, X[:, 1:R + 1:2], X[:, 2:R + 2:2], MAX),
                    (Lo3[:, 0:R:2], X[:, 0:R:2], Pm, MIN),
                    (Hi3[:, 0:R:2], X[:, 0:R:2], PM, MAX),
                    (tEv, X[:, 0:R:2], PM, MIN),
                    (Me3[:, 0:R:2], Pm, tEv, MAX),
                    (Lo3[:, 1:R:2], X[:, 3:R + 2:2], Pm, MIN),
                    (Hi3[:, 1:R:2], X[:, 3:R + 2:2], PM, MAX),
                    (tOv, X[:, 3:R + 2:2], PM, MIN),
                    (Me3[:, 1:R:2], Pm, tOv, MAX),
                ):
                    nc.vector.tensor_tensor(out=o, in0=a, in1=b, op=alu)

                mA_t = m_pool.tile([128, LSZ], f16, name="mA", tag="mA")
                mB_t = m_pool.tile([128, LSZ], f16, name="mB", tag="mB")
                mC_t = m_pool.tile([128, LSZ], f16, name="mC", tag="mC")
                N2 = R * WP
                for opname, src, dst in (
                    ("HMAX3W_ANT", Lo_t, mA_t),
                    ("HMED3W_ANT", Me_t, mB_t),
                    ("HMIN3W_ANT", Hi_t, mC_t),
                ):
                    _emit_custom_dve(
                        nc.vector, ops[opname], out=dst[:, 0:N2],
                        in0=src[:, 0:N2], in1=src[:, 2:N2 + 2], perf_max=1)

                # final med3 (all ops even-offset fp16 step-1 -> 2x mode)
                mT = f_pool.tile([128, N2], f16, name="mT", tag="mT")
                mU = f_pool.tile([128, N2], f16, name="mU", tag="mU")
                mV = f_pool.tile([128, N2], f16, name="mV", tag="mV")
                ot = ot_pool.tile([128, R * WP + 2], f16, name="ot",
                                  tag="ot")
                sl = slice(0, N2)
                nc.vector.tensor_tensor(out=mT[:], in0=mA_t[:, sl],
                                        in1=mB_t[:, sl], op=MIN)
                nc.vector.tensor_tensor(out=mU[:], in0=mA_t[:, sl],
                                        in1=mB_t[:, sl], op=MAX)
                nc.vector.tensor_tensor(out=mV[:], in0=mU[:],
                                        in1=mC_t[:, sl], op=MIN)
                nc.vector.tensor_tensor(out=ot[:, sl], in0=mT[:],
                                        in1=mV[:], op=MAX)
                return ot

            ot_off = 1 if sliding2x else 0

            def stage_d(k, ot):
                """ACT cast fp16 -> fp32 + DMA out."""
                r0 = k * R
                o32 = o32_pool.tile([128, R * W], f32, name="o32", tag="o32")
                o3 = o32.rearrange("p (r w) -> p r w", w=W)
                ot3 = ot[:, ot_off:ot_off + R * WP].rearrange(
                    "p (r w) -> p r w", w=WP)
                nc.scalar.copy(out=o3[:], in_=ot3[:, :, 0:W])
                # separate DGE queue from the input loads (sync) so stores
                # never head-of-line-block the next loads
                nc.scalar.dma_start(out=og[:, r0:r0 + R, :], in_=o3[0:64])
                nc.scalar.dma_start(out=og[:, HH + r0:HH + r0 + R, :],
                                    in_=o3[64:128])

            Xs = {0: stage_a(0)}
            ots = {}
            ddel = 1 if use_gp else 0   # store-delay (GPSIMD-final only)
            for k in range(K):
                if k + 1 < K:
                    Xs[k + 1] = stage_a(k + 1)
                if k - ddel in ots:
                    stage_d(k - ddel, ots.pop(k - ddel))
                ots[k] = (stage_bc(k, Xs.pop(k)) if use_gp
                          else stage_bc_dve(k, Xs.pop(k)))
                if ddel == 0:
                    stage_d(k, ots.pop(k))
            for kk in sorted(ots):
                stage_d(kk, ots.pop(kk))

    nc.compile()
    return nc


def _build_v4(sizes=(4, 8, 12, 12, 16, 16, 16, 16, 16, 8, 4), in_bufs=2,
              x16_bufs=2):
    """All-DVE variant with heterogeneous chunk sizes: small first/last
    chunks shorten the pipeline ramp/tail, large middle chunks amortize
    per-instruction overhead."""
    import concourse.bacc as bacc
    import concourse.mybir as mybir
    from concourse.tile import TileContext

    ops = _register_custom_ops()
    MIN = mybir.AluOpType.min
    MAX = mybir.AluOpType.max
    f32 = mybir.dt.float32
    f16 = mybir.dt.float16

    sizes = list(sizes)
    assert sum(sizes) == HH and all(s % 2 == 0 for s in sizes)
    K = len(sizes)
    r0s = [sum(sizes[:i]) for i in range(K)]
    RM = max(sizes)
    LSZ = RM * WP + 4

    nc = bacc.Bacc("TRN2", name="median_pool2d_v4")
    x = nc.dram_tensor("x", [C, H, W], f32, kind="ExternalInput")
    out = nc.dram_tensor("out", [C, H, W], f32, kind="ExternalOutput")
    xg = x.ap()
    og = out.ap()

    with TileContext(nc) as tc:
        with (
            tc.tile_pool(name="in32", bufs=in_bufs) as in_pool,
            tc.tile_pool(name="x16p", bufs=x16_bufs) as x16_pool,
            tc.tile_pool(name="vert", bufs=1) as v_pool,
            tc.tile_pool(name="mrg", bufs=1) as m_pool,
            tc.tile_pool(name="fin", bufs=1) as f_pool,
            tc.tile_pool(name="ot16", bufs=2) as ot_pool,
            tc.tile_pool(name="out32", bufs=1) as o32_pool,
        ):
            def stile(name):
                return v_pool.tile([128, LSZ], f16, name=name, tag=name)

            Lo_t = stile("Lo")
            Me_t = stile("Me")
            Hi_t = stile("Hi")
            for t in (Lo_t, Me_t, Hi_t):
                nc.gpsimd.memset(t[:, 0:2], 0.0)
                # zero every distinct chunk-size tail once; stale row data
                # at a smaller-chunk tail only feeds pad-column junk
                for rc in sorted(set(sizes)):
                    nc.gpsimd.memset(t[:, 2 + rc * WP:2 + rc * WP + 2], 0.0)

            def vtile(name, rows):
                return v_pool.tile([128, rows * WP], f16, name=name,
                                   tag=name)

            Pm_t = vtile("Pm", RM // 2)
            PM_t = vtile("PM", RM // 2)
            tE_t = vtile("tE", RM)

            def stage_a(k):
                rc = sizes[k]
                r0 = r0s[k]
                s32 = in_pool.tile([128, (RM + 2) * W], f32, name="s32",
                                   tag="s32")
                s3 = s32[:, 0:(rc + 2) * W].rearrange("p (r w) -> p r w",
                                                      w=W)
                if k == 0:
                    nc.gpsimd.memset(s3[0:64, 0:1], 0.0)
                    nc.sync.dma_start(out=s3[0:64, 1:rc + 2],
                                      in_=xg[:, 0:rc + 1, :])
                else:
                    nc.sync.dma_start(out=s3[0:64],
                                      in_=xg[:, r0 - 1:r0 + rc + 1, :])
                if k == K - 1:
                    nc.gpsimd.memset(s3[64:128, rc + 1:rc + 2], 0.0)
                    nc.sync.dma_start(out=s3[64:128, 0:rc + 1],
                                      in_=xg[:, HH + r0 - 1:H, :])
                else:
                    nc.sync.dma_start(
                        out=s3[64:128],
                        in_=xg[:, HH + r0 - 1:HH + r0 + rc + 1, :])
                x16 = x16_pool.tile([128, (RM + 2) * WP], f16, name="x16",
                                    tag="x16")
                X = x16[:, 0:(rc + 2) * WP].rearrange("p (r w) -> p r w",
                                                      w=WP)
                nc.gpsimd.memset(X[:, :, W:WP], 0.0)
                nc.scalar.copy(out=X[:, :, 0:W], in_=s3[:])
                return x16

            def _ap(view, dims):
                """Hand-built access pattern over `view`'s tile (keeps the
                partition dim and base offset)."""
                w = view.copy()
                p0 = list(view.ap)[0]
                w.ap = mybir.VecI64Pair(
                    [[int(p0[0]), int(p0[1])]]
                    + [[int(s), int(c)] for s, c in dims])
                return w

            def stage_b(k, x16):
                rc = sizes[k]
                Rh = rc // 2
                N2 = rc * WP
                X = x16[:, 0:(rc + 2) * WP].rearrange("p (r w) -> p r w",
                                                      w=WP)
                Pm, PM = (
                    t[:, 0:Rh * WP].rearrange("p (r w) -> p r w", w=WP)
                    for t in (Pm_t, PM_t))
                nc.vector.tensor_tensor(out=Pm, in0=X[:, 1:rc + 1:2],
                                        in1=X[:, 2:rc + 2:2], op=MIN)
                nc.vector.tensor_tensor(out=PM, in0=X[:, 1:rc + 1:2],
                                        in1=X[:, 2:rc + 2:2], op=MAX)
                # merged even/odd completions: one op per output array.
                # iteration order (pair i, parity g, col w); out rows are
                # consecutive; the X operand taps rows (2i, 2i+3); Pm/PM
                # broadcast over the parity dim (0-stride).
                grp = [[2 * WP, Rh], [3 * WP, 2], [1, WP]]
                bcast = [[WP, Rh], [0, 2], [1, WP]]
                rows = [[2 * WP, Rh], [WP, 2], [1, WP]]
                a_g = _ap(x16[:, 0:(rc + 2) * WP], grp)
                lo_o = _ap(Lo_t[:, 2:2 + N2], rows)
                hi_o = _ap(Hi_t[:, 2:2 + N2], rows)
                me_o = _ap(Me_t[:, 2:2 + N2], rows)
                pm_b = _ap(Pm_t[:, 0:Rh * WP], bcast)
                pM_b = _ap(PM_t[:, 0:Rh * WP], bcast)
                tEO = _ap(tE_t[:, 0:N2], rows)
                nc.vector.tensor_tensor(out=lo_o, in0=a_g, in1=pm_b, op=MIN)
                nc.vector.tensor_tensor(out=hi_o, in0=a_g, in1=pM_b, op=MAX)
                nc.vector.tensor_tensor(out=tEO, in0=a_g, in1=pM_b, op=MIN)
                nc.vector.tensor_tensor(out=me_o, in0=pm_b, in1=tEO,
                                        op=MAX)

                mA_t = m_pool.tile([128, LSZ], f16, name="mA", tag="mA")
                mB_t = m_pool.tile([128, LSZ], f16, name="mB", tag="mB")
                mC_t = m_pool.tile([128, LSZ], f16, name="mC", tag="mC")
                for opname, src, dst in (
                    ("HMAX3W_ANT", Lo_t, mA_t),
                    ("HMED3W_ANT", Me_t, mB_t),
                    ("HMIN3W_ANT", Hi_t, mC_t),
                ):
                    _emit_custom_dve(
                        nc.vector, ops[opname], out=dst[:, 0:N2],
                        in0=src[:, 0:N2], in1=src[:, 2:N2 + 2], perf_max=1)

                mT = f_pool.tile([128, RM * WP], f16, name="mT", tag="mT")
                mU = f_pool.tile([128, RM * WP], f16, name="mU", tag="mU")
                # mB is dead once mT/mU are done; reuse its buffer (all
                # ops on the DVE, in-order, so aliasing is dependency-free)
                mV = m_pool.tile([128, LSZ], f16, name="mV", tag="mB")
                ot = ot_pool.tile([128, RM * WP + 2], f16, name="ot",
                                  tag="ot")
                sl = slice(0, N2)
                nc.vector.tensor_tensor(out=mT[:, sl], in0=mA_t[:, sl],
                                        in1=mB_t[:, sl], op=MIN)
                nc.vector.tensor_tensor(out=mU[:, sl], in0=mA_t[:, sl],
                                        in1=mB_t[:, sl], op=MAX)
                nc.vector.tensor_tensor(out=mV[:, sl], in0=mU[:, sl],
                                        in1=mC_t[:, sl], op=MIN)
                nc.vector.tensor_tensor(out=ot[:, sl], in0=mT[:, sl],
                                        in1=mV[:, sl], op=MAX)
                return ot

            def stage_d(k, ot):
                rc = sizes[k]
                r0 = r0s[k]
                o32 = o32_pool.tile([128, RM * W], f32, name="o32",
                                    tag="o32")
                o3 = o32[:, 0:rc * W].rearrange("p (r w) -> p r w", w=W)
                ot3 = ot[:, 1:1 + rc * WP].rearrange("p (r w) -> p r w",
                                                     w=WP)
                nc.scalar.copy(out=o3[:], in_=ot3[:, :, 0:W])
                nc.scalar.dma_start(out=og[:, r0:r0 + rc, :], in_=o3[0:64])
                nc.scalar.dma_start(out=og[:, HH + r0:HH + r0 + rc, :],
                                    in_=o3[64:128])

            Xs = {0: stage_a(0)}
            for k in range(K):
                if k + 1 < K:
                    Xs[k + 1] = stage_a(k + 1)
                ot = stage_b(k, Xs.pop(k))
                stage_d(k, ot)

    nc.compile()
    return nc


def _build_shared(R=16, gp_frac=0.0, dtype="float32", in_bufs=None,
                  out_bufs=None):
    """Fallback: 15-op/pixel fp32 TT network (previous working kernel)."""
    import concourse.bacc as bacc
    import concourse.mybir as mybir
    from concourse.tile import TileContext

    MIN = mybir.AluOpType.min
    MAX = mybir.AluOpType.max
    f32 = mybir.dt.float32
    cdt = getattr(mybir.dt, dtype)

    WPP = W + 2
    assert HH % R == 0 and R % 2 == 0
    K = HH // R
    Rh = R // 2

    nc = bacc.Bacc("TRN2", name="median_pool2d_s")
    x = nc.dram_tensor("x", [C, H, W], f32, kind="ExternalInput")
    out = nc.dram_tensor("out", [C, H, W], f32, kind="ExternalOutput")
    xg = x.ap()
    og = out.ap()

    def tt(out_ap, in0, in1, op):
        nc.vector.tensor_tensor(out=out_ap, in0=in0, in1=in1, op=op)

    if in_bufs is None:
        in_bufs = 3 if R <= 8 else 2
    if out_bufs is None:
        out_bufs = 3 if R <= 8 else 1
    with TileContext(nc) as tc:
        with (
            tc.tile_pool(name="io_in", bufs=in_bufs) as in_pool,
            tc.tile_pool(name="io_out", bufs=out_bufs) as out_pool,
            tc.tile_pool(name="work", bufs=1) as w_pool,
        ):
            def wtile(name, rows, width, tag=None):
                t = w_pool.tile([128, rows * width], cdt, name=name,
                                tag=tag or name)
                return t.rearrange("p (r w) -> p r w", w=width)

            for k in range(K):
                r0 = k * R
                it = in_pool.tile([128, (R + 2) * WPP], cdt, name="it",
                                  tag="it")
                it3 = it.rearrange("p (r w) -> p r w", w=WPP)
                nc.vector.memset(it3[:, :, 0:WPP:WPP - 1], 0.0)
                if k == 0:
                    nc.vector.memset(it3[0:64, 0:1, 1:W + 1], 0.0)
                    nc.sync.dma_start(out=it3[0:64, 1:R + 2, 1:W + 1],
                                      in_=xg[:, 0:R + 1, :])
                else:
                    nc.sync.dma_start(out=it3[0:64, :, 1:W + 1],
                                      in_=xg[:, r0 - 1:r0 + R + 1, :])
                if k == K - 1:
                    nc.vector.memset(it3[64:128, R + 1:R + 2, 1:W + 1], 0.0)
                    nc.sync.dma_start(out=it3[64:128, 0:R + 1, 1:W + 1],
                                      in_=xg[:, HH + r0 - 1:H, :])
                else:
                    nc.sync.dma_start(out=it3[64:128, :, 1:W + 1],
                                      in_=xg[:, HH + r0 - 1:HH + r0 + R + 1, :])

                Pm = wtile("Pm", Rh, WPP)
                PM = wtile("PM", Rh, WPP)
                tt(Pm, it3[:, 1:R + 1:2, :], it3[:, 2:R + 2:2, :], MIN)
                tt(PM, it3[:, 1:R + 1:2, :], it3[:, 2:R + 2:2, :], MAX)

                Lo3 = wtile("Lo", R, WPP)
                Me3 = wtile("Me", R, WPP)
                Hi3 = wtile("Hi", R, WPP)
                tE = wtile("tE", Rh, WPP)
                tO = wtile("tO", Rh, WPP)
                a_e = it3[:, 0:R:2, :]
                a_o = it3[:, 3:R + 2:2, :]
                tt(Lo3[:, 0:R:2], a_e, Pm, MIN)
                tt(Hi3[:, 0:R:2], a_e, PM, MAX)
                tt(tE, a_e, PM, MIN)
                tt(Me3[:, 0:R:2], Pm, tE, MAX)
                tt(Lo3[:, 1:R:2], a_o, Pm, MIN)
                tt(Hi3[:, 1:R:2], a_o, PM, MAX)
                tt(tO, a_o, PM, MIN)
                tt(Me3[:, 1:R:2], Pm, tO, MAX)

                NP = W // 2 + 1
                PA = wtile("PA", R, NP, tag="Pm")
                PC = wtile("PC", R, NP, tag="PM")
                Um = wtile("Um", R, NP, tag="tE")
                Vm = wtile("Vm", R, NP, tag="tO")
                tBe = wtile("tBe", R, W // 2, tag="Pm")
                tBo = wtile("tBo", R, W // 2, tag="PM")
                mA = wtile("mA", R, W)
                mB = wtile("mB", R, W)
                mC = wtile("mC", R, W)

                ev = slice(0, WPP, 2)
                od = slice(1, WPP, 2)
                tt(PA, Lo3[:, :, ev], Lo3[:, :, od], MAX)
                tt(mA[:, :, 0:W:2], PA[:, :, 0:NP - 1], Lo3[:, :, 2:WPP:2],
                   MAX)
                tt(mA[:, :, 1:W:2], PA[:, :, 1:NP], Lo3[:, :, 1:WPP - 2:2],
                   MAX)

                tt(PC, Hi3[:, :, ev], Hi3[:, :, od], MIN)
                tt(mC[:, :, 0:W:2], PC[:, :, 0:NP - 1], Hi3[:, :, 2:WPP:2],
                   MIN)
                tt(mC[:, :, 1:W:2], PC[:, :, 1:NP], Hi3[:, :, 1:WPP - 2:2],
                   MIN)

                tt(Um, Me3[:, :, ev], Me3[:, :, od], MIN)
                tt(Vm, Me3[:, :, ev], Me3[:, :, od], MAX)
                tt(tBe, Me3[:, :, 2:WPP:2], Vm[:, :, 0:NP - 1], MIN)
                tt(mB[:, :, 0:W:2], Um[:, :, 0:NP - 1], tBe, MAX)
                tt(tBo, Me3[:, :, 1:WPP - 2:2], Vm[:, :, 1:NP], MIN)
                tt(mB[:, :, 1:W:2], Um[:, :, 1:NP], tBo, MAX)

                mT = wtile("mT", R, W, tag="Lo")
                mU = wtile("mU", R, W, tag="Me")
                mV = wtile("mV", R, W, tag="Hi")
                ot = out_pool.tile([128, R * W], cdt, name="ot", tag="ot")
                ot3 = ot.rearrange("p (r w) -> p r w", w=W)
                tt(mT, mA, mB, MIN)
                tt(mU, mA, mB, MAX)
                tt(mV, mU, mC, MIN)
                tt(ot3, mT, mV, MAX)

                nc.sync.dma_start(out=og[:, r0:r0 + R, :], in_=ot3[0:64])
                nc.sync.dma_start(out=og[:, HH + r0:HH + r0 + R, :],
                                  in_=ot3[64:128])

    nc.compile()
    return nc


def _get_nc(variant="v3", **kw):
    key = (variant, tuple(sorted(kw.items())))
    if key not in _CACHE:
        if variant == "v4":
            _CACHE[key] = _build_v4(**kw)
        elif variant == "v3":
            _CACHE[key] = _build_v3(**kw)
        else:
            _CACHE[key] = _build_shared(**kw)
    return _CACHE[key]


_LAST_NC = None


def kernel(x: np.ndarray) -> np.ndarray:
    """MedianPool2d(3x3, s=1, p=1) on 8 NeuronCores."""
    global _LAST_NC
    from concourse.bass_utils import run_bass_kernel_spmd

    assert x.shape == (B, C, H, W), x.shape
    x = np.ascontiguousarray(x, dtype=np.float32)
    try:
        nc = _get_nc("v4")
    except Exception:
        try:
            nc = _get_nc("v3")
        except Exception:
            nc = _get_nc("shared", R=16)
    _LAST_NC = nc
    in_maps = [{"x": x[i]} for i in range(NCORES)]
    res = run_bass_kernel_spmd(nc, in_maps, core_ids=list(range(NCORES)))
    return np.stack([r["out"] for r in res.results], axis=0)


# revision 43
# speedup vs baseline: 2.3797x; 1.0047x over previous
"""MedianPool2d (3x3, stride 1, zero-pad 1) Trainium2 Bass kernel.

Full input x: (8, 64, 256, 256) fp32.  Sharding: pure data parallel over
batch -> core i processes x[i] (64, 256, 256).

Design (v4: fp16 + custom packed-2x sliding DVE ops):
  - Per-core layout: 128 SBUF partitions = (h, c), h in {0,1} = top/bottom
    128-row half, c = channel.  Heterogeneous chunks of output rows per
    partition (small first/last chunks shorten pipeline ramp/tail).
  - DMA loads fp32 rows into a staging tile; the Activation engine casts
    fp32 -> fp16 into a padded row layout (WP=258: 256 cols + 2 zero pad
    cols).  fp16 is exact for the median network itself (pure min/max
    selection); only the input cast rounds (~2.1e-4 l2 rel, gate 2e-2).
  - Vertical sort3 (Lo/Me/Hi per output row) as fp16 tensor_tensor
    min/max on the DVE with row-pair sharing: all ops full-row step-1
    4B-aligned -> genuine 2x DVE mode (0.52 ns/elem).  The even/odd
    completions are merged into single instructions via hand-built
    access patterns (a (pair, parity, col) iteration with a 3*WP-stride
    group dim on the X operand and a 0-stride parity broadcast on the
    pair operand) - 6 vertical instructions per chunk instead of 10.
  - Horizontal 3-tap merge via three hand-authored custom DVE micro-op
    programs (HMAX3W/HMED3W/HMIN3W, registered into the per-NEFF DVE
    table at runtime): ONE streaming pass each computes
    out[j] = f(T[j], T[j+1], T[j+2]) over the flat row stream.  The APs
    (src0 = T[0:N], src1 = T[2:N+2], both fp16 step-1 4B-aligned) make
    the RTL select 2x_1p packed mode, where the crossbar exposes four
    consecutive taps per cycle (SRC_0, SRC_0_HI, SRC_1, SRC_1_HI) - both
    parities' windows are pure spatial, even result -> WR0_LO, odd ->
    WR0_HI, 2 elem/cycle.  Each row carries 2 trailing zero pad cols so
    the previous row's tail doubles as the next row's left pad; a
    2-element zero prefix seeds row 0.  HW-validated bit-exact.
  - Final med3(mA, mB, mC) as 4 more fp16 2x TTs on the DVE.
    (GPSIMD tensor_tensor min/max and DMA-CCE min/max accumulate are both
    rejected by the walrus codegen, so GPSIMD only does small memsets.)
  - Output cast fp16 -> fp32 on the Activation engine, then DMA out on a
    separate DGE queue from the loads.

Effective cost ~12 TT-op-equivalents/pixel at 2x (vs 15 at 1x fp32 for
the baseline) -> 229.7us/core vs 546.7us baseline (2.38x).
"""

import numpy as np

B, C, H, W = 8, 64, 256, 256
NCORES = 8
HH = H // 2          # rows per half-strip
WP = W + 2           # padded row width (2 trailing zero cols)

_CACHE = {}
_OPS = None


def _register_custom_ops():
    """Register the three sliding-window custom DVE ops (idempotent).

    Each op streams src0 = T[j], src1 = T[j+1] and computes a 3-tap
    window f(T[j-1], T[j], T[j+1]) per output element:
      - T[j], T[j+1] arrive spatially on the two source streams;
      - T[j-1] = src1 two elements back, read via NEXT_ALU_OUT_A from a
        pipeline stage that latches raw src1 into its a-flop each cycle.
    Outputs j=0,1 use pre-instruction flop state (garbage); callers must
    treat the first two output elements as scrap.
    """
    global _OPS
    if _OPS is not None:
        return _OPS

    import concourse.dve_ops as dve_ops
    from concourse.dve_spec import Spec, Src0, Src1, minn
    from concourse.dve_uop import (
        ENABLE,
        AluInp,
        AluOp,
        DelayInp,
        DveOpSpec,
        InpSel,
        OutPath,
        OutSel,
        Trigger,
        UopConfig,
    )

    def _shift2(in1):
        z = np.zeros_like(in1[..., :2])
        return np.concatenate([z, in1[..., :-2]], axis=-1)

    def _hmax3_ref(in0, in1, c0, c1, c2):
        return np.maximum(np.maximum(in0, in1), _shift2(in1))

    def _hmin3_ref(in0, in1, c0, c1, c2):
        return np.minimum(np.minimum(in0, in1), _shift2(in1))

    def _hmed3_ref(in0, in1, c0, c1, c2):
        c = _shift2(in1)
        return np.maximum(np.minimum(in0, in1),
                          np.minimum(np.maximum(in0, in1), c))

    def _mm3_uops(op3):
        """out[j] = op3(src0[j], src1[j], src1[j-2]) single-uop program."""
        u = UopConfig()
        u.enable_input(InpSel.SRC_0, 1)     # -> PREV_DELAY_0 at blk0
        u.enable_input(InpSel.SRC_1, 2)     # -> PREV_DELAY_1 at blk0
        u.require_inp0 = ENABLE
        u.require_inp1 = ENABLE
        u.trigger = (Trigger.SRC_TENSOR_DONE, Trigger.NONE, Trigger.NONE)
        dp = u.datapath_config
        # blk0: m = op3(src0, src1)
        dp[0].enable_alu(op3, AluInp.PREV_DELAY_0, AluInp.PREV_DELAY_1)
        dp[0].pass_through_delay(1)
        # blk1: out = op3(m, src1[j-2])  (blk2's a-flop, prev cycle)
        dp[1].enable_alu(op3, AluInp.PREV_ALU_OUT, AluInp.NEXT_ALU_OUT_A)
        dp[1].pass_through_delay(1)
        # blk2: latch raw src1 into the a-flop; stash out in delay chain 0
        dp[2].enable_alu(AluOp.BYPASS, AluInp.PREV_DELAY_1)
        dp[2].alu_out_a_enable = ENABLE
        dp[2].enable_delay_from_src(DelayInp.PREV_ALU_OUT, 0)
        for kblk in range(3, 8):
            dp[kblk].pass_through_alu().pass_through_delay(0)
        u.enable_output(OutSel.DELAY_0, OutPath.WR0_LO)
        return [u]

    def _med3_uops():
        """out[j] = med3(src0[j], src1[j], src1[j-2]) single-uop program."""
        u = UopConfig()
        u.enable_input(InpSel.SRC_0, 1)     # chain 0
        u.enable_input(InpSel.SRC_1, 2)     # chain 1
        u.require_inp0 = ENABLE
        u.require_inp1 = ENABLE
        u.trigger = (Trigger.SRC_TENSOR_DONE, Trigger.NONE, Trigger.NONE)
        dp = u.datapath_config
        # blk0: p = min(a, b)
        dp[0].enable_alu(AluOp.MIN, AluInp.PREV_DELAY_0, AluInp.PREV_DELAY_1)
        dp[0].pass_through_delay(0, 1)
        # blk1: q = max(a, b); stash p in chain 2
        dp[1].enable_alu(AluOp.MAX, AluInp.PREV_DELAY_0, AluInp.PREV_DELAY_1)
        dp[1].enable_delay_from_src(DelayInp.PREV_ALU_OUT, 2)
        dp[1].pass_through_delay(1)
        # blk2: t = min(q, c) with c = src1[j-2] via blk3's a-flop
        dp[2].enable_alu(AluOp.MIN, AluInp.PREV_ALU_OUT, AluInp.NEXT_ALU_OUT_A)
        dp[2].pass_through_delay(1, 2)
        # blk3: latch raw src1 into a-flop; stash t in chain 3
        dp[3].enable_alu(AluOp.BYPASS, AluInp.PREV_DELAY_1)
        dp[3].alu_out_a_enable = ENABLE
        dp[3].enable_delay_from_src(DelayInp.PREV_ALU_OUT, 3)
        dp[3].pass_through_delay(2)
        # blk4: out = max(p, t)
        dp[4].enable_alu(AluOp.MAX, AluInp.PREV_DELAY_2, AluInp.PREV_DELAY_3)
        for kblk in range(5, 8):
            dp[kblk].pass_through_alu()
        u.enable_output(OutSel.ALU_OUT, OutPath.WR0_LO)
        return [u]

    # ---- 2x (packed fp16) window-start variants -------------------------
    # Semantics: out[j] = f(T[j], T[j+1], T[j+2]) with src0 = T[0:N],
    # src1 = T[2:N+2], all APs fp16 step-1 4B-aligned so the RTL always
    # selects 2x_1p.  Per pair-cycle the crossbar exposes 4 consecutive
    # taps (SRC_0, SRC_0_HI, SRC_1, SRC_1_HI); both parities' windows are
    # pure spatial.  Even result -> WR0_LO, odd -> WR0_HI.

    def _shiftcat(in0, in1):
        # full stream T (len N+2) from the two views
        return np.concatenate([in0, in1[..., -2:]], axis=-1)

    def _hmax3w_ref(in0, in1, c0, c1, c2):
        T = _shiftcat(in0, in1)
        return np.maximum(np.maximum(T[..., :-2], T[..., 1:-1]), T[..., 2:])

    def _hmin3w_ref(in0, in1, c0, c1, c2):
        T = _shiftcat(in0, in1)
        return np.minimum(np.minimum(T[..., :-2], T[..., 1:-1]), T[..., 2:])

    def _hmed3w_ref(in0, in1, c0, c1, c2):
        T = _shiftcat(in0, in1)
        a, b, c = T[..., :-2], T[..., 1:-1], T[..., 2:]
        return np.maximum(np.minimum(a, b),
                          np.minimum(np.maximum(a, b), c))

    def _dummy_1x(op3):
        """Placeholder REGULAR-mode program (never selected: the APs
        always satisfy the 2x_1p conditions)."""
        u = UopConfig()
        u.enable_input(InpSel.SRC_0, 1)
        u.enable_input(InpSel.SRC_1, 2)
        u.require_inp0 = ENABLE
        u.require_inp1 = ENABLE
        u.trigger = (Trigger.SRC_TENSOR_DONE, Trigger.NONE, Trigger.NONE)
        dp = u.datapath_config
        dp[0].enable_alu(op3, AluInp.PREV_DELAY_0, AluInp.PREV_DELAY_1)
        for kblk in range(1, 8):
            dp[kblk].pass_through_alu()
        u.enable_output(OutSel.ALU_OUT, OutPath.WR0_LO)
        return [u]

    def _mm3_w2x_uops(op3):
        """2x program: E = op3(S0, S0H, S1) -> WR0_LO, O = op3(S0H, S1,
        S1H) -> WR0_HI."""
        u = UopConfig()
        u.enable_input(InpSel.SRC_0, 1)      # chain 0
        u.enable_input(InpSel.SRC_0_HI, 2)   # chain 1
        u.enable_input(InpSel.SRC_1, 3)      # chain 2
        u.enable_input(InpSel.SRC_1_HI, 4)   # chain 3
        u.require_inp0 = ENABLE
        u.require_inp1 = ENABLE
        u.trigger = (Trigger.SRC_TENSOR_DONE, Trigger.NONE, Trigger.NONE)
        dp = u.datapath_config
        dp[0].enable_alu(op3, AluInp.PREV_DELAY_0, AluInp.PREV_DELAY_1)
        dp[0].pass_through_delay(1, 2, 3)
        dp[1].enable_alu(op3, AluInp.PREV_ALU_OUT, AluInp.PREV_DELAY_2)
        dp[1].pass_through_delay(1, 2, 3)
        dp[2].enable_alu(op3, AluInp.PREV_DELAY_1, AluInp.PREV_DELAY_2)
        dp[2].enable_delay_from_src(DelayInp.PREV_ALU_OUT, 4)   # E
        dp[2].pass_through_delay(3)
        dp[3].enable_alu(op3, AluInp.PREV_ALU_OUT, AluInp.PREV_DELAY_3)
        dp[3].pass_through_delay(4)
        for kblk in range(4, 8):
            dp[kblk].pass_through_alu().pass_through_delay(4)
        u.enable_output(OutSel.DELAY_4, OutPath.WR0_LO)
        u.enable_output(OutSel.ALU_OUT, OutPath.WR0_HI)
        return [u]

    def _med3_w2x_uops():
        """2x program: E = med3(S0, S0H, S1) -> WR0_LO, O = med3(S0H, S1,
        S1H) -> WR0_HI.  Exactly 8 ALU blocks."""
        u = UopConfig()
        u.enable_input(InpSel.SRC_0, 1)      # chain 0
        u.enable_input(InpSel.SRC_0_HI, 2)   # chain 1
        u.enable_input(InpSel.SRC_1, 3)      # chain 2
        u.enable_input(InpSel.SRC_1_HI, 4)   # chain 3
        u.require_inp0 = ENABLE
        u.require_inp1 = ENABLE
        u.trigger = (Trigger.SRC_TENSOR_DONE, Trigger.NONE, Trigger.NONE)
        dp = u.datapath_config
        # E-half: med3(d0, d1, d2)
        dp[0].enable_alu(AluOp.MIN, AluInp.PREV_DELAY_0, AluInp.PREV_DELAY_1)
        dp[0].pass_through_delay(0, 1, 2, 3)
        dp[1].enable_alu(AluOp.MAX, AluInp.PREV_DELAY_0, AluInp.PREV_DELAY_1)
        dp[1].enable_delay_from_src(DelayInp.PREV_ALU_OUT, 4)   # pE
        dp[1].pass_through_delay(1, 2, 3)
        dp[2].enable_alu(AluOp.MIN, AluInp.PREV_ALU_OUT, AluInp.PREV_DELAY_2)
        dp[2].pass_through_delay(1, 2, 3, 4)
        dp[3].enable_alu(AluOp.MAX, AluInp.PREV_DELAY_4, AluInp.PREV_ALU_OUT)
        dp[3].pass_through_delay(1, 2, 3)
        # O-half: med3(d1, d2, d3)
        dp[4].enable_alu(AluOp.MIN, AluInp.PREV_DELAY_1, AluInp.PREV_DELAY_2)
        dp[4].enable_delay_from_src(DelayInp.PREV_ALU_OUT, 4)   # E
        dp[4].pass_through_delay(1, 2, 3)
        dp[5].enable_alu(AluOp.MAX, AluInp.PREV_DELAY_1, AluInp.PREV_DELAY_2)
        dp[5].enable_delay_from_src(DelayInp.PREV_ALU_OUT, 5)   # pO
        dp[5].pass_through_delay(3, 4)
        dp[6].enable_alu(AluOp.MIN, AluInp.PREV_ALU_OUT, AluInp.PREV_DELAY_3)
        dp[6].pass_through_delay(4, 5)
        dp[7].enable_alu(AluOp.MAX, AluInp.PREV_DELAY_5, AluInp.PREV_ALU_OUT)
        dp[7].pass_through_delay(4)
        u.enable_output(OutSel.DELAY_4, OutPath.WR0_LO)
        u.enable_output(OutSel.ALU_OUT, OutPath.WR0_HI)
        return [u]

    class _SlidingOp:
        """Duck-typed DveOp: hand-authored uops instead of lower(spec)."""

        def __init__(self, name, reference, uops_builder, uops_2x_builder=None):
            # body is a placeholder that reads Src0+Src1 (rd1_en) and no
            # C2/C3; only `reference` is ever evaluated (bass_interp).
            self.name = name
            self.spec = Spec(body=minn(Src0, Src1), reference=reference)
            self.subdim = False
            self.perf_en = {}
            self._builder = uops_builder
            self._builder_2x = uops_2x_builder
            self._cache = {}

        def compile(self, ver):
            if ver not in self._cache:
                s = DveOpSpec(
                    name=self.name,
                    opcode=dve_ops.get_dve_sub_opcode(self.name),
                    uops=self._builder(),
                    uops_2x=(self._builder_2x() if self._builder_2x
                             else None),
                    perf_max=1 if self._builder_2x else 0,
                    rd1_en=True,
                )
                s.validate(ver)
                self._cache[ver] = s
            return self._cache[ver]

    defs = [
        ("HMAX3_ANT", _hmax3_ref, lambda: _mm3_uops(AluOp.MAX), None),
        ("HMIN3_ANT", _hmin3_ref, lambda: _mm3_uops(AluOp.MIN), None),
        ("HMED3_ANT", _hmed3_ref, _med3_uops, None),
        ("HMAX3W_ANT", _hmax3w_ref, lambda: _dummy_1x(AluOp.MAX),
         lambda: _mm3_w2x_uops(AluOp.MAX)),
        ("HMIN3W_ANT", _hmin3w_ref, lambda: _dummy_1x(AluOp.MIN),
         lambda: _mm3_w2x_uops(AluOp.MIN)),
        ("HMED3W_ANT", _hmed3w_ref, lambda: _dummy_1x(AluOp.MIN),
         _med3_w2x_uops),
    ]
    ops = {}
    for name, ref, builder, builder2x in defs:
        if name not in dve_ops._SUB_OPCODE_FOR_NAME:
            row = max(dve_ops._SUB_OPCODE_FOR_NAME.values()) + 1
            assert row < 0x20, "custom DVE opcode rows exhausted"
            dve_ops._SUB_OPCODE_FOR_NAME[name] = row
        op = _SlidingOp(name, ref, builder, builder2x)
        # replace any previous registration (idempotent across reloads)
        dve_ops.OPS[:] = [o for o in dve_ops.OPS if o.name != name] + [op]
        dve_ops.CUSTOM_DVE_SPECS[name] = op.spec
        ops[name] = op
    _OPS = ops
    return ops


def _emit_custom_dve(v, op, out, in0, in1, perf_max=0):
    """Like bass.Vector._custom_dve but with perf_max set at construction
    (the Tile scheduler drops post-hoc attribute edits)."""
    import concourse.bass_isa as bass_isa
    import concourse.mybir as mybir
    from concourse.dve_ops import get_dve_sub_opcode

    b = v.bass
    if op.name not in b.m.ant_custom_dve_ops:
        b.m.ant_custom_dve_ops = sorted({*b.m.ant_custom_dve_ops, op.name})
    shape = bass_isa.CustomDveShape.TTSS
    isa_opcode = b.isa.Opcode[
        f"NEURON_ISA_TPB_OPCODE_CUSTOM_DVE_ANT_{shape.slot()}"
    ].value
    imm = lambda: mybir.ImmediateValue(dtype=mybir.dt.float32, value=0.0)
    return v.add_instruction(
        bass_isa.InstCustomDveAnt(
            name=b.get_next_instruction_name(),
            op_name=op.name,
            rd1_en=True,
            subdim=0,
            imm2=0.0,
            shape=shape,
            row=get_dve_sub_opcode(op.name),
            perf_max=perf_max,
            isa_opcode=isa_opcode,
            ins=[v.lower_ap(in0, for_isa=True),
                 v.lower_ap(in1, for_isa=True), imm(), imm()],
            outs=[v.lower_ap(out, for_isa=True)],
        )
    )


def _build_v3(R=8, final_gp_frac=0.65, n_vert_gp=1, in_bufs=2, mrg_bufs=1,
              x16_bufs=2, dve_tail=True, sliding2x=True, use_gp=False):
    """fp16 + custom sliding ops + ACT casts + GPSIMD final stage.

    Software-pipelined emission (engines execute their streams in order):
    per iteration k we emit  A(k+1) = load+cast of the NEXT chunk,
    D(k-1) = store of the PREVIOUS chunk, then B(k) = DVE work and
    C(k) = final stage.  This keeps the ACT in-cast ahead of the
    GPSIMD-dependent out-cast in the ACT/sync instruction streams.

    n_final_gp: how many of the 4 final med3 tensor_tensor ops run on
    GPSIMD (rest on DVE).  n_vert_gp: how many vertical ops on GPSIMD.
    dve_tail: run the last chunk's final stage on the DVE (shorter tail).
    """
    import concourse.bacc as bacc
    import concourse.mybir as mybir
    from concourse.tile import TileContext

    ops = _register_custom_ops()
    MIN = mybir.AluOpType.min
    MAX = mybir.AluOpType.max
    f32 = mybir.dt.float32
    f16 = mybir.dt.float16

    assert HH % R == 0 and R % 2 == 0
    K = HH // R
    Rh = R // 2
    NS = 2 + R * WP          # sliding stream length (2 prefix + rows)
    LSZ = R * WP + 4         # sliding tile: prefix 2 + rows + tail 2

    nc = bacc.Bacc("TRN2", name="median_pool2d_v3")
    x = nc.dram_tensor("x", [C, H, W], f32, kind="ExternalInput")
    out = nc.dram_tensor("out", [C, H, W], f32, kind="ExternalOutput")
    xg = x.ap()
    og = out.ap()

    with TileContext(nc) as tc:
        with (
            tc.tile_pool(name="in32", bufs=in_bufs) as in_pool,
            tc.tile_pool(name="x16p", bufs=x16_bufs) as x16_pool,
            tc.tile_pool(name="vert", bufs=1) as v_pool,
            tc.tile_pool(name="mrg", bufs=mrg_bufs) as m_pool,
            tc.tile_pool(name="fin", bufs=1) as f_pool,
            tc.tile_pool(name="ot16", bufs=2) as ot_pool,
            tc.tile_pool(name="out32", bufs=1) as o32_pool,
        ):
            # persistent sliding tiles: prefix/tail zeros written once
            def stile(name):
                t = v_pool.tile([128, LSZ], f16, name=name, tag=name)
                rows = t[:, 2:2 + R * WP].rearrange("p (r w) -> p r w", w=WP)
                return t, rows

            Lo_t, Lo3 = stile("Lo")
            Me_t, Me3 = stile("Me")
            Hi_t, Hi3 = stile("Hi")
            for t in (Lo_t, Me_t, Hi_t):
                nc.gpsimd.memset(t[:, 0:2], 0.0)
                nc.gpsimd.memset(t[:, 2 + R * WP:LSZ], 0.0)

            def vtile(name, rows):
                t = v_pool.tile([128, rows * WP], f16, name=name, tag=name)
                return t.rearrange("p (r w) -> p r w", w=WP)

            Pm = vtile("Pm", Rh)
            PM = vtile("PM", Rh)
            tEv = vtile("tE", Rh)
            tOv = vtile("tO", Rh)

            def stage_a(k):
                """DMA fp32 rows + ACT cast to padded fp16 rows."""
                r0 = k * R
                s32 = in_pool.tile([128, (R + 2) * W], f32, name="s32",
                                   tag="s32")
                s3 = s32.rearrange("p (r w) -> p r w", w=W)
                if k == 0:
                    nc.vector.memset(s3[0:64, 0:1], 0.0)
                    nc.sync.dma_start(out=s3[0:64, 1:R + 2],
                                      in_=xg[:, 0:R + 1, :])
                else:
                    nc.sync.dma_start(out=s3[0:64],
                                      in_=xg[:, r0 - 1:r0 + R + 1, :])
                if k == K - 1:
                    nc.vector.memset(s3[64:128, R + 1:R + 2], 0.0)
                    nc.sync.dma_start(out=s3[64:128, 0:R + 1],
                                      in_=xg[:, HH + r0 - 1:H, :])
                else:
                    nc.sync.dma_start(out=s3[64:128],
                                      in_=xg[:, HH + r0 - 1:HH + r0 + R + 1, :])
                x16 = x16_pool.tile([128, (R + 2) * WP], f16, name="x16",
                                    tag="x16")
                X = x16.rearrange("p (r w) -> p r w", w=WP)
                nc.gpsimd.memset(X[:, :, W:WP], 0.0)
                nc.scalar.copy(out=X[:, :, 0:W], in_=s3[:])
                return X

            def stage_bc(k, X):
                """DVE vertical + sliding, then final stage."""
                # ---- vertical sort3 (fp16 TT, 2x mode, pair-shared) ----
                vops = [
                    (Pm, X[:, 1:R + 1:2], X[:, 2:R + 2:2], MIN),
                    (PM, X[:, 1:R + 1:2], X[:, 2:R + 2:2], MAX),
                    (Lo3[:, 0:R:2], X[:, 0:R:2], Pm, MIN),
                    (Hi3[:, 0:R:2], X[:, 0:R:2], PM, MAX),
                    (tEv, X[:, 0:R:2], PM, MIN),
                    (Me3[:, 0:R:2], Pm, tEv, MAX),
                    (Lo3[:, 1:R:2], X[:, 3:R + 2:2], Pm, MIN),
                    (Hi3[:, 1:R:2], X[:, 3:R + 2:2], PM, MAX),
                    (tOv, X[:, 3:R + 2:2], PM, MIN),
                    (Me3[:, 1:R:2], Pm, tOv, MAX),
                ]
                for i, (o, a, b, alu) in enumerate(vops):
                    eng = (nc.gpsimd if i >= len(vops) - n_vert_gp
                           else nc.vector)
                    eng.tensor_tensor(out=o, in0=a, in1=b, op=alu)

                # ---- horizontal 3-tap merge: custom sliding DVE ops ----
                mA_t = m_pool.tile([128, LSZ], f16, name="mA", tag="mA")
                mB_t = m_pool.tile([128, LSZ], f16, name="mB", tag="mB")
                mC_t = m_pool.tile([128, LSZ], f16, name="mC", tag="mC")
                if sliding2x:
                    # window-start form at 2x: out[j] = f(T[j..j+2]);
                    # all APs even-offset fp16 step-1 -> RTL picks 2x_1p.
                    N2 = R * WP
                    for opname, src, dst in (
                        ("HMAX3W_ANT", Lo_t, mA_t),
                        ("HMED3W_ANT", Me_t, mB_t),
                        ("HMIN3W_ANT", Hi_t, mC_t),
                    ):
                        _emit_custom_dve(
                            nc.vector, ops[opname], out=dst[:, 0:N2],
                            in0=src[:, 0:N2], in1=src[:, 2:N2 + 2],
                            perf_max=1)
                    off = 0   # outputs shifted one left; final reads from 0
                else:
                    nc.vector._custom_dve(ops["HMAX3_ANT"],
                                          out=mA_t[:, 0:NS],
                                          in0=Lo_t[:, 0:NS],
                                          in1=Lo_t[:, 1:NS + 1])
                    nc.vector._custom_dve(ops["HMIN3_ANT"],
                                          out=mC_t[:, 0:NS],
                                          in0=Hi_t[:, 0:NS],
                                          in1=Hi_t[:, 1:NS + 1])
                    nc.vector._custom_dve(ops["HMED3_ANT"],
                                          out=mB_t[:, 0:NS],
                                          in0=Me_t[:, 0:NS],
                                          in1=Me_t[:, 1:NS + 1])
                    off = 2

                # ---- final med3(mA, mB, mC) ----
                mT = f_pool.tile([128, R * WP], f16, name="mT", tag="mT")
                mU = f_pool.tile([128, R * WP], f16, name="mU", tag="mU")
                mV = m_pool.tile([128, LSZ], f16, name="mV", tag="mA")
                ot = ot_pool.tile([128, R * WP + 2], f16, name="ot",
                                  tag="ot")
                Af = mA_t[:, off:off + R * WP]
                Bf = mB_t[:, off:off + R * WP]
                Cf = mC_t[:, off:off + R * WP]
                # split the 4-op chain by element range: each engine runs an
                # independent chain over its own slice (no cross-engine deps)
                g = 0.0 if (dve_tail and k == K - 1) else final_gp_frac
                S = 2 * int(R * WP * (1.0 - g) / 2)
                for eng, s0, s1 in ((nc.vector, 0, S),
                                    (nc.gpsimd, S, R * WP)):
                    if s1 <= s0:
                        continue
                    sl = slice(s0, s1)
                    osl = slice(off + s0, off + s1)
                    eng.tensor_tensor(out=mT[:, sl], in0=mA_t[:, osl],
                                      in1=mB_t[:, osl], op=MIN)
                    eng.tensor_tensor(out=mU[:, sl], in0=mA_t[:, osl],
                                      in1=mB_t[:, osl], op=MAX)
                    eng.tensor_tensor(out=mV[:, sl], in0=mU[:, sl],
                                      in1=mC_t[:, osl], op=MIN)
                    eng.tensor_tensor(out=ot[:, sl], in0=mT[:, sl],
                                      in1=mV[:, sl], op=MAX)
                return ot

            def stage_bc_dve(k, X):
                """All-DVE variant: GPSIMD tensor_tensor min/max is
                rejected by the walrus codegen, so everything runs on the
                DVE; dead Lo/Me/Hi buffers are re-used for the final
                temporaries (same engine, in-order, so aliasing is free)."""
                for o, a, b, alu in (
                    (Pm, X[:, 1:R + 1:2], X[:, 2:R + 2:2], MIN),
                    (PM, X[:, 1:R + 1:2], X[:, 2:R + 2:2], MAX),
                    (Lo3[:, 0:R:2], X[:, 0:R:2], Pm, MIN),
                    (Hi3[:, 0:R:2], X[:, 0:R:2], PM, MAX),
                    (tEv, X[:, 0:R:2], PM, MIN),
                    (Me3[:, 0:R:2], Pm, tEv, MAX),
                    (Lo3[:, 1:R:2], X[:, 3:R + 2:2], Pm, MIN),
                    (Hi3[:, 1:R:2], X[:, 3:R + 2:2], PM, MAX),
                    (tOv, X[:, 3:R + 2:2], PM, MIN),
                    (Me3[:, 1:R:2], Pm, tOv, MAX),
                ):
                    nc.vector.tensor_tensor(out=o, in0=a, in1=b, op=alu)

                mA_t = m_pool.tile([128, LSZ], f16, name="mA", tag="mA")
                mB_t = m_pool.tile([128, LSZ], f16, name="mB", tag="mB")
                mC_t = m_pool.tile([128, LSZ], f16, name="mC", tag="mC")
                N2 = R * WP
                for opname, src, dst in (
                    ("HMAX3W_ANT", Lo_t, mA_t),
                    ("HMED3W_ANT", Me_t, mB_t),
                    ("HMIN3W_ANT", Hi_t, mC_t),
                ):
                    _emit_custom_dve(
                        nc.vector, ops[opname], out=dst[:, 0:N2],
                        in0=src[:, 0:N2], in1=src[:, 2:N2 + 2], perf_max=1)

                # final med3 (all ops even-offset fp16 step-1 -> 2x mode)
                mT = f_pool.tile([128, N2], f16, name="mT", tag="mT")
                mU = f_pool.tile([128, N2], f16, name="mU", tag="mU")
                mV = f_pool.tile([128, N2], f16, name="mV", tag="mV")
                ot = ot_pool.tile([128, R * WP + 2], f16, name="ot",
                                  tag="ot")
                sl = slice(0, N2)
                nc.vector.tensor_tensor(out=mT[:], in0=mA_t[:, sl],
                                        in1=mB_t[:, sl], op=MIN)
                nc.vector.tensor_tensor(out=mU[:], in0=mA_t[:, sl],
                                        in1=mB_t[:, sl], op=MAX)
                nc.vector.tensor_tensor(out=mV[:], in0=mU[:],
                                        in1=mC_t[:, sl], op=MIN)
                nc.vector.tensor_tensor(out=ot[:, sl], in0=mT[:],
                                        in1=mV[:], op=MAX)
                return ot

            ot_off = 1 if sliding2x else 0

            def stage_d(k, ot):
                """ACT cast fp16 -> fp32 + DMA out."""
                r0 = k * R
                o32 = o32_pool.tile([128, R * W], f32, name="o32", tag="o32")
                o3 = o32.rearrange("p (r w) -> p r w", w=W)
                ot3 = ot[:, ot_off:ot_off + R * WP].rearrange(
                    "p (r w) -> p r w", w=WP)
                nc.scalar.copy(out=o3[:], in_=ot3[:, :, 0:W])
                # separate DGE queue from the input loads (sync) so stores
                # never head-of-line-block the next loads
                nc.scalar.dma_start(out=og[:, r0:r0 + R, :], in_=o3[0:64])
                nc.scalar.dma_start(out=og[:, HH + r0:HH + r0 + R, :],
                                    in_=o3[64:128])

            Xs = {0: stage_a(0)}
            ots = {}
            ddel = 1 if use_gp else 0   # store-delay (GPSIMD-final only)
            for k in range(K):
                if k + 1 < K:
                    Xs[k + 1] = stage_a(k + 1)
                if k - ddel in ots:
                    stage_d(k - ddel, ots.pop(k - ddel))
                ots[k] = (stage_bc(k, Xs.pop(k)) if use_gp
                          else stage_bc_dve(k, Xs.pop(k)))
                if ddel == 0:
                    stage_d(k, ots.pop(k))
            for kk in sorted(ots):
                stage_d(kk, ots.pop(kk))

    nc.compile()
    return nc


def _build_v4(sizes=(4, 8, 12, 16, 20, 20, 20, 16, 8, 4), in_bufs=2,
              x16_bufs=1):
    """All-DVE variant with heterogeneous chunk sizes: small first/last
    chunks shorten the pipeline ramp/tail, large middle chunks amortize
    per-instruction overhead."""
    import concourse.bacc as bacc
    import concourse.mybir as mybir
    from concourse.tile import TileContext

    ops = _register_custom_ops()
    MIN = mybir.AluOpType.min
    MAX = mybir.AluOpType.max
    f32 = mybir.dt.float32
    f16 = mybir.dt.float16

    sizes = list(sizes)
    assert sum(sizes) == HH and all(s % 2 == 0 for s in sizes)
    K = len(sizes)
    r0s = [sum(sizes[:i]) for i in range(K)]
    RM = max(sizes)
    LSZ = RM * WP + 4

    nc = bacc.Bacc("TRN2", name="median_pool2d_v4")
    x = nc.dram_tensor("x", [C, H, W], f32, kind="ExternalInput")
    out = nc.dram_tensor("out", [C, H, W], f32, kind="ExternalOutput")
    xg = x.ap()
    og = out.ap()

    with TileContext(nc) as tc:
        with (
            tc.tile_pool(name="in32", bufs=in_bufs) as in_pool,
            tc.tile_pool(name="x16p", bufs=x16_bufs) as x16_pool,
            tc.tile_pool(name="vert", bufs=1) as v_pool,
            tc.tile_pool(name="mrg", bufs=1) as m_pool,
            tc.tile_pool(name="fin", bufs=1) as f_pool,
            tc.tile_pool(name="ot16", bufs=2) as ot_pool,
            tc.tile_pool(name="out32", bufs=1) as o32_pool,
        ):
            def stile(name):
                return v_pool.tile([128, LSZ], f16, name=name, tag=name)

            Lo_t = stile("Lo")
            Me_t = stile("Me")
            Hi_t = stile("Hi")
            for t in (Lo_t, Me_t, Hi_t):
                nc.gpsimd.memset(t[:, 0:2], 0.0)
                # zero every distinct chunk-size tail once; stale row data
                # at a smaller-chunk tail only feeds pad-column junk
                for rc in sorted(set(sizes)):
                    nc.gpsimd.memset(t[:, 2 + rc * WP:2 + rc * WP + 2], 0.0)

            def vtile(name, rows):
                return v_pool.tile([128, rows * WP], f16, name=name,
                                   tag=name)

            Pm_t = vtile("Pm", RM // 2)
            PM_t = vtile("PM", RM // 2)
            tE_t = vtile("tE", RM)

            def stage_a(k):
                rc = sizes[k]
                r0 = r0s[k]
                s32 = in_pool.tile([128, (RM + 2) * W], f32, name="s32",
                                   tag="s32")
                s3 = s32[:, 0:(rc + 2) * W].rearrange("p (r w) -> p r w",
                                                      w=W)
                if k == 0:
                    nc.gpsimd.memset(s3[0:64, 0:1], 0.0)
                    nc.sync.dma_start(out=s3[0:64, 1:rc + 2],
                                      in_=xg[:, 0:rc + 1, :])
                else:
                    nc.sync.dma_start(out=s3[0:64],
                                      in_=xg[:, r0 - 1:r0 + rc + 1, :])
                if k == K - 1:
                    nc.gpsimd.memset(s3[64:128, rc + 1:rc + 2], 0.0)
                    nc.sync.dma_start(out=s3[64:128, 0:rc + 1],
                                      in_=xg[:, HH + r0 - 1:H, :])
                else:
                    nc.sync.dma_start(
                        out=s3[64:128],
                        in_=xg[:, HH + r0 - 1:HH + r0 + rc + 1, :])
                x16 = x16_pool.tile([128, (RM + 2) * WP], f16, name="x16",
                                    tag="x16")
                X = x16[:, 0:(rc + 2) * WP].rearrange("p (r w) -> p r w",
                                                      w=WP)
                nc.gpsimd.memset(X[:, :, W:WP], 0.0)
                nc.scalar.copy(out=X[:, :, 0:W], in_=s3[:])
                return x16

            def _ap(view, dims):
                """Hand-built access pattern over `view`'s tile (keeps the
                partition dim and base offset)."""
                w = view.copy()
                p0 = list(view.ap)[0]
                w.ap = mybir.VecI64Pair(
                    [[int(p0[0]), int(p0[1])]]
                    + [[int(s), int(c)] for s, c in dims])
                return w

            def stage_b(k, x16):
                rc = sizes[k]
                Rh = rc // 2
                N2 = rc * WP
                X = x16[:, 0:(rc + 2) * WP].rearrange("p (r w) -> p r w",
                                                      w=WP)
                Pm, PM = (
                    t[:, 0:Rh * WP].rearrange("p (r w) -> p r w", w=WP)
                    for t in (Pm_t, PM_t))
                nc.vector.tensor_tensor(out=Pm, in0=X[:, 1:rc + 1:2],
                                        in1=X[:, 2:rc + 2:2], op=MIN)
                nc.vector.tensor_tensor(out=PM, in0=X[:, 1:rc + 1:2],
                                        in1=X[:, 2:rc + 2:2], op=MAX)
                # merged even/odd completions: one op per output array.
                # iteration order (pair i, parity g, col w); out rows are
                # consecutive; the X operand taps rows (2i, 2i+3); Pm/PM
                # broadcast over the parity dim (0-stride).
                grp = [[2 * WP, Rh], [3 * WP, 2], [1, WP]]
                bcast = [[WP, Rh], [0, 2], [1, WP]]
                rows = [[2 * WP, Rh], [WP, 2], [1, WP]]
                a_g = _ap(x16[:, 0:(rc + 2) * WP], grp)
                lo_o = _ap(Lo_t[:, 2:2 + N2], rows)
                hi_o = _ap(Hi_t[:, 2:2 + N2], rows)
                me_o = _ap(Me_t[:, 2:2 + N2], rows)
                pm_b = _ap(Pm_t[:, 0:Rh * WP], bcast)
                pM_b = _ap(PM_t[:, 0:Rh * WP], bcast)
                tEO = _ap(tE_t[:, 0:N2], rows)
                nc.vector.tensor_tensor(out=lo_o, in0=a_g, in1=pm_b, op=MIN)
                nc.vector.tensor_tensor(out=hi_o, in0=a_g, in1=pM_b, op=MAX)
                nc.vector.tensor_tensor(out=tEO, in0=a_g, in1=pM_b, op=MIN)
                nc.vector.tensor_tensor(out=me_o, in0=pm_b, in1=tEO,
                                        op=MAX)

                mA_t = m_pool.tile([128, LSZ], f16, name="mA", tag="mA")
                mB_t = m_pool.tile([128, LSZ], f16, name="mB", tag="mB")
                mC_t = m_pool.tile([128, LSZ], f16, name="mC", tag="mC")
                for opname, src, dst in (
                    ("HMAX3W_ANT", Lo_t, mA_t),
                    ("HMED3W_ANT", Me_t, mB_t),
                    ("HMIN3W_ANT", Hi_t, mC_t),
                ):
                    _emit_custom_dve(
                        nc.vector, ops[opname], out=dst[:, 0:N2],
                        in0=src[:, 0:N2], in1=src[:, 2:N2 + 2], perf_max=1)

                mT = f_pool.tile([128, RM * WP], f16, name="mT", tag="mT")
                mU = f_pool.tile([128, RM * WP], f16, name="mU", tag="mU")
                # mB is dead once mT/mU are done; reuse its buffer (all
                # ops on the DVE, in-order, so aliasing is dependency-free)
                mV = m_pool.tile([128, LSZ], f16, name="mV", tag="mB")
                ot = ot_pool.tile([128, RM * WP + 2], f16, name="ot",
                                  tag="ot")
                sl = slice(0, N2)
                nc.vector.tensor_tensor(out=mT[:, sl], in0=mA_t[:, sl],
                                        in1=mB_t[:, sl], op=MIN)
                nc.vector.tensor_tensor(out=mU[:, sl], in0=mA_t[:, sl],
                                        in1=mB_t[:, sl], op=MAX)
                nc.vector.tensor_tensor(out=mV[:, sl], in0=mU[:, sl],
                                        in1=mC_t[:, sl], op=MIN)
                nc.vector.tensor_tensor(out=ot[:, sl], in0=mT[:, sl],
                                        in1=mV[:, sl], op=MAX)
                return ot

            def stage_d(k, ot):
                rc = sizes[k]
                r0 = r0s[k]
                o32 = o32_pool.tile([128, RM * W], f32, name="o32",
                                    tag="o32")
                o3 = o32[:, 0:rc * W].rearrange("p (r w) -> p r w", w=W)
                ot3 = ot[:, 1:1 + rc * WP].rearrange("p (r w) -> p r w",
                                                     w=WP)
                nc.scalar.copy(out=o3[:], in_=ot3[:, :, 0:W])
                nc.scalar.dma_start(out=og[:, r0:r0 + rc, :], in_=o3[0:64])
                nc.scalar.dma_start(out=og[:, HH + r0:HH + r0 + rc, :],
                                    in_=o3[64:128])

            Xs = {0: stage_a(0)}
            for k in range(K):
                if k + 1 < K:
                    Xs[k + 1] = stage_a(k + 1)
                ot = stage_b(k, Xs.pop(k))
                stage_d(k, ot)

    nc.compile()
    return nc


def _build_shared(R=16, gp_frac=0.0, dtype="float32", in_bufs=None,
                  out_bufs=None):
    """Fallback: 15-op/pixel fp32 TT network (previous working kernel)."""
    import concourse.bacc as bacc
    import concourse.mybir as mybir
    from concourse.tile import TileContext

    MIN = mybir.AluOpType.min
    MAX = mybir.AluOpType.max
    f32 = mybir.dt.float32
    cdt = getattr(mybir.dt, dtype)

    WPP = W + 2
    assert HH % R == 0 and R % 2 == 0
    K = HH // R
    Rh = R // 2

    nc = bacc.Bacc("TRN2", name="median_pool2d_s")
    x = nc.dram_tensor("x", [C, H, W], f32, kind="ExternalInput")
    out = nc.dram_tensor("out", [C, H, W], f32, kind="ExternalOutput")
    xg = x.ap()
    og = out.ap()

    def tt(out_ap, in0, in1, op):
        nc.vector.tensor_tensor(out=out_ap, in0=in0, in1=in1, op=op)

    if in_bufs is None:
        in_bufs = 3 if R <= 8 else 2
    if out_bufs is None:
        out_bufs = 3 if R <= 8 else 1
    with TileContext(nc) as tc:
        with (
            tc.tile_pool(name="io_in", bufs=in_bufs) as in_pool,
            tc.tile_pool(name="io_out", bufs=out_bufs) as out_pool,
            tc.tile_pool(name="work", bufs=1) as w_pool,
        ):
            def wtile(name, rows, width, tag=None):
                t = w_pool.tile([128, rows * width], cdt, name=name,
                                tag=tag or name)
                return t.rearrange("p (r w) -> p r w", w=width)

            for k in range(K):
                r0 = k * R
                it = in_pool.tile([128, (R + 2) * WPP], cdt, name="it",
                                  tag="it")
                it3 = it.rearrange("p (r w) -> p r w", w=WPP)
                nc.vector.memset(it3[:, :, 0:WPP:WPP - 1], 0.0)
                if k == 0:
                    nc.vector.memset(it3[0:64, 0:1, 1:W + 1], 0.0)
                    nc.sync.dma_start(out=it3[0:64, 1:R + 2, 1:W + 1],
                                      in_=xg[:, 0:R + 1, :])
                else:
                    nc.sync.dma_start(out=it3[0:64, :, 1:W + 1],
                                      in_=xg[:, r0 - 1:r0 + R + 1, :])
                if k == K - 1:
                    nc.vector.memset(it3[64:128, R + 1:R + 2, 1:W + 1], 0.0)
                    nc.sync.dma_start(out=it3[64:128, 0:R + 1, 1:W + 1],
                                      in_=xg[:, HH + r0 - 1:H, :])
                else:
                    nc.sync.dma_start(out=it3[64:128, :, 1:W + 1],
                                      in_=xg[:, HH + r0 - 1:HH + r0 + R + 1, :])

                Pm = wtile("Pm", Rh, WPP)
                PM = wtile("PM", Rh, WPP)
                tt(Pm, it3[:, 1:R + 1:2, :], it3[:, 2:R + 2:2, :], MIN)
                tt(PM, it3[:, 1:R + 1:2, :], it3[:, 2:R + 2:2, :], MAX)

                Lo3 = wtile("Lo", R, WPP)
                Me3 = wtile("Me", R, WPP)
                Hi3 = wtile("Hi", R, WPP)
                tE = wtile("tE", Rh, WPP)
                tO = wtile("tO", Rh, WPP)
                a_e = it3[:, 0:R:2, :]
                a_o = it3[:, 3:R + 2:2, :]
                tt(Lo3[:, 0:R:2], a_e, Pm, MIN)
                tt(Hi3[:, 0:R:2], a_e, PM, MAX)
                tt(tE, a_e, PM, MIN)
                tt(Me3[:, 0:R:2], Pm, tE, MAX)
                tt(Lo3[:, 1:R:2], a_o, Pm, MIN)
                tt(Hi3[:, 1:R:2], a_o, PM, MAX)
                tt(tO, a_o, PM, MIN)
                tt(Me3[:, 1:R:2], Pm, tO, MAX)

                NP = W // 2 + 1
                PA = wtile("PA", R, NP, tag="Pm")
                PC = wtile("PC", R, NP, tag="PM")
                Um = wtile("Um", R, NP, tag="tE")
                Vm = wtile("Vm", R, NP, tag="tO")
                tBe = wtile("tBe", R, W // 2, tag="Pm")
                tBo = wtile("tBo", R, W // 2, tag="PM")
                mA = wtile("mA", R, W)
                mB = wtile("mB", R, W)
                mC = wtile("mC", R, W)

                ev = slice(0, WPP, 2)
                od = slice(1, WPP, 2)
                tt(PA, Lo3[:, :, ev], Lo3[:, :, od], MAX)
                tt(mA[:, :, 0:W:2], PA[:, :, 0:NP - 1], Lo3[:, :, 2:WPP:2],
                   MAX)
                tt(mA[:, :, 1:W:2], PA[:, :, 1:NP], Lo3[:, :, 1:WPP - 2:2],
                   MAX)

                tt(PC, Hi3[:, :, ev], Hi3[:, :, od], MIN)
                tt(mC[:, :, 0:W:2], PC[:, :, 0:NP - 1], Hi3[:, :, 2:WPP:2],
                   MIN)
                tt(mC[:, :, 1:W:2], PC[:, :, 1:NP], Hi3[:, :, 1:WPP - 2:2],
                   MIN)

                tt(Um, Me3[:, :, ev], Me3[:, :, od], MIN)
                tt(Vm, Me3[:, :, ev], Me3[:, :, od], MAX)
                tt(tBe, Me3[:, :, 2:WPP:2], Vm[:, :, 0:NP - 1], MIN)
                tt(mB[:, :, 0:W:2], Um[:, :, 0:NP - 1], tBe, MAX)
                tt(tBo, Me3[:, :, 1:WPP - 2:2], Vm[:, :, 1:NP], MIN)
                tt(mB[:, :, 1:W:2], Um[:, :, 1:NP], tBo, MAX)

                mT = wtile("mT", R, W, tag="Lo")
                mU = wtile("mU", R, W, tag="Me")
                mV = wtile("mV", R, W, tag="Hi")
                ot = out_pool.tile([128, R * W], cdt, name="ot", tag="ot")
                ot3 = ot.rearrange("p (r w) -> p r w", w=W)
                tt(mT, mA, mB, MIN)
                tt(mU, mA, mB, MAX)
                tt(mV, mU, mC, MIN)
                tt(ot3, mT, mV, MAX)

                nc.sync.dma_start(out=og[:, r0:r0 + R, :], in_=ot3[0:64])
                nc.sync.dma_start(out=og[:, HH + r0:HH + r0 + R, :],
                                  in_=ot3[64:128])

    nc.compile()
    return nc


def _get_nc(variant="v3", **kw):
    key = (variant, tuple(sorted(kw.items())))
    if key not in _CACHE:
        if variant == "v4":
            _CACHE[key] = _build_v4(**kw)
        elif variant == "v3":
            _CACHE[key] = _build_v3(**kw)
        else:
            _CACHE[key] = _build_shared(**kw)
    return _CACHE[key]


_LAST_NC = None


def kernel(x: np.ndarray) -> np.ndarray:
    """MedianPool2d(3x3, s=1, p=1) on 8 NeuronCores."""
    global _LAST_NC
    from concourse.bass_utils import run_bass_kernel_spmd

    assert x.shape == (B, C, H, W), x.shape
    x = np.ascontiguousarray(x, dtype=np.float32)
    try:
        nc = _get_nc("v4")
    except Exception:
        try:
            nc = _get_nc("v3")
        except Exception:
            nc = _get_nc("shared", R=16)
    _LAST_NC = nc
    in_maps = [{"x": x[i]} for i in range(NCORES)]
    res = run_bass_kernel_spmd(nc, in_maps, core_ids=list(range(NCORES)))
    return np.stack([r["out"] for r in res.results], axis=0)


# revision 45
# speedup vs baseline: 2.3972x; 1.0074x over previous
"""MedianPool2d (3x3, stride 1, zero-pad 1) Trainium2 Bass kernel.

Full input x: (8, 64, 256, 256) fp32.  Sharding: pure data parallel over
batch -> core i processes x[i] (64, 256, 256).

Design (v4: fp16 + custom packed-2x sliding DVE ops):
  - Per-core layout: 128 SBUF partitions = (h, c), h in {0,1} = top/bottom
    128-row half, c = channel.  Heterogeneous chunks of output rows per
    partition (small first/last chunks shorten pipeline ramp/tail).
  - DMA loads fp32 rows into a staging tile; the Activation engine casts
    fp32 -> fp16 into a padded row layout (WP=258: 256 cols + 2 zero pad
    cols).  fp16 is exact for the median network itself (pure min/max
    selection); only the input cast rounds (~2.1e-4 l2 rel, gate 2e-2).
  - Vertical sort3 (Lo/Me/Hi per output row) as fp16 tensor_tensor
    min/max on the DVE with row-pair sharing: all ops full-row step-1
    4B-aligned -> genuine 2x DVE mode (0.52 ns/elem).  The even/odd
    completions are merged into single instructions via hand-built
    access patterns (a (pair, parity, col) iteration with a 3*WP-stride
    group dim on the X operand and a 0-stride parity broadcast on the
    pair operand) - 6 vertical instructions per chunk instead of 10.
  - Horizontal 3-tap merge via three hand-authored custom DVE micro-op
    programs (HMAX3W/HMED3W/HMIN3W, registered into the per-NEFF DVE
    table at runtime): ONE streaming pass each computes
    out[j] = f(T[j], T[j+1], T[j+2]) over the flat row stream.  The APs
    (src0 = T[0:N], src1 = T[2:N+2], both fp16 step-1 4B-aligned) make
    the RTL select 2x_1p packed mode, where the crossbar exposes four
    consecutive taps per cycle (SRC_0, SRC_0_HI, SRC_1, SRC_1_HI) - both
    parities' windows are pure spatial, even result -> WR0_LO, odd ->
    WR0_HI, 2 elem/cycle.  Each row carries 2 trailing zero pad cols so
    the previous row's tail doubles as the next row's left pad; a
    2-element zero prefix seeds row 0.  HW-validated bit-exact.
  - Final med3(mA, mB, mC) as 4 more fp16 2x TTs on the DVE.
    (GPSIMD tensor_tensor min/max and DMA-CCE min/max accumulate are both
    rejected by the walrus codegen, so GPSIMD only does small memsets.)
  - Output cast fp16 -> fp32 on the Activation engine, then DMA out on a
    separate DGE queue from the loads.

Effective cost ~12 TT-op-equivalents/pixel at 2x (vs 15 at 1x fp32 for
the baseline) -> 228.1us/core vs 546.7us baseline (2.40x).
"""

import numpy as np

B, C, H, W = 8, 64, 256, 256
NCORES = 8
HH = H // 2          # rows per half-strip
WP = W + 2           # padded row width (2 trailing zero cols)

_CACHE = {}
_OPS = None


def _register_custom_ops():
    """Register the three sliding-window custom DVE ops (idempotent).

    Each op streams src0 = T[j], src1 = T[j+1] and computes a 3-tap
    window f(T[j-1], T[j], T[j+1]) per output element:
      - T[j], T[j+1] arrive spatially on the two source streams;
      - T[j-1] = src1 two elements back, read via NEXT_ALU_OUT_A from a
        pipeline stage that latches raw src1 into its a-flop each cycle.
    Outputs j=0,1 use pre-instruction flop state (garbage); callers must
    treat the first two output elements as scrap.
    """
    global _OPS
    if _OPS is not None:
        return _OPS

    import concourse.dve_ops as dve_ops
    from concourse.dve_spec import Spec, Src0, Src1, minn
    from concourse.dve_uop import (
        ENABLE,
        AluInp,
        AluOp,
        DelayInp,
        DveOpSpec,
        InpSel,
        OutPath,
        OutSel,
        Trigger,
        UopConfig,
    )

    def _shift2(in1):
        z = np.zeros_like(in1[..., :2])
        return np.concatenate([z, in1[..., :-2]], axis=-1)

    def _hmax3_ref(in0, in1, c0, c1, c2):
        return np.maximum(np.maximum(in0, in1), _shift2(in1))

    def _hmin3_ref(in0, in1, c0, c1, c2):
        return np.minimum(np.minimum(in0, in1), _shift2(in1))

    def _hmed3_ref(in0, in1, c0, c1, c2):
        c = _shift2(in1)
        return np.maximum(np.minimum(in0, in1),
                          np.minimum(np.maximum(in0, in1), c))

    def _mm3_uops(op3):
        """out[j] = op3(src0[j], src1[j], src1[j-2]) single-uop program."""
        u = UopConfig()
        u.enable_input(InpSel.SRC_0, 1)     # -> PREV_DELAY_0 at blk0
        u.enable_input(InpSel.SRC_1, 2)     # -> PREV_DELAY_1 at blk0
        u.require_inp0 = ENABLE
        u.require_inp1 = ENABLE
        u.trigger = (Trigger.SRC_TENSOR_DONE, Trigger.NONE, Trigger.NONE)
        dp = u.datapath_config
        # blk0: m = op3(src0, src1)
        dp[0].enable_alu(op3, AluInp.PREV_DELAY_0, AluInp.PREV_DELAY_1)
        dp[0].pass_through_delay(1)
        # blk1: out = op3(m, src1[j-2])  (blk2's a-flop, prev cycle)
        dp[1].enable_alu(op3, AluInp.PREV_ALU_OUT, AluInp.NEXT_ALU_OUT_A)
        dp[1].pass_through_delay(1)
        # blk2: latch raw src1 into the a-flop; stash out in delay chain 0
        dp[2].enable_alu(AluOp.BYPASS, AluInp.PREV_DELAY_1)
        dp[2].alu_out_a_enable = ENABLE
        dp[2].enable_delay_from_src(DelayInp.PREV_ALU_OUT, 0)
        for kblk in range(3, 8):
            dp[kblk].pass_through_alu().pass_through_delay(0)
        u.enable_output(OutSel.DELAY_0, OutPath.WR0_LO)
        return [u]

    def _med3_uops():
        """out[j] = med3(src0[j], src1[j], src1[j-2]) single-uop program."""
        u = UopConfig()
        u.enable_input(InpSel.SRC_0, 1)     # chain 0
        u.enable_input(InpSel.SRC_1, 2)     # chain 1
        u.require_inp0 = ENABLE
        u.require_inp1 = ENABLE
        u.trigger = (Trigger.SRC_TENSOR_DONE, Trigger.NONE, Trigger.NONE)
        dp = u.datapath_config
        # blk0: p = min(a, b)
        dp[0].enable_alu(AluOp.MIN, AluInp.PREV_DELAY_0, AluInp.PREV_DELAY_1)
        dp[0].pass_through_delay(0, 1)
        # blk1: q = max(a, b); stash p in chain 2
        dp[1].enable_alu(AluOp.MAX, AluInp.PREV_DELAY_0, AluInp.PREV_DELAY_1)
        dp[1].enable_delay_from_src(DelayInp.PREV_ALU_OUT, 2)
        dp[1].pass_through_delay(1)
        # blk2: t = min(q, c) with c = src1[j-2] via blk3's a-flop
        dp[2].enable_alu(AluOp.MIN, AluInp.PREV_ALU_OUT, AluInp.NEXT_ALU_OUT_A)
        dp[2].pass_through_delay(1, 2)
        # blk3: latch raw src1 into a-flop; stash t in chain 3
        dp[3].enable_alu(AluOp.BYPASS, AluInp.PREV_DELAY_1)
        dp[3].alu_out_a_enable = ENABLE
        dp[3].enable_delay_from_src(DelayInp.PREV_ALU_OUT, 3)
        dp[3].pass_through_delay(2)
        # blk4: out = max(p, t)
        dp[4].enable_alu(AluOp.MAX, AluInp.PREV_DELAY_2, AluInp.PREV_DELAY_3)
        for kblk in range(5, 8):
            dp[kblk].pass_through_alu()
        u.enable_output(OutSel.ALU_OUT, OutPath.WR0_LO)
        return [u]

    # ---- 2x (packed fp16) window-start variants -------------------------
    # Semantics: out[j] = f(T[j], T[j+1], T[j+2]) with src0 = T[0:N],
    # src1 = T[2:N+2], all APs fp16 step-1 4B-aligned so the RTL always
    # selects 2x_1p.  Per pair-cycle the crossbar exposes 4 consecutive
    # taps (SRC_0, SRC_0_HI, SRC_1, SRC_1_HI); both parities' windows are
    # pure spatial.  Even result -> WR0_LO, odd -> WR0_HI.

    def _shiftcat(in0, in1):
        # full stream T (len N+2) from the two views
        return np.concatenate([in0, in1[..., -2:]], axis=-1)

    def _hmax3w_ref(in0, in1, c0, c1, c2):
        T = _shiftcat(in0, in1)
        return np.maximum(np.maximum(T[..., :-2], T[..., 1:-1]), T[..., 2:])

    def _hmin3w_ref(in0, in1, c0, c1, c2):
        T = _shiftcat(in0, in1)
        return np.minimum(np.minimum(T[..., :-2], T[..., 1:-1]), T[..., 2:])

    def _hmed3w_ref(in0, in1, c0, c1, c2):
        T = _shiftcat(in0, in1)
        a, b, c = T[..., :-2], T[..., 1:-1], T[..., 2:]
        return np.maximum(np.minimum(a, b),
                          np.minimum(np.maximum(a, b), c))

    def _dummy_1x(op3):
        """Placeholder REGULAR-mode program (never selected: the APs
        always satisfy the 2x_1p conditions)."""
        u = UopConfig()
        u.enable_input(InpSel.SRC_0, 1)
        u.enable_input(InpSel.SRC_1, 2)
        u.require_inp0 = ENABLE
        u.require_inp1 = ENABLE
        u.trigger = (Trigger.SRC_TENSOR_DONE, Trigger.NONE, Trigger.NONE)
        dp = u.datapath_config
        dp[0].enable_alu(op3, AluInp.PREV_DELAY_0, AluInp.PREV_DELAY_1)
        for kblk in range(1, 8):
            dp[kblk].pass_through_alu()
        u.enable_output(OutSel.ALU_OUT, OutPath.WR0_LO)
        return [u]

    def _mm3_w2x_uops(op3):
        """2x program: E = op3(S0, S0H, S1) -> WR0_LO, O = op3(S0H, S1,
        S1H) -> WR0_HI."""
        u = UopConfig()
        u.enable_input(InpSel.SRC_0, 1)      # chain 0
        u.enable_input(InpSel.SRC_0_HI, 2)   # chain 1
        u.enable_input(InpSel.SRC_1, 3)      # chain 2
        u.enable_input(InpSel.SRC_1_HI, 4)   # chain 3
        u.require_inp0 = ENABLE
        u.require_inp1 = ENABLE
        u.trigger = (Trigger.SRC_TENSOR_DONE, Trigger.NONE, Trigger.NONE)
        dp = u.datapath_config
        dp[0].enable_alu(op3, AluInp.PREV_DELAY_0, AluInp.PREV_DELAY_1)
        dp[0].pass_through_delay(1, 2, 3)
        dp[1].enable_alu(op3, AluInp.PREV_ALU_OUT, AluInp.PREV_DELAY_2)
        dp[1].pass_through_delay(1, 2, 3)
        dp[2].enable_alu(op3, AluInp.PREV_DELAY_1, AluInp.PREV_DELAY_2)
        dp[2].enable_delay_from_src(DelayInp.PREV_ALU_OUT, 4)   # E
        dp[2].pass_through_delay(3)
        dp[3].enable_alu(op3, AluInp.PREV_ALU_OUT, AluInp.PREV_DELAY_3)
        dp[3].pass_through_delay(4)
        for kblk in range(4, 8):
            dp[kblk].pass_through_alu().pass_through_delay(4)
        u.enable_output(OutSel.DELAY_4, OutPath.WR0_LO)
        u.enable_output(OutSel.ALU_OUT, OutPath.WR0_HI)
        return [u]

    def _med3_w2x_uops():
        """2x program: E = med3(S0, S0H, S1) -> WR0_LO, O = med3(S0H, S1,
        S1H) -> WR0_HI.  Exactly 8 ALU blocks."""
        u = UopConfig()
        u.enable_input(InpSel.SRC_0, 1)      # chain 0
        u.enable_input(InpSel.SRC_0_HI, 2)   # chain 1
        u.enable_input(InpSel.SRC_1, 3)      # chain 2
        u.enable_input(InpSel.SRC_1_HI, 4)   # chain 3
        u.require_inp0 = ENABLE
        u.require_inp1 = ENABLE
        u.trigger = (Trigger.SRC_TENSOR_DONE, Trigger.NONE, Trigger.NONE)
        dp = u.datapath_config
        # E-half: med3(d0, d1, d2)
        dp[0].enable_alu(AluOp.MIN, AluInp.PREV_DELAY_0, AluInp.PREV_DELAY_1)
        dp[0].pass_through_delay(0, 1, 2, 3)
        dp[1].enable_alu(AluOp.MAX, AluInp.PREV_DELAY_0, AluInp.PREV_DELAY_1)
        dp[1].enable_delay_from_src(DelayInp.PREV_ALU_OUT, 4)   # pE
        dp[1].pass_through_delay(1, 2, 3)
        dp[2].enable_alu(AluOp.MIN, AluInp.PREV_ALU_OUT, AluInp.PREV_DELAY_2)
        dp[2].pass_through_delay(1, 2, 3, 4)
        dp[3].enable_alu(AluOp.MAX, AluInp.PREV_DELAY_4, AluInp.PREV_ALU_OUT)
        dp[3].pass_through_delay(1, 2, 3)
        # O-half: med3(d1, d2, d3)
        dp[4].enable_alu(AluOp.MIN, AluInp.PREV_DELAY_1, AluInp.PREV_DELAY_2)
        dp[4].enable_delay_from_src(DelayInp.PREV_ALU_OUT, 4)   # E
        dp[4].pass_through_delay(1, 2, 3)
        dp[5].enable_alu(AluOp.MAX, AluInp.PREV_DELAY_1, AluInp.PREV_DELAY_2)
        dp[5].enable_delay_from_src(DelayInp.PREV_ALU_OUT, 5)   # pO
        dp[5].pass_through_delay(3, 4)
        dp[6].enable_alu(AluOp.MIN, AluInp.PREV_ALU_OUT, AluInp.PREV_DELAY_3)
        dp[6].pass_through_delay(4, 5)
        dp[7].enable_alu(AluOp.MAX, AluInp.PREV_DELAY_5, AluInp.PREV_ALU_OUT)
        dp[7].pass_through_delay(4)
        u.enable_output(OutSel.DELAY_4, OutPath.WR0_LO)
        u.enable_output(OutSel.ALU_OUT, OutPath.WR0_HI)
        return [u]

    class _SlidingOp:
        """Duck-typed DveOp: hand-authored uops instead of lower(spec)."""

        def __init__(self, name, reference, uops_builder, uops_2x_builder=None):
            # body is a placeholder that reads Src0+Src1 (rd1_en) and no
            # C2/C3; only `reference` is ever evaluated (bass_interp).
            self.name = name
            self.spec = Spec(body=minn(Src0, Src1), reference=reference)
            self.subdim = False
            self.perf_en = {}
            self._builder = uops_builder
            self._builder_2x = uops_2x_builder
            self._cache = {}

        def compile(self, ver):
            if ver not in self._cache:
                s = DveOpSpec(
                    name=self.name,
                    opcode=dve_ops.get_dve_sub_opcode(self.name),
                    uops=self._builder(),
                    uops_2x=(self._builder_2x() if self._builder_2x
                             else None),
                    perf_max=1 if self._builder_2x else 0,
                    rd1_en=True,
                )
                s.validate(ver)
                self._cache[ver] = s
            return self._cache[ver]

    defs = [
        ("HMAX3_ANT", _hmax3_ref, lambda: _mm3_uops(AluOp.MAX), None),
        ("HMIN3_ANT", _hmin3_ref, lambda: _mm3_uops(AluOp.MIN), None),
        ("HMED3_ANT", _hmed3_ref, _med3_uops, None),
        ("HMAX3W_ANT", _hmax3w_ref, lambda: _dummy_1x(AluOp.MAX),
         lambda: _mm3_w2x_uops(AluOp.MAX)),
        ("HMIN3W_ANT", _hmin3w_ref, lambda: _dummy_1x(AluOp.MIN),
         lambda: _mm3_w2x_uops(AluOp.MIN)),
        ("HMED3W_ANT", _hmed3w_ref, lambda: _dummy_1x(AluOp.MIN),
         _med3_w2x_uops),
    ]
    ops = {}
    for name, ref, builder, builder2x in defs:
        if name not in dve_ops._SUB_OPCODE_FOR_NAME:
            row = max(dve_ops._SUB_OPCODE_FOR_NAME.values()) + 1
            assert row < 0x20, "custom DVE opcode rows exhausted"
            dve_ops._SUB_OPCODE_FOR_NAME[name] = row
        op = _SlidingOp(name, ref, builder, builder2x)
        # replace any previous registration (idempotent across reloads)
        dve_ops.OPS[:] = [o for o in dve_ops.OPS if o.name != name] + [op]
        dve_ops.CUSTOM_DVE_SPECS[name] = op.spec
        ops[name] = op
    _OPS = ops
    return ops


def _emit_custom_dve(v, op, out, in0, in1, perf_max=0):
    """Like bass.Vector._custom_dve but with perf_max set at construction
    (the Tile scheduler drops post-hoc attribute edits)."""
    import concourse.bass_isa as bass_isa
    import concourse.mybir as mybir
    from concourse.dve_ops import get_dve_sub_opcode

    b = v.bass
    if op.name not in b.m.ant_custom_dve_ops:
        b.m.ant_custom_dve_ops = sorted({*b.m.ant_custom_dve_ops, op.name})
    shape = bass_isa.CustomDveShape.TTSS
    isa_opcode = b.isa.Opcode[
        f"NEURON_ISA_TPB_OPCODE_CUSTOM_DVE_ANT_{shape.slot()}"
    ].value
    imm = lambda: mybir.ImmediateValue(dtype=mybir.dt.float32, value=0.0)
    return v.add_instruction(
        bass_isa.InstCustomDveAnt(
            name=b.get_next_instruction_name(),
            op_name=op.name,
            rd1_en=True,
            subdim=0,
            imm2=0.0,
            shape=shape,
            row=get_dve_sub_opcode(op.name),
            perf_max=perf_max,
            isa_opcode=isa_opcode,
            ins=[v.lower_ap(in0, for_isa=True),
                 v.lower_ap(in1, for_isa=True), imm(), imm()],
            outs=[v.lower_ap(out, for_isa=True)],
        )
    )


def _build_v3(R=8, final_gp_frac=0.65, n_vert_gp=1, in_bufs=2, mrg_bufs=1,
              x16_bufs=2, dve_tail=True, sliding2x=True, use_gp=False):
    """fp16 + custom sliding ops + ACT casts + GPSIMD final stage.

    Software-pipelined emission (engines execute their streams in order):
    per iteration k we emit  A(k+1) = load+cast of the NEXT chunk,
    D(k-1) = store of the PREVIOUS chunk, then B(k) = DVE work and
    C(k) = final stage.  This keeps the ACT in-cast ahead of the
    GPSIMD-dependent out-cast in the ACT/sync instruction streams.

    n_final_gp: how many of the 4 final med3 tensor_tensor ops run on
    GPSIMD (rest on DVE).  n_vert_gp: how many vertical ops on GPSIMD.
    dve_tail: run the last chunk's final stage on the DVE (shorter tail).
    """
    import concourse.bacc as bacc
    import concourse.mybir as mybir
    from concourse.tile import TileContext

    ops = _register_custom_ops()
    MIN = mybir.AluOpType.min
    MAX = mybir.AluOpType.max
    f32 = mybir.dt.float32
    f16 = mybir.dt.float16

    assert HH % R == 0 and R % 2 == 0
    K = HH // R
    Rh = R // 2
    NS = 2 + R * WP          # sliding stream length (2 prefix + rows)
    LSZ = R * WP + 4         # sliding tile: prefix 2 + rows + tail 2

    nc = bacc.Bacc("TRN2", name="median_pool2d_v3")
    x = nc.dram_tensor("x", [C, H, W], f32, kind="ExternalInput")
    out = nc.dram_tensor("out", [C, H, W], f32, kind="ExternalOutput")
    xg = x.ap()
    og = out.ap()

    with TileContext(nc) as tc:
        with (
            tc.tile_pool(name="in32", bufs=in_bufs) as in_pool,
            tc.tile_pool(name="x16p", bufs=x16_bufs) as x16_pool,
            tc.tile_pool(name="vert", bufs=1) as v_pool,
            tc.tile_pool(name="mrg", bufs=mrg_bufs) as m_pool,
            tc.tile_pool(name="fin", bufs=1) as f_pool,
            tc.tile_pool(name="ot16", bufs=2) as ot_pool,
            tc.tile_pool(name="out32", bufs=1) as o32_pool,
        ):
            # persistent sliding tiles: prefix/tail zeros written once
            def stile(name):
                t = v_pool.tile([128, LSZ], f16, name=name, tag=name)
                rows = t[:, 2:2 + R * WP].rearrange("p (r w) -> p r w", w=WP)
                return t, rows

            Lo_t, Lo3 = stile("Lo")
            Me_t, Me3 = stile("Me")
            Hi_t, Hi3 = stile("Hi")
            for t in (Lo_t, Me_t, Hi_t):
                nc.gpsimd.memset(t[:, 0:2], 0.0)
                nc.gpsimd.memset(t[:, 2 + R * WP:LSZ], 0.0)

            def vtile(name, rows):
                t = v_pool.tile([128, rows * WP], f16, name=name, tag=name)
                return t.rearrange("p (r w) -> p r w", w=WP)

            Pm = vtile("Pm", Rh)
            PM = vtile("PM", Rh)
            tEv = vtile("tE", Rh)
            tOv = vtile("tO", Rh)

            def stage_a(k):
                """DMA fp32 rows + ACT cast to padded fp16 rows."""
                r0 = k * R
                s32 = in_pool.tile([128, (R + 2) * W], f32, name="s32",
                                   tag="s32")
                s3 = s32.rearrange("p (r w) -> p r w", w=W)
                if k == 0:
                    nc.vector.memset(s3[0:64, 0:1], 0.0)
                    nc.sync.dma_start(out=s3[0:64, 1:R + 2],
                                      in_=xg[:, 0:R + 1, :])
                else:
                    nc.sync.dma_start(out=s3[0:64],
                                      in_=xg[:, r0 - 1:r0 + R + 1, :])
                if k == K - 1:
                    nc.vector.memset(s3[64:128, R + 1:R + 2], 0.0)
                    nc.sync.dma_start(out=s3[64:128, 0:R + 1],
                                      in_=xg[:, HH + r0 - 1:H, :])
                else:
                    nc.sync.dma_start(out=s3[64:128],
                                      in_=xg[:, HH + r0 - 1:HH + r0 + R + 1, :])
                x16 = x16_pool.tile([128, (R + 2) * WP], f16, name="x16",
                                    tag="x16")
                X = x16.rearrange("p (r w) -> p r w", w=WP)
                nc.gpsimd.memset(X[:, :, W:WP], 0.0)
                nc.scalar.copy(out=X[:, :, 0:W], in_=s3[:])
                return X

            def stage_bc(k, X):
                """DVE vertical + sliding, then final stage."""
                # ---- vertical sort3 (fp16 TT, 2x mode, pair-shared) ----
                vops = [
                    (Pm, X[:, 1:R + 1:2], X[:, 2:R + 2:2], MIN),
                    (PM, X[:, 1:R + 1:2], X[:, 2:R + 2:2], MAX),
                    (Lo3[:, 0:R:2], X[:, 0:R:2], Pm, MIN),
                    (Hi3[:, 0:R:2], X[:, 0:R:2], PM, MAX),
                    (tEv, X[:, 0:R:2], PM, MIN),
                    (Me3[:, 0:R:2], Pm, tEv, MAX),
                    (Lo3[:, 1:R:2], X[:, 3:R + 2:2], Pm, MIN),
                    (Hi3[:, 1:R:2], X[:, 3:R + 2:2], PM, MAX),
                    (tOv, X[:, 3:R + 2:2], PM, MIN),
                    (Me3[:, 1:R:2], Pm, tOv, MAX),
                ]
                for i, (o, a, b, alu) in enumerate(vops):
                    eng = (nc.gpsimd if i >= len(vops) - n_vert_gp
                           else nc.vector)
                    eng.tensor_tensor(out=o, in0=a, in1=b, op=alu)

                # ---- horizontal 3-tap merge: custom sliding DVE ops ----
                mA_t = m_pool.tile([128, LSZ], f16, name="mA", tag="mA")
                mB_t = m_pool.tile([128, LSZ], f16, name="mB", tag="mB")
                mC_t = m_pool.tile([128, LSZ], f16, name="mC", tag="mC")
                if sliding2x:
                    # window-start form at 2x: out[j] = f(T[j..j+2]);
                    # all APs even-offset fp16 step-1 -> RTL picks 2x_1p.
                    N2 = R * WP
                    for opname, src, dst in (
                        ("HMAX3W_ANT", Lo_t, mA_t),
                        ("HMED3W_ANT", Me_t, mB_t),
                        ("HMIN3W_ANT", Hi_t, mC_t),
                    ):
                        _emit_custom_dve(
                            nc.vector, ops[opname], out=dst[:, 0:N2],
                            in0=src[:, 0:N2], in1=src[:, 2:N2 + 2],
                            perf_max=1)
                    off = 0   # outputs shifted one left; final reads from 0
                else:
                    nc.vector._custom_dve(ops["HMAX3_ANT"],
                                          out=mA_t[:, 0:NS],
                                          in0=Lo_t[:, 0:NS],
                                          in1=Lo_t[:, 1:NS + 1])
                    nc.vector._custom_dve(ops["HMIN3_ANT"],
                                          out=mC_t[:, 0:NS],
                                          in0=Hi_t[:, 0:NS],
                                          in1=Hi_t[:, 1:NS + 1])
                    nc.vector._custom_dve(ops["HMED3_ANT"],
                                          out=mB_t[:, 0:NS],
                                          in0=Me_t[:, 0:NS],
                                          in1=Me_t[:, 1:NS + 1])
                    off = 2

                # ---- final med3(mA, mB, mC) ----
                mT = f_pool.tile([128, R * WP], f16, name="mT", tag="mT")
                mU = f_pool.tile([128, R * WP], f16, name="mU", tag="mU")
                mV = m_pool.tile([128, LSZ], f16, name="mV", tag="mA")
                ot = ot_pool.tile([128, R * WP + 2], f16, name="ot",
                                  tag="ot")
                Af = mA_t[:, off:off + R * WP]
                Bf = mB_t[:, off:off + R * WP]
                Cf = mC_t[:, off:off + R * WP]
                # split the 4-op chain by element range: each engine runs an
                # independent chain over its own slice (no cross-engine deps)
                g = 0.0 if (dve_tail and k == K - 1) else final_gp_frac
                S = 2 * int(R * WP * (1.0 - g) / 2)
                for eng, s0, s1 in ((nc.vector, 0, S),
                                    (nc.gpsimd, S, R * WP)):
                    if s1 <= s0:
                        continue
                    sl = slice(s0, s1)
                    osl = slice(off + s0, off + s1)
                    eng.tensor_tensor(out=mT[:, sl], in0=mA_t[:, osl],
                                      in1=mB_t[:, osl], op=MIN)
                    eng.tensor_tensor(out=mU[:, sl], in0=mA_t[:, osl],
                                      in1=mB_t[:, osl], op=MAX)
                    eng.tensor_tensor(out=mV[:, sl], in0=mU[:, sl],
                                      in1=mC_t[:, osl], op=MIN)
                    eng.tensor_tensor(out=ot[:, sl], in0=mT[:, sl],
                                      in1=mV[:, sl], op=MAX)
                return ot

            def stage_bc_dve(k, X):
                """All-DVE variant: GPSIMD tensor_tensor min/max is
                rejected by the walrus codegen, so everything runs on the
                DVE; dead Lo/Me/Hi buffers are re-used for the final
                temporaries (same engine, in-order, so aliasing is free)."""
                for o, a, b, alu in (
                    (Pm, X[:, 1:R + 1:2], X[:, 2:R + 2:2], MIN),
                    (PM, X[:, 1:R + 1:2], X[:, 2:R + 2:2], MAX),
                    (Lo3[:, 0:R:2], X[:, 0:R:2], Pm, MIN),
                    (Hi3[:, 0:R:2], X[:, 0:R:2], PM, MAX),
                    (tEv, X[:, 0:R:2], PM, MIN),
                    (Me3[:, 0:R:2], Pm, tEv, MAX),
                    (Lo3[:, 1:R:2], X[:, 3:R + 2:2], Pm, MIN),
                    (Hi3[:, 1:R:2], X[:, 3:R + 2:2], PM, MAX),
                    (tOv, X[:, 3:R + 2:2], PM, MIN),
                    (Me3[:, 1:R:2], Pm, tOv, MAX),
                ):
                    nc.vector.tensor_tensor(out=o, in0=a, in1=b, op=alu)

                mA_t = m_pool.tile([128, LSZ], f16, name="mA", tag="mA")
                mB_t = m_pool.tile([128, LSZ], f16, name="mB", tag="mB")
                mC_t = m_pool.tile([128, LSZ], f16, name="mC", tag="mC")
                N2 = R * WP
                for opname, src, dst in (
                    ("HMAX3W_ANT", Lo_t, mA_t),
                    ("HMED3W_ANT", Me_t, mB_t),
                    ("HMIN3W_ANT", Hi_t, mC_t),
                ):
                    _emit_custom_dve(
                        nc.vector, ops[opname], out=dst[:, 0:N2],
                        in0=src[:, 0:N2], in1=src[:, 2:N2 + 2], perf_max=1)

                # final med3 (all ops even-offset fp16 step-1 -> 2x mode)
                mT = f_pool.tile([128, N2], f16, name="mT", tag="mT")
                mU = f_pool.tile([128, N2], f16, name="mU", tag="mU")
                mV = f_pool.tile([128, N2], f16, name="mV", tag="mV")
                ot = ot_pool.tile([128, R * WP + 2], f16, name="ot",
                                  tag="ot")
                sl = slice(0, N2)
                nc.vector.tensor_tensor(out=mT[:], in0=mA_t[:, sl],
                                        in1=mB_t[:, sl], op=MIN)
                nc.vector.tensor_tensor(out=mU[:], in0=mA_t[:, sl],
                                        in1=mB_t[:, sl], op=MAX)
                nc.vector.tensor_tensor(out=mV[:], in0=mU[:],
                                        in1=mC_t[:, sl], op=MIN)
                nc.vector.tensor_tensor(out=ot[:, sl], in0=mT[:],
                                        in1=mV[:], op=MAX)
                return ot

            ot_off = 1 if sliding2x else 0

            def stage_d(k, ot):
                """ACT cast fp16 -> fp32 + DMA out."""
                r0 = k * R
                o32 = o32_pool.tile([128, R * W], f32, name="o32", tag="o32")
                o3 = o32.rearrange("p (r w) -> p r w", w=W)
                ot3 = ot[:, ot_off:ot_off + R * WP].rearrange(
                    "p (r w) -> p r w", w=WP)
                nc.scalar.copy(out=o3[:], in_=ot3[:, :, 0:W])
                # separate DGE queue from the input loads (sync) so stores
                # never head-of-line-block the next loads
                nc.scalar.dma_start(out=og[:, r0:r0 + R, :], in_=o3[0:64])
                nc.scalar.dma_start(out=og[:, HH + r0:HH + r0 + R, :],
                                    in_=o3[64:128])

            Xs = {0: stage_a(0)}
            ots = {}
            ddel = 1 if use_gp else 0   # store-delay (GPSIMD-final only)
            for k in range(K):
                if k + 1 < K:
                    Xs[k + 1] = stage_a(k + 1)
                if k - ddel in ots:
                    stage_d(k - ddel, ots.pop(k - ddel))
                ots[k] = (stage_bc(k, Xs.pop(k)) if use_gp
                          else stage_bc_dve(k, Xs.pop(k)))
                if ddel == 0:
                    stage_d(k, ots.pop(k))
            for kk in sorted(ots):
                stage_d(kk, ots.pop(kk))

    nc.compile()
    return nc


def _build_v4(sizes=(4, 8, 12, 16, 20, 20, 20, 16, 8, 4), in_bufs=2,
              x16_bufs=1):
    """All-DVE variant with heterogeneous chunk sizes: small first/last
    chunks shorten the pipeline ramp/tail, large middle chunks amortize
    per-instruction overhead."""
    import concourse.bacc as bacc
    import concourse.mybir as mybir
    from concourse.tile import TileContext

    ops = _register_custom_ops()
    MIN = mybir.AluOpType.min
    MAX = mybir.AluOpType.max
    f32 = mybir.dt.float32
    f16 = mybir.dt.float16

    sizes = list(sizes)
    assert sum(sizes) == HH and all(s % 2 == 0 for s in sizes)
    K = len(sizes)
    r0s = [sum(sizes[:i]) for i in range(K)]
    RM = max(sizes)
    LSZ = RM * WP + 4

    nc = bacc.Bacc("TRN2", name="median_pool2d_v4")
    x = nc.dram_tensor("x", [C, H, W], f32, kind="ExternalInput")
    out = nc.dram_tensor("out", [C, H, W], f32, kind="ExternalOutput")
    xg = x.ap()
    og = out.ap()

    with TileContext(nc) as tc:
        with (
            tc.tile_pool(name="in32", bufs=in_bufs) as in_pool,
            tc.tile_pool(name="x16p", bufs=x16_bufs) as x16_pool,
            tc.tile_pool(name="vert", bufs=1) as v_pool,
            tc.tile_pool(name="mrg", bufs=1) as m_pool,
            tc.tile_pool(name="fin", bufs=1) as f_pool,
            tc.tile_pool(name="ot16", bufs=2) as ot_pool,
            tc.tile_pool(name="out32", bufs=1) as o32_pool,
        ):
            def stile(name):
                return v_pool.tile([128, LSZ], f16, name=name, tag=name)

            Lo_t = stile("Lo")
            Me_t = stile("Me")
            Hi_t = stile("Hi")
            for t in (Lo_t, Me_t, Hi_t):
                nc.gpsimd.memset(t[:, 0:2], 0.0)
                # zero every distinct chunk-size tail once; stale row data
                # at a smaller-chunk tail only feeds pad-column junk
                for rc in sorted(set(sizes)):
                    nc.gpsimd.memset(t[:, 2 + rc * WP:2 + rc * WP + 2], 0.0)

            def vtile(name, rows):
                return v_pool.tile([128, rows * WP], f16, name=name,
                                   tag=name)

            Pm_t = vtile("Pm", RM // 2)
            PM_t = vtile("PM", RM // 2)
            tE_t = vtile("tE", RM)

            def stage_a(k):
                rc = sizes[k]
                r0 = r0s[k]
                s32 = in_pool.tile([128, (RM + 2) * W], f32, name="s32",
                                   tag="s32")
                s3 = s32[:, 0:(rc + 2) * W].rearrange("p (r w) -> p r w",
                                                      w=W)
                if k == 0:
                    nc.gpsimd.memset(s3[0:64, 0:1], 0.0)
                    nc.sync.dma_start(out=s3[0:64, 1:rc + 2],
                                      in_=xg[:, 0:rc + 1, :])
                else:
                    nc.sync.dma_start(out=s3[0:64],
                                      in_=xg[:, r0 - 1:r0 + rc + 1, :])
                if k == K - 1:
                    nc.gpsimd.memset(s3[64:128, rc + 1:rc + 2], 0.0)
                    nc.sync.dma_start(out=s3[64:128, 0:rc + 1],
                                      in_=xg[:, HH + r0 - 1:H, :])
                else:
                    nc.sync.dma_start(
                        out=s3[64:128],
                        in_=xg[:, HH + r0 - 1:HH + r0 + rc + 1, :])
                x16 = x16_pool.tile([128, (RM + 2) * WP], f16, name="x16",
                                    tag="x16")
                X = x16[:, 0:(rc + 2) * WP].rearrange("p (r w) -> p r w",
                                                      w=WP)
                nc.gpsimd.memset(X[:, :, W:WP], 0.0)
                cast_eng = nc.vector if k == 0 else nc.scalar
                cast_eng.tensor_copy(out=X[:, :, 0:W], in_=s3[:]) \
                    if k == 0 else nc.scalar.copy(out=X[:, :, 0:W], in_=s3[:])
                return x16

            def _ap(view, dims):
                """Hand-built access pattern over `view`'s tile (keeps the
                partition dim and base offset)."""
                w = view.copy()
                p0 = list(view.ap)[0]
                w.ap = mybir.VecI64Pair(
                    [[int(p0[0]), int(p0[1])]]
                    + [[int(s), int(c)] for s, c in dims])
                return w

            def stage_b(k, x16):
                rc = sizes[k]
                Rh = rc // 2
                N2 = rc * WP
                X = x16[:, 0:(rc + 2) * WP].rearrange("p (r w) -> p r w",
                                                      w=WP)
                Pm, PM = (
                    t[:, 0:Rh * WP].rearrange("p (r w) -> p r w", w=WP)
                    for t in (Pm_t, PM_t))
                nc.vector.tensor_tensor(out=Pm, in0=X[:, 1:rc + 1:2],
                                        in1=X[:, 2:rc + 2:2], op=MIN)
                nc.vector.tensor_tensor(out=PM, in0=X[:, 1:rc + 1:2],
                                        in1=X[:, 2:rc + 2:2], op=MAX)
                # merged even/odd completions: one op per output array.
                # iteration order (pair i, parity g, col w); out rows are
                # consecutive; the X operand taps rows (2i, 2i+3); Pm/PM
                # broadcast over the parity dim (0-stride).
                grp = [[2 * WP, Rh], [3 * WP, 2], [1, WP]]
                bcast = [[WP, Rh], [0, 2], [1, WP]]
                rows = [[2 * WP, Rh], [WP, 2], [1, WP]]
                a_g = _ap(x16[:, 0:(rc + 2) * WP], grp)
                lo_o = _ap(Lo_t[:, 2:2 + N2], rows)
                hi_o = _ap(Hi_t[:, 2:2 + N2], rows)
                me_o = _ap(Me_t[:, 2:2 + N2], rows)
                pm_b = _ap(Pm_t[:, 0:Rh * WP], bcast)
                pM_b = _ap(PM_t[:, 0:Rh * WP], bcast)
                tEO = _ap(tE_t[:, 0:N2], rows)
                nc.vector.tensor_tensor(out=lo_o, in0=a_g, in1=pm_b, op=MIN)
                nc.vector.tensor_tensor(out=hi_o, in0=a_g, in1=pM_b, op=MAX)
                nc.vector.tensor_tensor(out=tEO, in0=a_g, in1=pM_b, op=MIN)
                nc.vector.tensor_tensor(out=me_o, in0=pm_b, in1=tEO,
                                        op=MAX)

                mA_t = m_pool.tile([128, LSZ], f16, name="mA", tag="mA")
                mB_t = m_pool.tile([128, LSZ], f16, name="mB", tag="mB")
                mC_t = m_pool.tile([128, LSZ], f16, name="mC", tag="mC")
                for opname, src, dst in (
                    ("HMAX3W_ANT", Lo_t, mA_t),
                    ("HMED3W_ANT", Me_t, mB_t),
                    ("HMIN3W_ANT", Hi_t, mC_t),
                ):
                    _emit_custom_dve(
                        nc.vector, ops[opname], out=dst[:, 0:N2],
                        in0=src[:, 0:N2], in1=src[:, 2:N2 + 2], perf_max=1)

                mT = f_pool.tile([128, RM * WP], f16, name="mT", tag="mT")
                mU = f_pool.tile([128, RM * WP], f16, name="mU", tag="mU")
                # mB is dead once mT/mU are done; reuse its buffer (all
                # ops on the DVE, in-order, so aliasing is dependency-free)
                mV = m_pool.tile([128, LSZ], f16, name="mV", tag="mB")
                ot = ot_pool.tile([128, RM * WP + 2], f16, name="ot",
                                  tag="ot")
                sl = slice(0, N2)
                nc.vector.tensor_tensor(out=mT[:, sl], in0=mA_t[:, sl],
                                        in1=mB_t[:, sl], op=MIN)
                nc.vector.tensor_tensor(out=mU[:, sl], in0=mA_t[:, sl],
                                        in1=mB_t[:, sl], op=MAX)
                nc.vector.tensor_tensor(out=mV[:, sl], in0=mU[:, sl],
                                        in1=mC_t[:, sl], op=MIN)
                nc.vector.tensor_tensor(out=ot[:, sl], in0=mT[:, sl],
                                        in1=mV[:, sl], op=MAX)
                return ot

            def stage_d(k, ot):
                rc = sizes[k]
                r0 = r0s[k]
                o32 = o32_pool.tile([128, RM * W], f32, name="o32",
                                    tag="o32")
                o3 = o32[:, 0:rc * W].rearrange("p (r w) -> p r w", w=W)
                ot3 = ot[:, 1:1 + rc * WP].rearrange("p (r w) -> p r w",
                                                     w=WP)
                if k == K - 1:
                    # tail: cast on the now-idle DVE (no ACT handoff) and
                    # pipeline the stores in row halves
                    h = rc // 2
                    nc.vector.tensor_copy(out=o3[:, 0:h],
                                          in_=ot3[:, 0:h, 0:W])
                    nc.scalar.dma_start(out=og[:, r0:r0 + h, :],
                                        in_=o3[0:64, 0:h])
                    nc.scalar.dma_start(
                        out=og[:, HH + r0:HH + r0 + h, :],
                        in_=o3[64:128, 0:h])
                    nc.vector.tensor_copy(out=o3[:, h:rc],
                                          in_=ot3[:, h:rc, 0:W])
                    nc.scalar.dma_start(out=og[:, r0 + h:r0 + rc, :],
                                        in_=o3[0:64, h:rc])
                    nc.scalar.dma_start(
                        out=og[:, HH + r0 + h:HH + r0 + rc, :],
                        in_=o3[64:128, h:rc])
                    return
                nc.scalar.copy(out=o3[:], in_=ot3[:, :, 0:W])
                nc.scalar.dma_start(out=og[:, r0:r0 + rc, :], in_=o3[0:64])
                nc.scalar.dma_start(out=og[:, HH + r0:HH + r0 + rc, :],
                                    in_=o3[64:128])

            Xs = {0: stage_a(0)}
            for k in range(K):
                if k + 1 < K:
                    Xs[k + 1] = stage_a(k + 1)
                ot = stage_b(k, Xs.pop(k))
                stage_d(k, ot)

    nc.compile()
    return nc


def _build_shared(R=16, gp_frac=0.0, dtype="float32", in_bufs=None,
                  out_bufs=None):
    """Fallback: 15-op/pixel fp32 TT network (previous working kernel)."""
    import concourse.bacc as bacc
    import concourse.mybir as mybir
    from concourse.tile import TileContext

    MIN = mybir.AluOpType.min
    MAX = mybir.AluOpType.max
    f32 = mybir.dt.float32
    cdt = getattr(mybir.dt, dtype)

    WPP = W + 2
    assert HH % R == 0 and R % 2 == 0
    K = HH // R
    Rh = R // 2

    nc = bacc.Bacc("TRN2", name="median_pool2d_s")
    x = nc.dram_tensor("x", [C, H, W], f32, kind="ExternalInput")
    out = nc.dram_tensor("out", [C, H, W], f32, kind="ExternalOutput")
    xg = x.ap()
    og = out.ap()

    def tt(out_ap, in0, in1, op):
        nc.vector.tensor_tensor(out=out_ap, in0=in0, in1=in1, op=op)

    if in_bufs is None:
        in_bufs = 3 if R <= 8 else 2
    if out_bufs is None:
        out_bufs = 3 if R <= 8 else 1
    with TileContext(nc) as tc:
        with (
            tc.tile_pool(name="io_in", bufs=in_bufs) as in_pool,
            tc.tile_pool(name="io_out", bufs=out_bufs) as out_pool,
            tc.tile_pool(name="work", bufs=1) as w_pool,
        ):
            def wtile(name, rows, width, tag=None):
                t = w_pool.tile([128, rows * width], cdt, name=name,
                                tag=tag or name)
                return t.rearrange("p (r w) -> p r w", w=width)

            for k in range(K):
                r0 = k * R
                it = in_pool.tile([128, (R + 2) * WPP], cdt, name="it",
                                  tag="it")
                it3 = it.rearrange("p (r w) -> p r w", w=WPP)
                nc.vector.memset(it3[:, :, 0:WPP:WPP - 1], 0.0)
                if k == 0:
                    nc.vector.memset(it3[0:64, 0:1, 1:W + 1], 0.0)
                    nc.sync.dma_start(out=it3[0:64, 1:R + 2, 1:W + 1],
                                      in_=xg[:, 0:R + 1, :])
                else:
                    nc.sync.dma_start(out=it3[0:64, :, 1:W + 1],
                                      in_=xg[:, r0 - 1:r0 + R + 1, :])
                if k == K - 1:
                    nc.vector.memset(it3[64:128, R + 1:R + 2, 1:W + 1], 0.0)
                    nc.sync.dma_start(out=it3[64:128, 0:R + 1, 1:W + 1],
                                      in_=xg[:, HH + r0 - 1:H, :])
                else:
                    nc.sync.dma_start(out=it3[64:128, :, 1:W + 1],
                                      in_=xg[:, HH + r0 - 1:HH + r0 + R + 1, :])

                Pm = wtile("Pm", Rh, WPP)
                PM = wtile("PM", Rh, WPP)
                tt(Pm, it3[:, 1:R + 1:2, :], it3[:, 2:R + 2:2, :], MIN)
                tt(PM, it3[:, 1:R + 1:2, :], it3[:, 2:R + 2:2, :], MAX)

                Lo3 = wtile("Lo", R, WPP)
                Me3 = wtile("Me", R, WPP)
                Hi3 = wtile("Hi", R, WPP)
                tE = wtile("tE", Rh, WPP)
                tO = wtile("tO", Rh, WPP)
                a_e = it3[:, 0:R:2, :]
                a_o = it3[:, 3:R + 2:2, :]
                tt(Lo3[:, 0:R:2], a_e, Pm, MIN)
                tt(Hi3[:, 0:R:2], a_e, PM, MAX)
                tt(tE, a_e, PM, MIN)
                tt(Me3[:, 0:R:2], Pm, tE, MAX)
                tt(Lo3[:, 1:R:2], a_o, Pm, MIN)
                tt(Hi3[:, 1:R:2], a_o, PM, MAX)
                tt(tO, a_o, PM, MIN)
                tt(Me3[:, 1:R:2], Pm, tO, MAX)

                NP = W // 2 + 1
                PA = wtile("PA", R, NP, tag="Pm")
                PC = wtile("PC", R, NP, tag="PM")
                Um = wtile("Um", R, NP, tag="tE")
                Vm = wtile("Vm", R, NP, tag="tO")
                tBe = wtile("tBe", R, W // 2, tag="Pm")
                tBo = wtile("tBo", R, W // 2, tag="PM")
                mA = wtile("mA", R, W)
                mB = wtile("mB", R, W)
                mC = wtile("mC", R, W)

                ev = slice(0, WPP, 2)
                od = slice(1, WPP, 2)
                tt(PA, Lo3[:, :, ev], Lo3[:, :, od], MAX)
                tt(mA[:, :, 0:W:2], PA[:, :, 0:NP - 1], Lo3[:, :, 2:WPP:2],
                   MAX)
                tt(mA[:, :, 1:W:2], PA[:, :, 1:NP], Lo3[:, :, 1:WPP - 2:2],
                   MAX)

                tt(PC, Hi3[:, :, ev], Hi3[:, :, od], MIN)
                tt(mC[:, :, 0:W:2], PC[:, :, 0:NP - 1], Hi3[:, :, 2:WPP:2],
                   MIN)
                tt(mC[:, :, 1:W:2], PC[:, :, 1:NP], Hi3[:, :, 1:WPP - 2:2],
                   MIN)

                tt(Um, Me3[:, :, ev], Me3[:, :, od], MIN)
                tt(Vm, Me3[:, :, ev], Me3[:, :, od], MAX)
                tt(tBe, Me3[:, :, 2:WPP:2], Vm[:, :, 0:NP - 1], MIN)
                tt(mB[:, :, 0:W:2], Um[:, :, 0:NP - 1], tBe, MAX)
                tt(tBo, Me3[:, :, 1:WPP - 2:2], Vm[:, :, 1:NP], MIN)
                tt(mB[:, :, 1:W:2], Um[:, :, 1:NP], tBo, MAX)

                mT = wtile("mT", R, W, tag="Lo")
                mU = wtile("mU", R, W, tag="Me")
                mV = wtile("mV", R, W, tag="Hi")
                ot = out_pool.tile([128, R * W], cdt, name="ot", tag="ot")
                ot3 = ot.rearrange("p (r w) -> p r w", w=W)
                tt(mT, mA, mB, MIN)
                tt(mU, mA, mB, MAX)
                tt(mV, mU, mC, MIN)
                tt(ot3, mT, mV, MAX)

                nc.sync.dma_start(out=og[:, r0:r0 + R, :], in_=ot3[0:64])
                nc.sync.dma_start(out=og[:, HH + r0:HH + r0 + R, :],
                                  in_=ot3[64:128])

    nc.compile()
    return nc


def _get_nc(variant="v3", **kw):
    key = (variant, tuple(sorted(kw.items())))
    if key not in _CACHE:
        if variant == "v4":
            _CACHE[key] = _build_v4(**kw)
        elif variant == "v3":
            _CACHE[key] = _build_v3(**kw)
        else:
            _CACHE[key] = _build_shared(**kw)
    return _CACHE[key]


_LAST_NC = None


def kernel(x: np.ndarray) -> np.ndarray:
    """MedianPool2d(3x3, s=1, p=1) on 8 NeuronCores."""
    global _LAST_NC
    from concourse.bass_utils import run_bass_kernel_spmd

    assert x.shape == (B, C, H, W), x.shape
    x = np.ascontiguousarray(x, dtype=np.float32)
    try:
        nc = _get_nc("v4")
    except Exception:
        try:
            nc = _get_nc("v3")
        except Exception:
            nc = _get_nc("shared", R=16)
    _LAST_NC = nc
    in_maps = [{"x": x[i]} for i in range(NCORES)]
    res = run_bass_kernel_spmd(nc, in_maps, core_ids=list(range(NCORES)))
    return np.stack([r["out"] for r in res.results], axis=0)


# revision 47
# speedup vs baseline: 2.3983x; 1.0004x over previous
"""MedianPool2d (3x3, stride 1, zero-pad 1) Trainium2 Bass kernel.

Full input x: (8, 64, 256, 256) fp32.  Sharding: pure data parallel over
batch -> core i processes x[i] (64, 256, 256).

Design (v4: fp16 + custom packed-2x sliding DVE ops):
  - Per-core layout: 128 SBUF partitions = (h, c), h in {0,1} = top/bottom
    128-row half, c = channel.  Heterogeneous chunks of output rows per
    partition (small first/last chunks shorten pipeline ramp/tail).
  - DMA loads fp32 rows into a staging tile; the Activation engine casts
    fp32 -> fp16 into a padded row layout (WP=258: 256 cols + 2 zero pad
    cols).  fp16 is exact for the median network itself (pure min/max
    selection); only the input cast rounds (~2.1e-4 l2 rel, gate 2e-2).
  - Vertical sort3 (Lo/Me/Hi per output row) as fp16 tensor_tensor
    min/max on the DVE with row-pair sharing: all ops full-row step-1
    4B-aligned -> genuine 2x DVE mode (0.52 ns/elem).  The even/odd
    completions are merged into single instructions via hand-built
    access patterns (a (pair, parity, col) iteration with a 3*WP-stride
    group dim on the X operand and a 0-stride parity broadcast on the
    pair operand) - 6 vertical instructions per chunk instead of 10.
  - Horizontal 3-tap merge via three hand-authored custom DVE micro-op
    programs (HMAX3W/HMED3W/HMIN3W, registered into the per-NEFF DVE
    table at runtime): ONE streaming pass each computes
    out[j] = f(T[j], T[j+1], T[j+2]) over the flat row stream.  The APs
    (src0 = T[0:N], src1 = T[2:N+2], both fp16 step-1 4B-aligned) make
    the RTL select 2x_1p packed mode, where the crossbar exposes four
    consecutive taps per cycle (SRC_0, SRC_0_HI, SRC_1, SRC_1_HI) - both
    parities' windows are pure spatial, even result -> WR0_LO, odd ->
    WR0_HI, 2 elem/cycle.  Each row carries 2 trailing zero pad cols so
    the previous row's tail doubles as the next row's left pad; a
    2-element zero prefix seeds row 0.  HW-validated bit-exact.
  - Final med3(mA, mB, mC) as 4 more fp16 2x TTs on the DVE.
    (GPSIMD tensor_tensor min/max and DMA-CCE min/max accumulate are both
    rejected by the walrus codegen, so GPSIMD only does small memsets.)
  - Output cast fp16 -> fp32 on the Activation engine, then DMA out on a
    separate DGE queue from the loads.

Effective cost ~12 TT-op-equivalents/pixel at 2x (vs 15 at 1x fp32 for
the baseline) -> 228.0us/core vs 546.7us baseline (2.40x).
"""

import numpy as np

B, C, H, W = 8, 64, 256, 256
NCORES = 8
HH = H // 2          # rows per half-strip
WP = W + 2           # padded row width (2 trailing zero cols)

_CACHE = {}
_OPS = None


def _register_custom_ops():
    """Register the three sliding-window custom DVE ops (idempotent).

    Each op streams src0 = T[j], src1 = T[j+1] and computes a 3-tap
    window f(T[j-1], T[j], T[j+1]) per output element:
      - T[j], T[j+1] arrive spatially on the two source streams;
      - T[j-1] = src1 two elements back, read via NEXT_ALU_OUT_A from a
        pipeline stage that latches raw src1 into its a-flop each cycle.
    Outputs j=0,1 use pre-instruction flop state (garbage); callers must
    treat the first two output elements as scrap.
    """
    global _OPS
    if _OPS is not None:
        return _OPS

    import concourse.dve_ops as dve_ops
    from concourse.dve_spec import Spec, Src0, Src1, minn
    from concourse.dve_uop import (
        ENABLE,
        AluInp,
        AluOp,
        DelayInp,
        DveOpSpec,
        InpSel,
        OutPath,
        OutSel,
        Trigger,
        UopConfig,
    )

    def _shift2(in1):
        z = np.zeros_like(in1[..., :2])
        return np.concatenate([z, in1[..., :-2]], axis=-1)

    def _hmax3_ref(in0, in1, c0, c1, c2):
        return np.maximum(np.maximum(in0, in1), _shift2(in1))

    def _hmin3_ref(in0, in1, c0, c1, c2):
        return np.minimum(np.minimum(in0, in1), _shift2(in1))

    def _hmed3_ref(in0, in1, c0, c1, c2):
        c = _shift2(in1)
        return np.maximum(np.minimum(in0, in1),
                          np.minimum(np.maximum(in0, in1), c))

    def _mm3_uops(op3):
        """out[j] = op3(src0[j], src1[j], src1[j-2]) single-uop program."""
        u = UopConfig()
        u.enable_input(InpSel.SRC_0, 1)     # -> PREV_DELAY_0 at blk0
        u.enable_input(InpSel.SRC_1, 2)     # -> PREV_DELAY_1 at blk0
        u.require_inp0 = ENABLE
        u.require_inp1 = ENABLE
        u.trigger = (Trigger.SRC_TENSOR_DONE, Trigger.NONE, Trigger.NONE)
        dp = u.datapath_config
        # blk0: m = op3(src0, src1)
        dp[0].enable_alu(op3, AluInp.PREV_DELAY_0, AluInp.PREV_DELAY_1)
        dp[0].pass_through_delay(1)
        # blk1: out = op3(m, src1[j-2])  (blk2's a-flop, prev cycle)
        dp[1].enable_alu(op3, AluInp.PREV_ALU_OUT, AluInp.NEXT_ALU_OUT_A)
        dp[1].pass_through_delay(1)
        # blk2: latch raw src1 into the a-flop; stash out in delay chain 0
        dp[2].enable_alu(AluOp.BYPASS, AluInp.PREV_DELAY_1)
        dp[2].alu_out_a_enable = ENABLE
        dp[2].enable_delay_from_src(DelayInp.PREV_ALU_OUT, 0)
        for kblk in range(3, 8):
            dp[kblk].pass_through_alu().pass_through_delay(0)
        u.enable_output(OutSel.DELAY_0, OutPath.WR0_LO)
        return [u]

    def _med3_uops():
        """out[j] = med3(src0[j], src1[j], src1[j-2]) single-uop program."""
        u = UopConfig()
        u.enable_input(InpSel.SRC_0, 1)     # chain 0
        u.enable_input(InpSel.SRC_1, 2)     # chain 1
        u.require_inp0 = ENABLE
        u.require_inp1 = ENABLE
        u.trigger = (Trigger.SRC_TENSOR_DONE, Trigger.NONE, Trigger.NONE)
        dp = u.datapath_config
        # blk0: p = min(a, b)
        dp[0].enable_alu(AluOp.MIN, AluInp.PREV_DELAY_0, AluInp.PREV_DELAY_1)
        dp[0].pass_through_delay(0, 1)
        # blk1: q = max(a, b); stash p in chain 2
        dp[1].enable_alu(AluOp.MAX, AluInp.PREV_DELAY_0, AluInp.PREV_DELAY_1)
        dp[1].enable_delay_from_src(DelayInp.PREV_ALU_OUT, 2)
        dp[1].pass_through_delay(1)
        # blk2: t = min(q, c) with c = src1[j-2] via blk3's a-flop
        dp[2].enable_alu(AluOp.MIN, AluInp.PREV_ALU_OUT, AluInp.NEXT_ALU_OUT_A)
        dp[2].pass_through_delay(1, 2)
        # blk3: latch raw src1 into a-flop; stash t in chain 3
        dp[3].enable_alu(AluOp.BYPASS, AluInp.PREV_DELAY_1)
        dp[3].alu_out_a_enable = ENABLE
        dp[3].enable_delay_from_src(DelayInp.PREV_ALU_OUT, 3)
        dp[3].pass_through_delay(2)
        # blk4: out = max(p, t)
        dp[4].enable_alu(AluOp.MAX, AluInp.PREV_DELAY_2, AluInp.PREV_DELAY_3)
        for kblk in range(5, 8):
            dp[kblk].pass_through_alu()
        u.enable_output(OutSel.ALU_OUT, OutPath.WR0_LO)
        return [u]

    # ---- 2x (packed fp16) window-start variants -------------------------
    # Semantics: out[j] = f(T[j], T[j+1], T[j+2]) with src0 = T[0:N],
    # src1 = T[2:N+2], all APs fp16 step-1 4B-aligned so the RTL always
    # selects 2x_1p.  Per pair-cycle the crossbar exposes 4 consecutive
    # taps (SRC_0, SRC_0_HI, SRC_1, SRC_1_HI); both parities' windows are
    # pure spatial.  Even result -> WR0_LO, odd -> WR0_HI.

    def _shiftcat(in0, in1):
        # full stream T (len N+2) from the two views
        return np.concatenate([in0, in1[..., -2:]], axis=-1)

    def _hmax3w_ref(in0, in1, c0, c1, c2):
        T = _shiftcat(in0, in1)
        return np.maximum(np.maximum(T[..., :-2], T[..., 1:-1]), T[..., 2:])

    def _hmin3w_ref(in0, in1, c0, c1, c2):
        T = _shiftcat(in0, in1)
        return np.minimum(np.minimum(T[..., :-2], T[..., 1:-1]), T[..., 2:])

    def _hmed3w_ref(in0, in1, c0, c1, c2):
        T = _shiftcat(in0, in1)
        a, b, c = T[..., :-2], T[..., 1:-1], T[..., 2:]
        return np.maximum(np.minimum(a, b),
                          np.minimum(np.maximum(a, b), c))

    def _dummy_1x(op3):
        """Placeholder REGULAR-mode program (never selected: the APs
        always satisfy the 2x_1p conditions)."""
        u = UopConfig()
        u.enable_input(InpSel.SRC_0, 1)
        u.enable_input(InpSel.SRC_1, 2)
        u.require_inp0 = ENABLE
        u.require_inp1 = ENABLE
        u.trigger = (Trigger.SRC_TENSOR_DONE, Trigger.NONE, Trigger.NONE)
        dp = u.datapath_config
        dp[0].enable_alu(op3, AluInp.PREV_DELAY_0, AluInp.PREV_DELAY_1)
        for kblk in range(1, 8):
            dp[kblk].pass_through_alu()
        u.enable_output(OutSel.ALU_OUT, OutPath.WR0_LO)
        return [u]

    def _mm3_w2x_uops(op3):
        """2x program: E = op3(S0, S0H, S1) -> WR0_LO, O = op3(S0H, S1,
        S1H) -> WR0_HI."""
        u = UopConfig()
        u.enable_input(InpSel.SRC_0, 1)      # chain 0
        u.enable_input(InpSel.SRC_0_HI, 2)   # chain 1
        u.enable_input(InpSel.SRC_1, 3)      # chain 2
        u.enable_input(InpSel.SRC_1_HI, 4)   # chain 3
        u.require_inp0 = ENABLE
        u.require_inp1 = ENABLE
        u.trigger = (Trigger.SRC_TENSOR_DONE, Trigger.NONE, Trigger.NONE)
        dp = u.datapath_config
        dp[0].enable_alu(op3, AluInp.PREV_DELAY_0, AluInp.PREV_DELAY_1)
        dp[0].pass_through_delay(1, 2, 3)
        dp[1].enable_alu(op3, AluInp.PREV_ALU_OUT, AluInp.PREV_DELAY_2)
        dp[1].pass_through_delay(1, 2, 3)
        dp[2].enable_alu(op3, AluInp.PREV_DELAY_1, AluInp.PREV_DELAY_2)
        dp[2].enable_delay_from_src(DelayInp.PREV_ALU_OUT, 4)   # E
        dp[2].pass_through_delay(3)
        dp[3].enable_alu(op3, AluInp.PREV_ALU_OUT, AluInp.PREV_DELAY_3)
        dp[3].pass_through_delay(4)
        for kblk in range(4, 8):
            dp[kblk].pass_through_alu().pass_through_delay(4)
        u.enable_output(OutSel.DELAY_4, OutPath.WR0_LO)
        u.enable_output(OutSel.ALU_OUT, OutPath.WR0_HI)
        return [u]

    def _med3_w2x_uops():
        """2x program: E = med3(S0, S0H, S1) -> WR0_LO, O = med3(S0H, S1,
        S1H) -> WR0_HI.  Exactly 8 ALU blocks."""
        u = UopConfig()
        u.enable_input(InpSel.SRC_0, 1)      # chain 0
        u.enable_input(InpSel.SRC_0_HI, 2)   # chain 1
        u.enable_input(InpSel.SRC_1, 3)      # chain 2
        u.enable_input(InpSel.SRC_1_HI, 4)   # chain 3
        u.require_inp0 = ENABLE
        u.require_inp1 = ENABLE
        u.trigger = (Trigger.SRC_TENSOR_DONE, Trigger.NONE, Trigger.NONE)
        dp = u.datapath_config
        # E-half: med3(d0, d1, d2)
        dp[0].enable_alu(AluOp.MIN, AluInp.PREV_DELAY_0, AluInp.PREV_DELAY_1)
        dp[0].pass_through_delay(0, 1, 2, 3)
        dp[1].enable_alu(AluOp.MAX, AluInp.PREV_DELAY_0, AluInp.PREV_DELAY_1)
        dp[1].enable_delay_from_src(DelayInp.PREV_ALU_OUT, 4)   # pE
        dp[1].pass_through_delay(1, 2, 3)
        dp[2].enable_alu(AluOp.MIN, AluInp.PREV_ALU_OUT, AluInp.PREV_DELAY_2)
        dp[2].pass_through_delay(1, 2, 3, 4)
        dp[3].enable_alu(AluOp.MAX, AluInp.PREV_DELAY_4, AluInp.PREV_ALU_OUT)
        dp[3].pass_through_delay(1, 2, 3)
        # O-half: med3(d1, d2, d3)
        dp[4].enable_alu(AluOp.MIN, AluInp.PREV_DELAY_1, AluInp.PREV_DELAY_2)
        dp[4].enable_delay_from_src(DelayInp.PREV_ALU_OUT, 4)   # E
        dp[4].pass_through_delay(1, 2, 3)
        dp[5].enable_alu(AluOp.MAX, AluInp.PREV_DELAY_1, AluInp.PREV_DELAY_2)
        dp[5].enable_delay_from_src(DelayInp.PREV_ALU_OUT, 5)   # pO
        dp[5].pass_through_delay(3, 4)
        dp[6].enable_alu(AluOp.MIN, AluInp.PREV_ALU_OUT, AluInp.PREV_DELAY_3)
        dp[6].pass_through_delay(4, 5)
        dp[7].enable_alu(AluOp.MAX, AluInp.PREV_DELAY_5, AluInp.PREV_ALU_OUT)
        dp[7].pass_through_delay(4)
        u.enable_output(OutSel.DELAY_4, OutPath.WR0_LO)
        u.enable_output(OutSel.ALU_OUT, OutPath.WR0_HI)
        return [u]

    class _SlidingOp:
        """Duck-typed DveOp: hand-authored uops instead of lower(spec)."""

        def __init__(self, name, reference, uops_builder, uops_2x_builder=None):
            # body is a placeholder that reads Src0+Src1 (rd1_en) and no
            # C2/C3; only `reference` is ever evaluated (bass_interp).
            self.name = name
            self.spec = Spec(body=minn(Src0, Src1), reference=reference)
            self.subdim = False
            self.perf_en = {}
            self._builder = uops_builder
            self._builder_2x = uops_2x_builder
            self._cache = {}

        def compile(self, ver):
            if ver not in self._cache:
                s = DveOpSpec(
                    name=self.name,
                    opcode=dve_ops.get_dve_sub_opcode(self.name),
                    uops=self._builder(),
                    uops_2x=(self._builder_2x() if self._builder_2x
                             else None),
                    perf_max=1 if self._builder_2x else 0,
                    rd1_en=True,
                )
                s.validate(ver)
                self._cache[ver] = s
            return self._cache[ver]

    defs = [
        ("HMAX3_ANT", _hmax3_ref, lambda: _mm3_uops(AluOp.MAX), None),
        ("HMIN3_ANT", _hmin3_ref, lambda: _mm3_uops(AluOp.MIN), None),
        ("HMED3_ANT", _hmed3_ref, _med3_uops, None),
        ("HMAX3W_ANT", _hmax3w_ref, lambda: _dummy_1x(AluOp.MAX),
         lambda: _mm3_w2x_uops(AluOp.MAX)),
        ("HMIN3W_ANT", _hmin3w_ref, lambda: _dummy_1x(AluOp.MIN),
         lambda: _mm3_w2x_uops(AluOp.MIN)),
        ("HMED3W_ANT", _hmed3w_ref, lambda: _dummy_1x(AluOp.MIN),
         _med3_w2x_uops),
    ]
    ops = {}
    for name, ref, builder, builder2x in defs:
        if name not in dve_ops._SUB_OPCODE_FOR_NAME:
            row = max(dve_ops._SUB_OPCODE_FOR_NAME.values()) + 1
            assert row < 0x20, "custom DVE opcode rows exhausted"
            dve_ops._SUB_OPCODE_FOR_NAME[name] = row
        op = _SlidingOp(name, ref, builder, builder2x)
        # replace any previous registration (idempotent across reloads)
        dve_ops.OPS[:] = [o for o in dve_ops.OPS if o.name != name] + [op]
        dve_ops.CUSTOM_DVE_SPECS[name] = op.spec
        ops[name] = op
    _OPS = ops
    return ops


def _emit_custom_dve(v, op, out, in0, in1, perf_max=0):
    """Like bass.Vector._custom_dve but with perf_max set at construction
    (the Tile scheduler drops post-hoc attribute edits)."""
    import concourse.bass_isa as bass_isa
    import concourse.mybir as mybir
    from concourse.dve_ops import get_dve_sub_opcode

    b = v.bass
    if op.name not in b.m.ant_custom_dve_ops:
        b.m.ant_custom_dve_ops = sorted({*b.m.ant_custom_dve_ops, op.name})
    shape = bass_isa.CustomDveShape.TTSS
    isa_opcode = b.isa.Opcode[
        f"NEURON_ISA_TPB_OPCODE_CUSTOM_DVE_ANT_{shape.slot()}"
    ].value
    imm = lambda: mybir.ImmediateValue(dtype=mybir.dt.float32, value=0.0)
    return v.add_instruction(
        bass_isa.InstCustomDveAnt(
            name=b.get_next_instruction_name(),
            op_name=op.name,
            rd1_en=True,
            subdim=0,
            imm2=0.0,
            shape=shape,
            row=get_dve_sub_opcode(op.name),
            perf_max=perf_max,
            isa_opcode=isa_opcode,
            ins=[v.lower_ap(in0, for_isa=True),
                 v.lower_ap(in1, for_isa=True), imm(), imm()],
            outs=[v.lower_ap(out, for_isa=True)],
        )
    )


def _build_v3(R=8, final_gp_frac=0.65, n_vert_gp=1, in_bufs=2, mrg_bufs=1,
              x16_bufs=2, dve_tail=True, sliding2x=True, use_gp=False):
    """fp16 + custom sliding ops + ACT casts + GPSIMD final stage.

    Software-pipelined emission (engines execute their streams in order):
    per iteration k we emit  A(k+1) = load+cast of the NEXT chunk,
    D(k-1) = store of the PREVIOUS chunk, then B(k) = DVE work and
    C(k) = final stage.  This keeps the ACT in-cast ahead of the
    GPSIMD-dependent out-cast in the ACT/sync instruction streams.

    n_final_gp: how many of the 4 final med3 tensor_tensor ops run on
    GPSIMD (rest on DVE).  n_vert_gp: how many vertical ops on GPSIMD.
    dve_tail: run the last chunk's final stage on the DVE (shorter tail).
    """
    import concourse.bacc as bacc
    import concourse.mybir as mybir
    from concourse.tile import TileContext

    ops = _register_custom_ops()
    MIN = mybir.AluOpType.min
    MAX = mybir.AluOpType.max
    f32 = mybir.dt.float32
    f16 = mybir.dt.float16

    assert HH % R == 0 and R % 2 == 0
    K = HH // R
    Rh = R // 2
    NS = 2 + R * WP          # sliding stream length (2 prefix + rows)
    LSZ = R * WP + 4         # sliding tile: prefix 2 + rows + tail 2

    nc = bacc.Bacc("TRN2", name="median_pool2d_v3")
    x = nc.dram_tensor("x", [C, H, W], f32, kind="ExternalInput")
    out = nc.dram_tensor("out", [C, H, W], f32, kind="ExternalOutput")
    xg = x.ap()
    og = out.ap()

    with TileContext(nc) as tc:
        with (
            tc.tile_pool(name="in32", bufs=in_bufs) as in_pool,
            tc.tile_pool(name="x16p", bufs=x16_bufs) as x16_pool,
            tc.tile_pool(name="vert", bufs=1) as v_pool,
            tc.tile_pool(name="mrg", bufs=mrg_bufs) as m_pool,
            tc.tile_pool(name="fin", bufs=1) as f_pool,
            tc.tile_pool(name="ot16", bufs=2) as ot_pool,
            tc.tile_pool(name="out32", bufs=1) as o32_pool,
        ):
            # persistent sliding tiles: prefix/tail zeros written once
            def stile(name):
                t = v_pool.tile([128, LSZ], f16, name=name, tag=name)
                rows = t[:, 2:2 + R * WP].rearrange("p (r w) -> p r w", w=WP)
                return t, rows

            Lo_t, Lo3 = stile("Lo")
            Me_t, Me3 = stile("Me")
            Hi_t, Hi3 = stile("Hi")
            for t in (Lo_t, Me_t, Hi_t):
                nc.gpsimd.memset(t[:, 0:2], 0.0)
                nc.gpsimd.memset(t[:, 2 + R * WP:LSZ], 0.0)

            def vtile(name, rows):
                t = v_pool.tile([128, rows * WP], f16, name=name, tag=name)
                return t.rearrange("p (r w) -> p r w", w=WP)

            Pm = vtile("Pm", Rh)
            PM = vtile("PM", Rh)
            tEv = vtile("tE", Rh)
            tOv = vtile("tO", Rh)

            def stage_a(k):
                """DMA fp32 rows + ACT cast to padded fp16 rows."""
                r0 = k * R
                s32 = in_pool.tile([128, (R + 2) * W], f32, name="s32",
                                   tag="s32")
                s3 = s32.rearrange("p (r w) -> p r w", w=W)
                if k == 0:
                    nc.vector.memset(s3[0:64, 0:1], 0.0)
                    nc.sync.dma_start(out=s3[0:64, 1:R + 2],
                                      in_=xg[:, 0:R + 1, :])
                else:
                    nc.sync.dma_start(out=s3[0:64],
                                      in_=xg[:, r0 - 1:r0 + R + 1, :])
                if k == K - 1:
                    nc.vector.memset(s3[64:128, R + 1:R + 2], 0.0)
                    nc.sync.dma_start(out=s3[64:128, 0:R + 1],
                                      in_=xg[:, HH + r0 - 1:H, :])
                else:
                    nc.sync.dma_start(out=s3[64:128],
                                      in_=xg[:, HH + r0 - 1:HH + r0 + R + 1, :])
                x16 = x16_pool.tile([128, (R + 2) * WP], f16, name="x16",
                                    tag="x16")
                X = x16.rearrange("p (r w) -> p r w", w=WP)
                nc.gpsimd.memset(X[:, :, W:WP], 0.0)
                nc.scalar.copy(out=X[:, :, 0:W], in_=s3[:])
                return X

            def stage_bc(k, X):
                """DVE vertical + sliding, then final stage."""
                # ---- vertical sort3 (fp16 TT, 2x mode, pair-shared) ----
                vops = [
                    (Pm, X[:, 1:R + 1:2], X[:, 2:R + 2:2], MIN),
                    (PM, X[:, 1:R + 1:2], X[:, 2:R + 2:2], MAX),
                    (Lo3[:, 0:R:2], X[:, 0:R:2], Pm, MIN),
                    (Hi3[:, 0:R:2], X[:, 0:R:2], PM, MAX),
                    (tEv, X[:, 0:R:2], PM, MIN),
                    (Me3[:, 0:R:2], Pm, tEv, MAX),
                    (Lo3[:, 1:R:2], X[:, 3:R + 2:2], Pm, MIN),
                    (Hi3[:, 1:R:2], X[:, 3:R + 2:2], PM, MAX),
                    (tOv, X[:, 3:R + 2:2], PM, MIN),
                    (Me3[:, 1:R:2], Pm, tOv, MAX),
                ]
                for i, (o, a, b, alu) in enumerate(vops):
                    eng = (nc.gpsimd if i >= len(vops) - n_vert_gp
                           else nc.vector)
                    eng.tensor_tensor(out=o, in0=a, in1=b, op=alu)

                # ---- horizontal 3-tap merge: custom sliding DVE ops ----
                mA_t = m_pool.tile([128, LSZ], f16, name="mA", tag="mA")
                mB_t = m_pool.tile([128, LSZ], f16, name="mB", tag="mB")
                mC_t = m_pool.tile([128, LSZ], f16, name="mC", tag="mC")
                if sliding2x:
                    # window-start form at 2x: out[j] = f(T[j..j+2]);
                    # all APs even-offset fp16 step-1 -> RTL picks 2x_1p.
                    N2 = R * WP
                    for opname, src, dst in (
                        ("HMAX3W_ANT", Lo_t, mA_t),
                        ("HMED3W_ANT", Me_t, mB_t),
                        ("HMIN3W_ANT", Hi_t, mC_t),
                    ):
                        _emit_custom_dve(
                            nc.vector, ops[opname], out=dst[:, 0:N2],
                            in0=src[:, 0:N2], in1=src[:, 2:N2 + 2],
                            perf_max=1)
                    off = 0   # outputs shifted one left; final reads from 0
                else:
                    nc.vector._custom_dve(ops["HMAX3_ANT"],
                                          out=mA_t[:, 0:NS],
                                          in0=Lo_t[:, 0:NS],
                                          in1=Lo_t[:, 1:NS + 1])
                    nc.vector._custom_dve(ops["HMIN3_ANT"],
                                          out=mC_t[:, 0:NS],
                                          in0=Hi_t[:, 0:NS],
                                          in1=Hi_t[:, 1:NS + 1])
                    nc.vector._custom_dve(ops["HMED3_ANT"],
                                          out=mB_t[:, 0:NS],
                                          in0=Me_t[:, 0:NS],
                                          in1=Me_t[:, 1:NS + 1])
                    off = 2

                # ---- final med3(mA, mB, mC) ----
                mT = f_pool.tile([128, R * WP], f16, name="mT", tag="mT")
                mU = f_pool.tile([128, R * WP], f16, name="mU", tag="mU")
                mV = m_pool.tile([128, LSZ], f16, name="mV", tag="mA")
                ot = ot_pool.tile([128, R * WP + 2], f16, name="ot",
                                  tag="ot")
                Af = mA_t[:, off:off + R * WP]
                Bf = mB_t[:, off:off + R * WP]
                Cf = mC_t[:, off:off + R * WP]
                # split the 4-op chain by element range: each engine runs an
                # independent chain over its own slice (no cross-engine deps)
                g = 0.0 if (dve_tail and k == K - 1) else final_gp_frac
                S = 2 * int(R * WP * (1.0 - g) / 2)
                for eng, s0, s1 in ((nc.vector, 0, S),
                                    (nc.gpsimd, S, R * WP)):
                    if s1 <= s0:
                        continue
                    sl = slice(s0, s1)
                    osl = slice(off + s0, off + s1)
                    eng.tensor_tensor(out=mT[:, sl], in0=mA_t[:, osl],
                                      in1=mB_t[:, osl], op=MIN)
                    eng.tensor_tensor(out=mU[:, sl], in0=mA_t[:, osl],
                                      in1=mB_t[:, osl], op=MAX)
                    eng.tensor_tensor(out=mV[:, sl], in0=mU[:, sl],
                                      in1=mC_t[:, osl], op=MIN)
                    eng.tensor_tensor(out=ot[:, sl], in0=mT[:, sl],
                                      in1=mV[:, sl], op=MAX)
                return ot

            def stage_bc_dve(k, X):
                """All-DVE variant: GPSIMD tensor_tensor min/max is
                rejected by the walrus codegen, so everything runs on the
                DVE; dead Lo/Me/Hi buffers are re-used for the final
                temporaries (same engine, in-order, so aliasing is free)."""
                for o, a, b, alu in (
                    (Pm, X[:, 1:R + 1:2], X[:, 2:R + 2:2], MIN),
                    (PM, X[:, 1:R + 1:2], X[:, 2:R + 2:2], MAX),
                    (Lo3[:, 0:R:2], X[:, 0:R:2], Pm, MIN),
                    (Hi3[:, 0:R:2], X[:, 0:R:2], PM, MAX),
                    (tEv, X[:, 0:R:2], PM, MIN),
                    (Me3[:, 0:R:2], Pm, tEv, MAX),
                    (Lo3[:, 1:R:2], X[:, 3:R + 2:2], Pm, MIN),
                    (Hi3[:, 1:R:2], X[:, 3:R + 2:2], PM, MAX),
                    (tOv, X[:, 3:R + 2:2], PM, MIN),
                    (Me3[:, 1:R:2], Pm, tOv, MAX),
                ):
                    nc.vector.tensor_tensor(out=o, in0=a, in1=b, op=alu)

                mA_t = m_pool.tile([128, LSZ], f16, name="mA", tag="mA")
                mB_t = m_pool.tile([128, LSZ], f16, name="mB", tag="mB")
                mC_t = m_pool.tile([128, LSZ], f16, name="mC", tag="mC")
                N2 = R * WP
                for opname, src, dst in (
                    ("HMAX3W_ANT", Lo_t, mA_t),
                    ("HMED3W_ANT", Me_t, mB_t),
                    ("HMIN3W_ANT", Hi_t, mC_t),
                ):
                    _emit_custom_dve(
                        nc.vector, ops[opname], out=dst[:, 0:N2],
                        in0=src[:, 0:N2], in1=src[:, 2:N2 + 2], perf_max=1)

                # final med3 (all ops even-offset fp16 step-1 -> 2x mode)
                mT = f_pool.tile([128, N2], f16, name="mT", tag="mT")
                mU = f_pool.tile([128, N2], f16, name="mU", tag="mU")
                mV = f_pool.tile([128, N2], f16, name="mV", tag="mV")
                ot = ot_pool.tile([128, R * WP + 2], f16, name="ot",
                                  tag="ot")
                sl = slice(0, N2)
                nc.vector.tensor_tensor(out=mT[:], in0=mA_t[:, sl],
                                        in1=mB_t[:, sl], op=MIN)
                nc.vector.tensor_tensor(out=mU[:], in0=mA_t[:, sl],
                                        in1=mB_t[:, sl], op=MAX)
                nc.vector.tensor_tensor(out=mV[:], in0=mU[:],
                                        in1=mC_t[:, sl], op=MIN)
                nc.vector.tensor_tensor(out=ot[:, sl], in0=mT[:],
                                        in1=mV[:], op=MAX)
                return ot

            ot_off = 1 if sliding2x else 0

            def stage_d(k, ot):
                """ACT cast fp16 -> fp32 + DMA out."""
                r0 = k * R
                o32 = o32_pool.tile([128, R * W], f32, name="o32", tag="o32")
                o3 = o32.rearrange("p (r w) -> p r w", w=W)
                ot3 = ot[:, ot_off:ot_off + R * WP].rearrange(
                    "p (r w) -> p r w", w=WP)
                nc.scalar.copy(out=o3[:], in_=ot3[:, :, 0:W])
                # separate DGE queue from the input loads (sync) so stores
                # never head-of-line-block the next loads
                nc.scalar.dma_start(out=og[:, r0:r0 + R, :], in_=o3[0:64])
                nc.scalar.dma_start(out=og[:, HH + r0:HH + r0 + R, :],
                                    in_=o3[64:128])

            Xs = {0: stage_a(0)}
            ots = {}
            ddel = 1 if use_gp else 0   # store-delay (GPSIMD-final only)
            for k in range(K):
                if k + 1 < K:
                    Xs[k + 1] = stage_a(k + 1)
                if k - ddel in ots:
                    stage_d(k - ddel, ots.pop(k - ddel))
                ots[k] = (stage_bc(k, Xs.pop(k)) if use_gp
                          else stage_bc_dve(k, Xs.pop(k)))
                if ddel == 0:
                    stage_d(k, ots.pop(k))
            for kk in sorted(ots):
                stage_d(kk, ots.pop(kk))

    nc.compile()
    return nc


def _build_v4(sizes=(4, 8, 12, 16, 20, 20, 20, 16, 8, 4), in_bufs=2,
              x16_bufs=1):
    """All-DVE variant with heterogeneous chunk sizes: small first/last
    chunks shorten the pipeline ramp/tail, large middle chunks amortize
    per-instruction overhead."""
    import concourse.bacc as bacc
    import concourse.mybir as mybir
    from concourse.tile import TileContext

    ops = _register_custom_ops()
    MIN = mybir.AluOpType.min
    MAX = mybir.AluOpType.max
    f32 = mybir.dt.float32
    f16 = mybir.dt.float16

    sizes = list(sizes)
    assert sum(sizes) == HH and all(s % 2 == 0 for s in sizes)
    K = len(sizes)
    r0s = [sum(sizes[:i]) for i in range(K)]
    RM = max(sizes)
    LSZ = RM * WP + 4

    nc = bacc.Bacc("TRN2", name="median_pool2d_v4")
    x = nc.dram_tensor("x", [C, H, W], f32, kind="ExternalInput")
    out = nc.dram_tensor("out", [C, H, W], f32, kind="ExternalOutput")
    xg = x.ap()
    og = out.ap()

    with TileContext(nc) as tc:
        with (
            tc.tile_pool(name="in32", bufs=in_bufs) as in_pool,
            tc.tile_pool(name="x16p", bufs=x16_bufs) as x16_pool,
            tc.tile_pool(name="vert", bufs=1) as v_pool,
            tc.tile_pool(name="mrg", bufs=1) as m_pool,
            tc.tile_pool(name="fin", bufs=1) as f_pool,
            tc.tile_pool(name="ot16", bufs=2) as ot_pool,
            tc.tile_pool(name="out32", bufs=1) as o32_pool,
        ):
            def stile(name):
                return v_pool.tile([128, LSZ], f16, name=name, tag=name)

            Lo_t = stile("Lo")
            Me_t = stile("Me")
            Hi_t = stile("Hi")
            for t in (Lo_t, Me_t, Hi_t):
                nc.gpsimd.memset(t[:, 0:2], 0.0)
                # zero every distinct chunk-size tail once; stale row data
                # at a smaller-chunk tail only feeds pad-column junk
                for rc in sorted(set(sizes)):
                    nc.gpsimd.memset(t[:, 2 + rc * WP:2 + rc * WP + 2], 0.0)

            def vtile(name, rows):
                return v_pool.tile([128, rows * WP], f16, name=name,
                                   tag=name)

            Pm_t = vtile("Pm", RM // 2)
            PM_t = vtile("PM", RM // 2)
            tE_t = vtile("tE", RM)

            def stage_a(k):
                rc = sizes[k]
                r0 = r0s[k]
                s32 = in_pool.tile([128, (RM + 2) * W], f32, name="s32",
                                   tag="s32")
                s3 = s32[:, 0:(rc + 2) * W].rearrange("p (r w) -> p r w",
                                                      w=W)
                if k == 0:
                    nc.gpsimd.memset(s3[0:64, 0:1], 0.0)
                    nc.sync.dma_start(out=s3[0:64, 1:rc + 2],
                                      in_=xg[:, 0:rc + 1, :])
                else:
                    nc.sync.dma_start(out=s3[0:64],
                                      in_=xg[:, r0 - 1:r0 + rc + 1, :])
                if k == K - 1:
                    nc.gpsimd.memset(s3[64:128, rc + 1:rc + 2], 0.0)
                    nc.sync.dma_start(out=s3[64:128, 0:rc + 1],
                                      in_=xg[:, HH + r0 - 1:H, :])
                else:
                    nc.sync.dma_start(
                        out=s3[64:128],
                        in_=xg[:, HH + r0 - 1:HH + r0 + rc + 1, :])
                x16 = x16_pool.tile([128, (RM + 2) * WP], f16, name="x16",
                                    tag="x16")
                X = x16[:, 0:(rc + 2) * WP].rearrange("p (r w) -> p r w",
                                                      w=WP)
                nc.gpsimd.memset(X[:, :, W:WP], 0.0)
                if k == 0:
                    # ramp: cast on the idle DVE, split by partition half
                    # so the top cast overlaps the bottom half's DMA
                    nc.vector.tensor_copy(out=X[0:64, :, 0:W],
                                          in_=s3[0:64])
                    nc.vector.tensor_copy(out=X[64:128, :, 0:W],
                                          in_=s3[64:128])
                else:
                    nc.scalar.copy(out=X[:, :, 0:W], in_=s3[:])
                return x16

            def _ap(view, dims):
                """Hand-built access pattern over `view`'s tile (keeps the
                partition dim and base offset)."""
                w = view.copy()
                p0 = list(view.ap)[0]
                w.ap = mybir.VecI64Pair(
                    [[int(p0[0]), int(p0[1])]]
                    + [[int(s), int(c)] for s, c in dims])
                return w

            def stage_b(k, x16):
                rc = sizes[k]
                Rh = rc // 2
                N2 = rc * WP
                X = x16[:, 0:(rc + 2) * WP].rearrange("p (r w) -> p r w",
                                                      w=WP)
                Pm, PM = (
                    t[:, 0:Rh * WP].rearrange("p (r w) -> p r w", w=WP)
                    for t in (Pm_t, PM_t))
                nc.vector.tensor_tensor(out=Pm, in0=X[:, 1:rc + 1:2],
                                        in1=X[:, 2:rc + 2:2], op=MIN)
                nc.vector.tensor_tensor(out=PM, in0=X[:, 1:rc + 1:2],
                                        in1=X[:, 2:rc + 2:2], op=MAX)
                # merged even/odd completions: one op per output array.
                # iteration order (pair i, parity g, col w); out rows are
                # consecutive; the X operand taps rows (2i, 2i+3); Pm/PM
                # broadcast over the parity dim (0-stride).
                grp = [[2 * WP, Rh], [3 * WP, 2], [1, WP]]
                bcast = [[WP, Rh], [0, 2], [1, WP]]
                rows = [[2 * WP, Rh], [WP, 2], [1, WP]]
                a_g = _ap(x16[:, 0:(rc + 2) * WP], grp)
                lo_o = _ap(Lo_t[:, 2:2 + N2], rows)
                hi_o = _ap(Hi_t[:, 2:2 + N2], rows)
                me_o = _ap(Me_t[:, 2:2 + N2], rows)
                pm_b = _ap(Pm_t[:, 0:Rh * WP], bcast)
                pM_b = _ap(PM_t[:, 0:Rh * WP], bcast)
                tEO = _ap(tE_t[:, 0:N2], rows)
                nc.vector.tensor_tensor(out=lo_o, in0=a_g, in1=pm_b, op=MIN)
                nc.vector.tensor_tensor(out=hi_o, in0=a_g, in1=pM_b, op=MAX)
                nc.vector.tensor_tensor(out=tEO, in0=a_g, in1=pM_b, op=MIN)
                nc.vector.tensor_tensor(out=me_o, in0=pm_b, in1=tEO,
                                        op=MAX)

                mA_t = m_pool.tile([128, LSZ], f16, name="mA", tag="mA")
                mB_t = m_pool.tile([128, LSZ], f16, name="mB", tag="mB")
                mC_t = m_pool.tile([128, LSZ], f16, name="mC", tag="mC")
                for opname, src, dst in (
                    ("HMAX3W_ANT", Lo_t, mA_t),
                    ("HMED3W_ANT", Me_t, mB_t),
                    ("HMIN3W_ANT", Hi_t, mC_t),
                ):
                    _emit_custom_dve(
                        nc.vector, ops[opname], out=dst[:, 0:N2],
                        in0=src[:, 0:N2], in1=src[:, 2:N2 + 2], perf_max=1)

                mT = f_pool.tile([128, RM * WP], f16, name="mT", tag="mT")
                mU = f_pool.tile([128, RM * WP], f16, name="mU", tag="mU")
                # mB is dead once mT/mU are done; reuse its buffer (all
                # ops on the DVE, in-order, so aliasing is dependency-free)
                mV = m_pool.tile([128, LSZ], f16, name="mV", tag="mB")
                ot = ot_pool.tile([128, RM * WP + 2], f16, name="ot",
                                  tag="ot")
                sl = slice(0, N2)
                nc.vector.tensor_tensor(out=mT[:, sl], in0=mA_t[:, sl],
                                        in1=mB_t[:, sl], op=MIN)
                nc.vector.tensor_tensor(out=mU[:, sl], in0=mA_t[:, sl],
                                        in1=mB_t[:, sl], op=MAX)
                nc.vector.tensor_tensor(out=mV[:, sl], in0=mU[:, sl],
                                        in1=mC_t[:, sl], op=MIN)
                nc.vector.tensor_tensor(out=ot[:, sl], in0=mT[:, sl],
                                        in1=mV[:, sl], op=MAX)
                return ot

            def stage_d(k, ot):
                rc = sizes[k]
                r0 = r0s[k]
                o32 = o32_pool.tile([128, RM * W], f32, name="o32",
                                    tag="o32")
                o3 = o32[:, 0:rc * W].rearrange("p (r w) -> p r w", w=W)
                ot3 = ot[:, 1:1 + rc * WP].rearrange("p (r w) -> p r w",
                                                     w=WP)
                if k == K - 1:
                    # tail: cast on the now-idle DVE (no ACT handoff) and
                    # pipeline the stores in row halves
                    h = rc // 2
                    nc.vector.tensor_copy(out=o3[:, 0:h],
                                          in_=ot3[:, 0:h, 0:W])
                    nc.scalar.dma_start(out=og[:, r0:r0 + h, :],
                                        in_=o3[0:64, 0:h])
                    nc.scalar.dma_start(
                        out=og[:, HH + r0:HH + r0 + h, :],
                        in_=o3[64:128, 0:h])
                    nc.vector.tensor_copy(out=o3[:, h:rc],
                                          in_=ot3[:, h:rc, 0:W])
                    nc.scalar.dma_start(out=og[:, r0 + h:r0 + rc, :],
                                        in_=o3[0:64, h:rc])
                    nc.scalar.dma_start(
                        out=og[:, HH + r0 + h:HH + r0 + rc, :],
                        in_=o3[64:128, h:rc])
                    return
                nc.scalar.copy(out=o3[:], in_=ot3[:, :, 0:W])
                nc.scalar.dma_start(out=og[:, r0:r0 + rc, :], in_=o3[0:64])
                nc.scalar.dma_start(out=og[:, HH + r0:HH + r0 + rc, :],
                                    in_=o3[64:128])

            Xs = {0: stage_a(0)}
            for k in range(K):
                if k + 1 < K:
                    Xs[k + 1] = stage_a(k + 1)
                ot = stage_b(k, Xs.pop(k))
                stage_d(k, ot)

    nc.compile()
    return nc


def _build_shared(R=16, gp_frac=0.0, dtype="float32", in_bufs=None,
                  out_bufs=None):
    """Fallback: 15-op/pixel fp32 TT network (previous working kernel)."""
    import concourse.bacc as bacc
    import concourse.mybir as mybir
    from concourse.tile import TileContext

    MIN = mybir.AluOpType.min
    MAX = mybir.AluOpType.max
    f32 = mybir.dt.float32
    cdt = getattr(mybir.dt, dtype)

    WPP = W + 2
    assert HH % R == 0 and R % 2 == 0
    K = HH // R
    Rh = R // 2

    nc = bacc.Bacc("TRN2", name="median_pool2d_s")
    x = nc.dram_tensor("x", [C, H, W], f32, kind="ExternalInput")
    out = nc.dram_tensor("out", [C, H, W], f32, kind="ExternalOutput")
    xg = x.ap()
    og = out.ap()

    def tt(out_ap, in0, in1, op):
        nc.vector.tensor_tensor(out=out_ap, in0=in0, in1=in1, op=op)

    if in_bufs is None:
        in_bufs = 3 if R <= 8 else 2
    if out_bufs is None:
        out_bufs = 3 if R <= 8 else 1
    with TileContext(nc) as tc:
        with (
            tc.tile_pool(name="io_in", bufs=in_bufs) as in_pool,
            tc.tile_pool(name="io_out", bufs=out_bufs) as out_pool,
            tc.tile_pool(name="work", bufs=1) as w_pool,
        ):
            def wtile(name, rows, width, tag=None):
                t = w_pool.tile([128, rows * width], cdt, name=name,
                                tag=tag or name)
                return t.rearrange("p (r w) -> p r w", w=width)

            for k in range(K):
                r0 = k * R
                it = in_pool.tile([128, (R + 2) * WPP], cdt, name="it",
                                  tag="it")
                it3 = it.rearrange("p (r w) -> p r w", w=WPP)
                nc.vector.memset(it3[:, :, 0:WPP:WPP - 1], 0.0)
                if k == 0:
                    nc.vector.memset(it3[0:64, 0:1, 1:W + 1], 0.0)
                    nc.sync.dma_start(out=it3[0:64, 1:R + 2, 1:W + 1],
                                      in_=xg[:, 0:R + 1, :])
                else:
                    nc.sync.dma_start(out=it3[0:64, :, 1:W + 1],
                                      in_=xg[:, r0 - 1:r0 + R + 1, :])
                if k == K - 1:
                    nc.vector.memset(it3[64:128, R + 1:R + 2, 1:W + 1], 0.0)
                    nc.sync.dma_start(out=it3[64:128, 0:R + 1, 1:W + 1],
                                      in_=xg[:, HH + r0 - 1:H, :])
                else:
                    nc.sync.dma_start(out=it3[64:128, :, 1:W + 1],
                                      in_=xg[:, HH + r0 - 1:HH + r0 + R + 1, :])

                Pm = wtile("Pm", Rh, WPP)
                PM = wtile("PM", Rh, WPP)
                tt(Pm, it3[:, 1:R + 1:2, :], it3[:, 2:R + 2:2, :], MIN)
                tt(PM, it3[:, 1:R + 1:2, :], it3[:, 2:R + 2:2, :], MAX)

                Lo3 = wtile("Lo", R, WPP)
                Me3 = wtile("Me", R, WPP)
                Hi3 = wtile("Hi", R, WPP)
                tE = wtile("tE", Rh, WPP)
                tO = wtile("tO", Rh, WPP)
                a_e = it3[:, 0:R:2, :]
                a_o = it3[:, 3:R + 2:2, :]
                tt(Lo3[:, 0:R:2], a_e, Pm, MIN)
                tt(Hi3[:, 0:R:2], a_e, PM, MAX)
                tt(tE, a_e, PM, MIN)
                tt(Me3[:, 0:R:2], Pm, tE, MAX)
                tt(Lo3[:, 1:R:2], a_o, Pm, MIN)
                tt(Hi3[:, 1:R:2], a_o, PM, MAX)
                tt(tO, a_o, PM, MIN)
                tt(Me3[:, 1:R:2], Pm, tO, MAX)

                NP = W // 2 + 1
                PA = wtile("PA", R, NP, tag="Pm")
                PC = wtile("PC", R, NP, tag="PM")
                Um = wtile("Um", R, NP, tag="tE")
                Vm = wtile("Vm", R, NP, tag="tO")
                tBe = wtile("tBe", R, W // 2, tag="Pm")
                tBo = wtile("tBo", R, W // 2, tag="PM")
                mA = wtile("mA", R, W)
                mB = wtile("mB", R, W)
                mC = wtile("mC", R, W)

                ev = slice(0, WPP, 2)
                od = slice(1, WPP, 2)
                tt(PA, Lo3[:, :, ev], Lo3[:, :, od], MAX)
                tt(mA[:, :, 0:W:2], PA[:, :, 0:NP - 1], Lo3[:, :, 2:WPP:2],
                   MAX)
                tt(mA[:, :, 1:W:2], PA[:, :, 1:NP], Lo3[:, :, 1:WPP - 2:2],
                   MAX)

                tt(PC, Hi3[:, :, ev], Hi3[:, :, od], MIN)
                tt(mC[:, :, 0:W:2], PC[:, :, 0:NP - 1], Hi3[:, :, 2:WPP:2],
                   MIN)
                tt(mC[:, :, 1:W:2], PC[:, :, 1:NP], Hi3[:, :, 1:WPP - 2:2],
                   MIN)

                tt(Um, Me3[:, :, ev], Me3[:, :, od], MIN)
                tt(Vm, Me3[:, :, ev], Me3[:, :, od], MAX)
                tt(tBe, Me3[:, :, 2:WPP:2], Vm[:, :, 0:NP - 1], MIN)
                tt(mB[:, :, 0:W:2], Um[:, :, 0:NP - 1], tBe, MAX)
                tt(tBo, Me3[:, :, 1:WPP - 2:2], Vm[:, :, 1:NP], MIN)
                tt(mB[:, :, 1:W:2], Um[:, :, 1:NP], tBo, MAX)

                mT = wtile("mT", R, W, tag="Lo")
                mU = wtile("mU", R, W, tag="Me")
                mV = wtile("mV", R, W, tag="Hi")
                ot = out_pool.tile([128, R * W], cdt, name="ot", tag="ot")
                ot3 = ot.rearrange("p (r w) -> p r w", w=W)
                tt(mT, mA, mB, MIN)
                tt(mU, mA, mB, MAX)
                tt(mV, mU, mC, MIN)
                tt(ot3, mT, mV, MAX)

                nc.sync.dma_start(out=og[:, r0:r0 + R, :], in_=ot3[0:64])
                nc.sync.dma_start(out=og[:, HH + r0:HH + r0 + R, :],
                                  in_=ot3[64:128])

    nc.compile()
    return nc


def _get_nc(variant="v3", **kw):
    key = (variant, tuple(sorted(kw.items())))
    if key not in _CACHE:
        if variant == "v4":
            _CACHE[key] = _build_v4(**kw)
        elif variant == "v3":
            _CACHE[key] = _build_v3(**kw)
        else:
            _CACHE[key] = _build_shared(**kw)
    return _CACHE[key]


_LAST_NC = None


def kernel(x: np.ndarray) -> np.ndarray:
    """MedianPool2d(3x3, s=1, p=1) on 8 NeuronCores."""
    global _LAST_NC
    from concourse.bass_utils import run_bass_kernel_spmd

    assert x.shape == (B, C, H, W), x.shape
    x = np.ascontiguousarray(x, dtype=np.float32)
    try:
        nc = _get_nc("v4")
    except Exception:
        try:
            nc = _get_nc("v3")
        except Exception:
            nc = _get_nc("shared", R=16)
    _LAST_NC = nc
    in_maps = [{"x": x[i]} for i in range(NCORES)]
    res = run_bass_kernel_spmd(nc, in_maps, core_ids=list(range(NCORES)))
    return np.stack([r["out"] for r in res.results], axis=0)
